# revision 62
# baseline (speedup 1.0000x reference)
"""Decoder block on 8 TRN2 NeuronCores — pipelined bf16 version.

Sharding: core c -> (batch b=c//2, half h=c%2): 512 query rows through the
full decoder; K/V span the full T=1024 of that batch element. All activations
feature-major [C, rows]; matmuls run as out = lhsT.T @ rhs in bf16 with fp32
PSUM accumulation.

Schedule: every engine's issue order is pinned to emission order; emission is
software-pipelined so the PE never waits: attention runs S(tk) / exp(tk-1) /
O(tk-2) with the next head-pair's K-projection matmuls as PE filler under the
ACT exp stream; weights are host-repacked to bf16 in DMA-contiguous layouts
and streamed >=1MB per transfer well ahead of use.
"""

import numpy as np
import ml_dtypes

import concourse.bass as bass
from bass_rust import add_dep_helper
import concourse.mybir as mybir
import concourse.tile as tile
from concourse import bacc
from concourse.bass_utils import run_bass_kernel_spmd

B, T, C, H = 4, 1024, 1024, 16
HD = C // H            # 64
DFF = 4096
EPS = 1e-5
P = 128
R = 512                # query rows per core
FT = C // P            # 8 feature ptiles
RT = T // P            # 8 key-row tiles
NCORES = 8

F32 = mybir.dt.float32
BF16 = mybir.dt.bfloat16
AF = mybir.ActivationFunctionType
NPBF = ml_dtypes.bfloat16

_CACHE = {}


def _emit(nc, tc, d, flags):
    sync = nc.sync
    ve = nc.vector
    se = nc.scalar
    te = nc.tensor
    ge = nc.gpsimd

    # Pin every engine's issue order to emission order (the emitted program
    # is a feasible sequential order by construction; this stops the Tile
    # scheduler from hoisting an instruction onto a busy slot).
    _last = {}

    def _chain(key, inst):
        prev = _last.get(key)
        if prev is not None:
            add_dep_helper(inst.ins, prev.ins, sync=False,
                           reason=f"{key} emission-order chain")
        _last[key] = inst
        return inst

    def dma(out, in_):
        return _chain("sp", sync.dma_start(out=out, in_=in_))

    def gdma(out, in_):
        return _chain("pool", ge.dma_start(out=out, in_=in_))

    class _Chained:
        def __init__(self, eng, key):
            self._eng = eng
            self._key = key

        def __getattr__(self, name):
            fn = getattr(self._eng, name)

            def wrapped(*a, **k):
                return _chain(self._key, fn(*a, **k))

            return wrapped

    ve = _Chained(ve, "dve")
    se = _Chained(se, "act")
    te = _Chained(te, "pe")

    sb = tc.alloc_tile_pool(name="sb", bufs=1)
    ps = tc.alloc_tile_pool(name="ps", bufs=1, space="PSUM")

    # ---- constants ----
    ones_t = sb.tile([P, R], BF16, tag="ones", bufs=1, name="ones_t")
    dma(out=ones_t[:], in_=d["ones"][:, :])
    ones_f = sb.tile([1, P], mybir.dt.float32r, tag="onesf", bufs=1,
                     name="ones_f")
    dma(out=ones_f[:], in_=d["ones_f"][:, :])
    eps_t = sb.tile([1, 1], F32, tag="eps", bufs=1, name="eps_t")
    ve.memset(eps_t[:], EPS)
    invc_t = sb.tile([1, 1], F32, tag="eps2", bufs=1, name="invc_t")
    ve.memset(invc_t[:], 1.0 / C)

    # ---- tile makers -------------------------------------------------
    def act8():
        # activation sets [P, FT, R]; FIFO reuse order must respect liveness
        return sb.tile([P, FT, R], BF16, tag="act8", bufs=5, name="act8")

    def big_tile(shape, name):
        return sb.tile(shape, BF16, tag="big", bufs=2, name=name)

    def wfm_tile(shape, name):
        return sb.tile(shape, BF16, tag="wfm", bufs=3, name=name)

    def wrm_tile(shape, name):
        return sb.tile(shape, BF16, tag="wrm", bufs=2, name=name)

    def mm_ps():
        return ps.tile([P, R], F32, tag="mm", bufs=2, name="mm")

    def stat_s(dt=F32):
        # short-lived row stats (dead before the next three allocs of this tag)
        return sb.tile([1, R], dt, tag="statS", bufs=4, name="statS")

    def stat_l(dt=F32):
        # long-lived row stats (mu / rstd / mu*rstd within one layernorm)
        return sb.tile([1, R], dt, tag="statL", bufs=3, name="statL")

    # ---- optional-bias helpers (zero in the graded configuration) ----
    def bias_fm(psm, bias_ap, mt, n):
        """psm[P, n] += b[mt*P : mt*P+P] outer ones_n (feature-major)."""
        b_f = sb.tile([1, P], F32, tag="biaf", bufs=2, name="b_fmf")
        dma(out=b_f[:], in_=bias_ap[mt * P:(mt + 1) * P][None, :])
        b_t = sb.tile([1, P], BF16, tag="bia", bufs=2, name="b_fm")
        ve.tensor_copy(b_t[:], b_f[:])
        te.matmul(psm[:, 0:n], b_t[:], ones_t[0:1, 0:n], start=False, stop=True)

    def bias_rm(psm, bias_ap, cc):
        """psm[P, 512] += ones_col outer b[cc*512 : cc*512+512] (row-major)."""
        b_f = sb.tile([1, R], F32, tag="biaf", bufs=2, name="b_rmf")
        dma(out=b_f[:], in_=bias_ap[cc * R:(cc + 1) * R][None, :])
        b_t = sb.tile([1, R], BF16, tag="bia", bufs=2, name="b_rm")
        ve.tensor_copy(b_t[:], b_f[:])
        te.matmul(psm[:], ones_t[0:1, 0:P], b_t[:], start=False, stop=True)

    # ---- generic fm linear: one output ptile ------------------------
    def fm_group(w_tile, mt, rhs_slices, bias_ap=None, bias_mt=None):
        """psum[P, 512] = sum_kt w_tile[:, mt, kt, :].T @ rhs_slices[kt]."""
        psm = mm_ps()
        last = len(rhs_slices) - 1
        for kt, rhs in enumerate(rhs_slices):
            te.matmul(psm[:], w_tile[:, mt, kt, :], rhs,
                      start=(kt == 0), stop=(kt == last and bias_ap is None))
        if bias_ap is not None:
            bias_fm(psm, bias_ap, bias_mt if bias_mt is not None else mt, R)
        return psm

    # ================= layernorm (stats + normalize) =================
    # Stats are accumulated tile-by-tile as inputs appear (x2 mul + two
    # ones-matmuls); normalize uses PE broadcast of rstd / mu*rstd.
    class LNState:
        pass

    def ln_begin():
        st = LNState()
        st.sx = ps.tile([1, R], F32, tag="o", bufs=2, name="ln_sx")
        st.sx2 = ps.tile([1, R], F32, tag="o", bufs=2, name="ln_sx2")
        return st

    def ln_accum(st, x_sl, kt):
        x2 = sb.tile([P, R], BF16, tag="x2", bufs=2, name="x2")
        ve.tensor_mul(x2[:], x_sl, x_sl)
        te.matmul(st.sx[:], ones_t[:, 0:1], x_sl,
                  start=(kt == 0), stop=(kt == FT - 1))
        te.matmul(st.sx2[:], ones_t[:, 0:1], x2[:],
                  start=(kt == 0), stop=(kt == FT - 1))

    def ln_stats(st):
        # mu is NEGATED so the apply step is x*rstd_b + (-mu*rstd)_b — a
        # commutative add with the PSUM operand in the proven first slot
        mu = stat_l()
        ve.tensor_scalar_mul(mu[:], st.sx[:], -1.0 / C)
        ex2 = stat_s()
        ve.tensor_scalar_mul(ex2[:], st.sx2[:], 1.0 / C)
        mu2 = stat_s()
        ve.tensor_mul(mu2[:], mu[:], mu[:])
        var = stat_s()
        ve.tensor_sub(var[:], ex2[:], mu2[:])
        # rstd = exp(-0.5 * ln(var + eps)) — keeps ACT on the ln/exp table set
        lnv = stat_s()
        se.activation(out=lnv[:], in_=var[:], func=AF.Ln,
                      bias=eps_t[0:1, 0:1], scale=1.0)
        rstd = stat_l(BF16)
        se.activation(out=rstd[:], in_=lnv[:], func=AF.Exp, scale=-0.5)
        # broadcast via PE into [P, R] (uses the idle "st" PSUM space);
        # applies read these straight from PSUM (PSUM-first operand)
        rb = ps.tile([P, R], F32, tag="st", bufs=2, name="ln_rb")
        te.matmul(rb[:], ones_t[0:1, 0:P], rstd[:], start=True, stop=True)
        musr = stat_l(BF16)
        ve.tensor_mul(musr[:], mu[:], rstd[:])
        mb = ps.tile([P, R], F32, tag="st", bufs=2, name="ln_mb")
        te.matmul(mb[:], ones_t[0:1, 0:P], musr[:], start=True, stop=True)
        st.rb, st.mb = rb, mb

    def ln_apply(st, x_sl, out_sl, w_ap, b_ap, kt, out_dt=BF16):
        # broadcasts read straight from PSUM (PSUM-first operand order)
        tmp = sb.tile([P, R], BF16, tag="lntmp", bufs=2, name="lntmp")
        ve.tensor_mul(tmp[:], st.rb[:], x_sl)
        if w_ap is None and b_ap is None:
            ve.tensor_add(out_sl, st.mb[:], tmp[:])
        else:
            nrm = sb.tile([P, R], BF16, tag="lntmp", bufs=2, name="lnnrm")
            ve.tensor_add(nrm[:], st.mb[:], tmp[:])
            w_t = sb.tile([P, 1], F32, tag="lnw", bufs=4, name="lnw")
            if w_ap is not None:
                dma(out=w_t[:], in_=w_ap[kt * P:(kt + 1) * P][:, None])
            else:
                ve.memset(w_t[:], 1.0)
            b_t = sb.tile([P, 1], F32, tag="lnw", bufs=4, name="lnb")
            if b_ap is not None:
                dma(out=b_t[:], in_=b_ap[kt * P:(kt + 1) * P][:, None])
            else:
                ve.memset(b_t[:], 0.0)
            ve.scalar_tensor_tensor(
                out_sl, nrm[:], w_t[:], b_t[:].to_broadcast((P, R)),
                op0=mybir.AluOpType.mult, op1=mybir.AluOpType.add)

    # =================================================================
    # Prologue DMAs (sp ring, in order of first use)
    # =================================================================
    # q-gen inputs first so the PE can start ~6us in (Wq arrives per-mt,
    # group 0 only needs the first slice); the rest stream underneath the
    # q/v projection work
    yq_all = act8()                           # slot 0
    dma(out=yq_all[:], in_=d["yq"].rearrange("(kt p) r -> p kt r", p=P))
    wq_t = wfm_tile([P, FT, FT, P], "wq")     # wfm slot 0
    for mt in range(FT):
        dma(out=wq_t[:, mt, :, :], in_=d["Wq_attn"][:, mt, :, :])
    ykv_all = big_tile([P, FT, T], "ykv")     # big slot 0
    dma(out=ykv_all[:], in_=d["ykv"].rearrange("(kt p) r -> p kt r", p=P))
    wv_t = wrm_tile([P, FT, T], "wv")         # wrm slot 0
    dma(out=wv_t[:], in_=d["Wv_attn"][:])
    wk_t = wfm_tile([P, FT, FT, P], "wk")     # wfm slot 1
    dma(out=wk_t[:], in_=d["Wk_attn"][:])

    b_attn = d.get("b_attn")

    # ================= A: q generation (feature-major) ===============
    q_all = act8()                            # slot 1
    for mt in range(FT):
        psm = fm_group(wq_t, mt, [yq_all[:, kt, :] for kt in range(FT)],
                       bias_ap=b_attn, bias_mt=mt)
        ve.tensor_copy(q_all[:, mt, :], psm[:])

    # more prefetch: cross-attn inputs + proj weights
    xkv_all = big_tile([P, FT, T], "xkv")     # big slot 1
    dma(out=xkv_all[:], in_=d["xkv"].rearrange("(kt p) r -> p kt r", p=P))
    wven_t = wrm_tile([P, FT, T], "wven")     # wrm slot 1
    dma(out=wven_t[:], in_=d["Wv_en"][:])
    wproj_t = wfm_tile([P, FT, FT, P], "wproj")   # wfm slot 2
    dma(out=wproj_t[:], in_=d["W_proj_p"][:])

    # ================= B: v generation (row-major, + ones col) =======
    def gen_v(src_all, w_rm, v_tiles, bias_ap, pfx):
        for cc in range(2):
            for rt in range(RT):
                psm = mm_ps()
                for kt in range(FT):
                    te.matmul(psm[:],
                              src_all[:, kt, rt * P:(rt + 1) * P],
                              w_rm[:, kt, cc * R:(cc + 1) * R],
                              start=(kt == 0),
                              stop=(kt == FT - 1 and bias_ap is None))
                if bias_ap is not None:
                    bias_rm(psm, bias_ap, cc)
                if cc == 0:
                    v_tiles[rt] = sb.tile([P, H, HD + 1], BF16, tag="v65",
                                          bufs=RT, name=f"{pfx}{rt}")
                data = ve.tensor_copy(
                    v_tiles[rt][:, cc * 8:(cc + 1) * 8, 0:HD],
                    psm[:].rearrange("p (h d) -> p h d", d=HD))
                if cc == 0:
                    oc = ve.tensor_copy(v_tiles[rt][:, :, HD], ones_t[:, 0:H])
                    add_dep_helper(oc.ins, data.ins, sync=False,
                                   reason="ones col after v data (slot order)")

    v_sb = [None] * RT
    gen_v(ykv_all, wv_t,  v_sb,
          (d["b_attn"][2 * C:3 * C] if "b_attn" in d else None), "v")

    # ================= K-projection helper ===========================
    # k ptile hp covers heads (2hp, 2hp+1): features on partitions, T keys
    # free.  Emitted in 18 closures (16 matmuls + 2 evicts) so attention can
    # interleave them two-per-unit as PE filler under the exp stream.
    def k_steps(w_t, src_all, hp, bias_ap, box, pfx):
        steps = []
        psms = {}

        def mk_mm(cc, kt):
            def go():
                if kt == 0:
                    psms[cc] = mm_ps()
                te.matmul(psms[cc][:], w_t[:, hp, kt, :],
                          src_all[:, kt, cc * R:(cc + 1) * R],
                          start=(kt == 0),
                          stop=(kt == FT - 1 and bias_ap is None))
            return go

        def mk_ev(cc):
            def go():
                if bias_ap is not None:
                    bias_fm(psms[cc], bias_ap, hp, R)
                if cc == 0:
                    box[0] = sb.tile([P, T], BF16, tag="ksb", bufs=2,
                                     name=f"{pfx}{hp}")
                ve.tensor_copy(box[0][:, cc * R:(cc + 1) * R], psms[cc][:])
            return go

        for cc in range(2):
            for kt in range(FT):
                steps.append(mk_mm(cc, kt))
            steps.append(mk_ev(cc))
        return steps

    # ================= attention =====================================
    def attention(q_all_t, k_w, k_src, k_bias, v_tiles, o_all_t, pfx,
                  tail_filler):
        """tail_filler: list of closures run as PE filler during the last
        head-pair (which has no next-K to generate)."""

        def norm_head(o_ps):
            # DVE half of softmax normalization, emitted right after the
            # final O matmuls: reciprocal straight off the PSUM ones-row,
            # and the o halves staged into SBUF (so the later mul's PSUM
            # operand can be the PE-broadcast reciprocal instead)
            # ordering matched to each consumer's deadline: per-s den+oc
            # pairs free that o_ps slot for the next head-pair's O matmul
            # (unit 2); the reciprocal chains follow, landing just before
            # their rb matmuls (unit 3)
            rcp, ocp, dens = [], [], []
            for s in range(2):
                den = stat_s()
                ve.tensor_copy(den[:], o_ps[s][HD:HD + 1, :])
                dens.append(den)
                oc = sb.tile([HD, R], BF16, tag="rbs", bufs=2,
                             name=f"{pfx}ocp")
                ve.tensor_copy(oc[:], o_ps[s][0:HD, :])
                ocp.append(oc)
            for s in range(2):
                rc = stat_s()
                ve.reciprocal_approx_fast(rc[:], dens[s][:])
                rcb = stat_s(BF16)
                ve.tensor_copy(rcb[:], rc[:])
                rcp.append(rcb)
            return rcp, ocp

        def norm_tail(hp, rcp, ocp):
            # PE broadcast + final scale, hidden under the next head-pair's
            # S matmuls
            for s in range(2):
                rb = ps.tile([P, R], F32, tag="mm", bufs=2, name=f"{pfx}rb")
                te.matmul(rb[0:HD, :], ones_t[0:1, 0:HD], rcp[s][:],
                          start=True, stop=True)
                if s == 0:
                    ve.tensor_mul(o_all_t[0:HD, hp, :], rb[0:HD, :],
                                  ocp[s][:])
                else:
                    tmp = sb.tile([HD, R], BF16, tag="otmp", bufs=2,
                                  name=f"{pfx}otmp")
                    ve.tensor_mul(tmp[:], rb[0:HD, :], ocp[s][:])
                    gdma(out=o_all_t[HD:P, hp, :], in_=tmp[:])

        kbox = [None]
        for s0 in k_steps(k_w, k_src, 0, k_bias, kbox, pfx + "k"):
            s0()
        k_cur = kbox[0]
        pending = None          # (hp, o_ps) awaiting normalization
        for hp in range(FT):
            # next head-pair's K projection as filler
            if hp + 1 < FT:
                nbox = [None]
                filler = k_steps(k_w, k_src, hp + 1, k_bias, nbox, pfx + "k")
            else:
                nbox = None
                filler = list(tail_filler)
            o_ps = [None, None]
            es_q = [None] * RT
            fi = 0
            for u in range(RT + 3):
                if u < RT:
                    st_t = ps.tile([P, 2 * R], F32, tag="st", bufs=2,
                                   name=f"{pfx}st")
                    for s in range(2):
                        off = HD * s
                        te.matmul(st_t[:, s * R:(s + 1) * R],
                                  k_cur[off:off + HD, u * P:(u + 1) * P],
                                  q_all_t[off:off + HD, hp, :],
                                  start=True, stop=True)
                    es = sb.tile([P, 2 * R], BF16, tag="es", bufs=4,
                                 name=f"{pfx}es")
                    se.activation(out=es[:], in_=st_t[:], func=AF.Exp,
                                  scale=0.125)
                    es_q[u] = es
                if u >= 3:
                    tko = u - 3
                    if tko == 0:
                        o_ps[0] = ps.tile([HD + 1, R], F32, tag="o", bufs=2,
                                          name=f"{pfx}o0")
                        o_ps[1] = ps.tile([HD + 1, R], F32, tag="o", bufs=2,
                                          name=f"{pfx}o1")
                    for s in range(2):
                        te.matmul(o_ps[s][:],
                                  v_tiles[tko][:, 2 * hp + s, :],
                                  es_q[tko][:, s * R:(s + 1) * R],
                                  start=(tko == 0), stop=(tko == RT - 1))
                # filler pacing: one pop at u4/u9 so the K psum slot (tag
                # "mm") recycles cleanly around norm_tail's rb matmuls —
                # the cc0 evict lands at u4 just before rb, and the cc1
                # psum allocates at u5 just after the rb/mul pair
                for _ in range(1 if u in (4, RT + 1) else 2):
                    if fi < len(filler):
                        filler[fi]()
                        fi += 1
                if u == 4 and pending is not None:
                    # norm_tail here gets ~5 units of PE cover for the
                    # reciprocal chain emitted at the last head-pair's end
                    norm_tail(*pending)
                    pending = None
            while fi < len(filler):
                filler[fi]()
                fi += 1
            rcp, ocp = norm_head(o_ps)
            pending = (hp, rcp, ocp)
            if nbox is not None:
                k_cur = nbox[0]
        norm_tail(*pending)

    o_all = act8()                            # slot 2
    attention(q_all, wk_t, ykv_all,
              (d["b_attn"][C:2 * C] if "b_attn" in d else None),
              v_sb, o_all, "sa", [])

    # ================= proj + residual, LN1 stats interleaved ========
    # ln_accum lags its producer by one tile so the sums matmuls hide
    # under the next output tile's matmul group instead of stalling on
    # the 3-op DVE eviction chain
    y1_all = act8()                           # slot 3
    ln1 = ln_begin()
    for mt in range(FT):
        psm = fm_group(wproj_t, mt, [o_all[:, kt, :] for kt in range(FT)],
                       bias_ap=d.get("b_proj"), bias_mt=mt)
        ve.tensor_add(y1_all[:, mt, :], psm[:], yq_all[:, mt, :])
        if mt > 0:
            ln_accum(ln1, y1_all[:, mt - 1, :], mt - 1)
    ln_accum(ln1, y1_all[:, FT - 1, :], FT - 1)

    # prefetch cross-attn K weights (wfm slot 0: wq dead) and the q2/cproj
    # weights (slots 1/2: wk and wproj are dead once proj is emitted), so
    # the LN1 seam and cross-attn aren't gated on weight DMAs
    wken_t = wfm_tile([P, FT, FT, P], "wken")
    dma(out=wken_t[:], in_=d["Wk_en"][:])
    wq2_t = wfm_tile([P, FT, FT, P], "wq2")       # wfm slot 1 (wk dead)
    dma(out=wq2_t[:], in_=d["W_q_p"][:])
    wcproj_t = wfm_tile([P, FT, FT, P], "wcp")    # wfm slot 2 (wproj dead)
    dma(out=wcproj_t[:], in_=d["W_cproj_p"][:])

    # ================= v2 generation (PE) overlapping LN1 (DVE) ======
    v2_sb = [None] * RT
    gen_v(xkv_all, wven_t, v2_sb,
          (d["b_en"][C:2 * C] if "b_en" in d else None), "w")

    ln_stats(ln1)
    y1n_all = act8()                          # slot 4
    for mt in range(FT):
        ln_apply(ln1, y1_all[:, mt, :], y1n_all[:, mt, :],
                 d.get("ln_w"), d.get("ln_b"), mt)

    # ================= q2 generation =================================
    q2_all = act8()                           # slot 5
    for mt in range(FT):
        psm = fm_group(wq2_t, mt, [y1n_all[:, kt, :] for kt in range(FT)],
                       bias_ap=d.get("b_q"), bias_mt=mt)
        ve.tensor_copy(q2_all[:, mt, :], psm[:])

    # prefetch first two FFN d1 weight chunks under cross-attention
    wd1_t = [None] * 4
    for ch in range(2):
        wd1_t[ch] = wrm_tile([P, FT, FT, P], f"wd1_{ch}")
        dma(out=wd1_t[ch][:],
            in_=d["W_d1_p"][:, ch * FT:(ch + 1) * FT, :, :])

    # ================= cross attention ===============================
    o2_all = act8()                           # slot 0 (yq dead)
    attention(q2_all, wken_t, xkv_all,
              (d["b_en"][0:C] if "b_en" in d else None),
              v2_sb, o2_all, "ca", [])

    # ================= cproj + residual (into y1n), LN2 ==============
    ln2 = ln_begin()
    for mt in range(FT):
        psm = fm_group(wcproj_t, mt, [o2_all[:, kt, :] for kt in range(FT)],
                       bias_ap=d.get("b_cproj"), bias_mt=mt)
        ve.tensor_add(y1n_all[:, mt, :], psm[:], y1n_all[:, mt, :])
        if mt > 0:
            ln_accum(ln2, y1n_all[:, mt - 1, :], mt - 1)
    ln_accum(ln2, y1n_all[:, FT - 1, :], FT - 1)
    ln_stats(ln2)
    xin_all = act8()                          # slot 1 (q dead)
    for mt in range(FT):
        ln_apply(ln2, y1n_all[:, mt, :], xin_all[:, mt, :],
                 d.get("ln1_w"), d.get("ln1_b"), mt)

    # ================= FFN ===========================================
    # F1: h = xin @ W_d1 (feature-major, 32 output ptiles in 4 weight chunks)
    ht_a = big_tile([P, 16, R], "ht_a")       # big slot 0 (ykv dead)
    ht_b = big_tile([P, 16, R], "ht_b")       # big slot 1 (xkv dead)

    # first d2 weight tile prefetches under F1
    wd2_t = [None] * FT
    wd2_t[0] = wfm_tile([P, 32, P], "wd2_0")
    dma(out=wd2_t[0][:], in_=d["W_d2_p"][:, 0, :, :])

    def ht_sl(kt):
        return (ht_a if kt < 16 else ht_b)[:, kt % 16, :]

    for ch in range(4):
        if wd1_t[ch] is None:
            wd1_t[ch] = wrm_tile([P, FT, FT, P], f"wd1_{ch}")
            dma(out=wd1_t[ch][:],
                in_=d["W_d1_p"][:, ch * FT:(ch + 1) * FT, :, :])
        w1 = wd1_t[ch]
        # prefetch chunk ch+2 while computing on ch
        nxt = ch + 2
        if nxt < 4 and wd1_t[nxt] is None:
            wd1_t[nxt] = wrm_tile([P, FT, FT, P], f"wd1_{nxt}")
            dma(out=wd1_t[nxt][:],
                in_=d["W_d1_p"][:, nxt * FT:(nxt + 1) * FT, :, :])
        for kk in range(FT):
            kt = ch * FT + kk
            psm = mm_ps()
            for ck in range(FT):
                te.matmul(psm[:], w1[:, kk, ck, :], xin_all[:, ck, :],
                          start=(ck == 0),
                          stop=(ck == FT - 1 and "b_d1" not in d))
            if "b_d1" in d:
                bias_fm(psm, d["b_d1"], kt, R)
            ve.tensor_copy(ht_sl(kt), psm[:])

    # F2: z = h @ W_d2 + xin, LN3 stats interleaved
    ln3 = ln_begin()
    z_all = act8()                            # slot 2 (o dead)
    for mt in range(FT):
        if wd2_t[mt] is None:
            wd2_t[mt] = wfm_tile([P, 32, P], f"wd2_{mt}")
            dma(out=wd2_t[mt][:], in_=d["W_d2_p"][:, mt, :, :])
        w2 = wd2_t[mt]
        if mt + 1 < FT and wd2_t[mt + 1] is None:
            wd2_t[mt + 1] = wfm_tile([P, 32, P], f"wd2_{mt + 1}")
            dma(out=wd2_t[mt + 1][:], in_=d["W_d2_p"][:, mt + 1, :, :])
        psm = mm_ps()
        for kt in range(32):
            te.matmul(psm[:], w2[:, kt, :], ht_sl(kt),
                      start=(kt == 0), stop=(kt == 31 and "b_d2" not in d))
        if "b_d2" in d:
            bias_fm(psm, d["b_d2"], mt, R)
        ve.tensor_add(z_all[:, mt, :], psm[:], xin_all[:, mt, :])
        if mt > 0:
            ln_accum(ln3, z_all[:, mt - 1, :], mt - 1)
    ln_accum(ln3, z_all[:, FT - 1, :], FT - 1)
    ln_stats(ln3)

    for pair in range(FT // 2):
        ot = sb.tile([P, 2, R], BF16, tag="outt", bufs=2, name="out_t")
        for j in range(2):
            mt = 2 * pair + j
            ln_apply(ln3, z_all[:, mt, :], ot[:, j, :],
                     d.get("ln2_w"), d.get("ln2_b"), mt)
        dma(out=d["out"][2 * pair * P:(2 * pair + 2) * P, :]
            .rearrange("(two p) r -> p two r", p=P),
            in_=ot[:])

    sb.release()
    ps.release()


def _fm_pack(W, n_out):
    """[C, n_out] -> [P, mt, kt, P] bf16 so lhsT tiles are DMA-contiguous."""
    W = np.asarray(W, np.float32)
    kt = W.shape[0] // P
    mt = n_out // P
    return np.ascontiguousarray(
        W.reshape(kt, P, mt, P).transpose(1, 2, 0, 3).astype(NPBF))


def _rm_pack(W):
    """[C, n_out] -> [P, kt, n_out] bf16 (rhs layout for row-major linears)."""
    W = np.asarray(W, np.float32)
    kt = W.shape[0] // P
    return np.ascontiguousarray(
        W.reshape(kt, P, W.shape[1]).transpose(1, 0, 2).astype(NPBF))


def _build(flags):
    nc = bacc.Bacc(trn_type="TRN2", target_bir_lowering=False, debug=False)
    d = {}

    def din(name, shape, dt=BF16):
        d[name] = nc.declare_dram_parameter(name, list(shape), dt,
                                            isOutput=False).ap()

    din("yq", (C, R))
    din("ykv", (C, T))
    din("xkv", (C, T))
    din("Wq_attn", (P, FT, FT, P))
    din("Wk_attn", (P, FT, FT, P))
    din("Wv_attn", (P, FT, T))
    din("W_proj_p", (P, FT, FT, P))
    din("Wk_en", (P, FT, FT, P))
    din("Wv_en", (P, FT, T))
    din("W_q_p", (P, FT, FT, P))
    din("W_cproj_p", (P, FT, FT, P))
    din("W_d1_p", (P, 32, FT, P))
    din("W_d2_p", (P, FT, 32, P))
    din("ones", (P, R))
    din("ones_f", (1, P), dt=mybir.dt.float32r)
    for nm, shape in (("b_attn", (3 * C,)), ("b_proj", (C,)), ("b_en", (2 * C,)),
                      ("b_q", (C,)), ("b_cproj", (C,)), ("b_d1", (DFF,)),
                      ("b_d2", (C,))):
        if nm in flags:
            din(nm, shape, dt=F32)
    for nm in ("ln_w", "ln_b", "ln1_w", "ln1_b", "ln2_w", "ln2_b"):
        if nm in flags:
            din(nm, (C,), dt=F32)
    d["out"] = nc.declare_dram_parameter("out", [C, R], BF16,
                                         isOutput=True).ap()

    with tile.TileContext(nc) as tc:
        _emit(nc, tc, d, flags)
    nc.compile()
    return nc


def _flags_of(inputs):
    flags = set()
    for nm in ("b_attn", "b_proj", "b_en", "b_q", "b_cproj", "b_d1", "b_d2"):
        if np.any(np.asarray(inputs[nm]) != 0):
            flags.add(nm)
    for nm, triv in (("ln_w", 1.0), ("ln_b", 0.0), ("ln1_w", 1.0),
                     ("ln1_b", 0.0), ("ln2_w", 1.0), ("ln2_b", 0.0)):
        if np.any(np.asarray(inputs[nm]) != triv):
            flags.add(nm)
    for a, b in (("ln_w", "ln_b"), ("ln1_w", "ln1_b"), ("ln2_w", "ln2_b")):
        if a in flags or b in flags:
            flags.add(a)
            flags.add(b)
    return flags


def _make_in_maps(inputs):
    flags = _flags_of(inputs)
    x = np.asarray(inputs["x"], np.float32)
    y = np.asarray(inputs["y"], np.float32)
    W_attn = np.asarray(inputs["W_attn"], np.float32)
    W_en = np.asarray(inputs["W_en"], np.float32)
    base = {
        "Wq_attn": _fm_pack(W_attn[:, 0:C], C),
        "Wk_attn": _fm_pack(W_attn[:, C:2 * C], C),
        "Wv_attn": _rm_pack(W_attn[:, 2 * C:3 * C]),
        "W_proj_p": _fm_pack(inputs["W_proj"], C),
        "Wk_en": _fm_pack(W_en[:, 0:C], C),
        "Wv_en": _rm_pack(W_en[:, C:2 * C]),
        "W_q_p": _fm_pack(inputs["W_q"], C),
        "W_cproj_p": _fm_pack(inputs["W_cproj"], C),
        "W_d1_p": np.ascontiguousarray(
            np.asarray(inputs["W_d1"], np.float32)
            .reshape(FT, P, 32, P).transpose(1, 2, 0, 3).astype(NPBF)),
        "W_d2_p": np.ascontiguousarray(
            np.asarray(inputs["W_d2"], np.float32)
            .reshape(32, P, FT, P).transpose(1, 2, 0, 3).astype(NPBF)),
        "ones": np.ones((P, R), NPBF),
        "ones_f": np.ones((1, P), np.float32),
    }
    for nm in flags:
        base[nm] = np.ascontiguousarray(np.asarray(inputs[nm], np.float32))
    yT = [np.ascontiguousarray(y[b].T.astype(NPBF)) for b in range(B)]
    xT = [np.ascontiguousarray(x[b].T.astype(NPBF)) for b in range(B)]
    in_maps = []
    for c in range(NCORES):
        b, h = divmod(c, 2)
        m = dict(base)
        m["ykv"] = yT[b]
        m["xkv"] = xT[b]
        m["yq"] = np.ascontiguousarray(yT[b][:, h * R:(h + 1) * R])
        in_maps.append(m)
    return in_maps


def kernel(x, y, W_attn, b_attn, W_proj, b_proj, ln_w, ln_b,
           W_en, b_en, W_q, b_q, W_cproj, b_cproj,
           ln1_w, ln1_b, ln2_w, ln2_b, W_d1, b_d1, W_d2, b_d2):
    inputs = dict(x=x, y=y, W_attn=W_attn, b_attn=b_attn, W_proj=W_proj,
                  b_proj=b_proj, ln_w=ln_w, ln_b=ln_b, W_en=W_en, b_en=b_en,
                  W_q=W_q, b_q=b_q, W_cproj=W_cproj, b_cproj=b_cproj,
                  ln1_w=ln1_w, ln1_b=ln1_b, ln2_w=ln2_w, ln2_b=ln2_b,
                  W_d1=W_d1, b_d1=b_d1, W_d2=W_d2, b_d2=b_d2)
    flags = _flags_of(inputs)
    key = tuple(sorted(flags))
    if key not in _CACHE:
        _CACHE[key] = _build(flags)
    nc = _CACHE[key]

    in_maps = _make_in_maps(inputs)
    res = run_bass_kernel_spmd(nc, in_maps, list(range(NCORES)))
    out = np.empty((B, T, C), np.float32)
    for c in range(NCORES):
        b, h = divmod(c, 2)
        out[b, h * R:(h + 1) * R, :] = \
            np.asarray(res.results[c]["out"], np.float32).T
    return out


# revision 63
# speedup vs baseline: 1.0238x; 1.0238x over previous
"""Decoder block on 8 TRN2 NeuronCores — pipelined bf16 version.

Sharding: core c -> (batch b=c//2, half h=c%2): 512 query rows through the
full decoder; K/V span the full T=1024 of that batch element. All activations
feature-major [C, rows]; matmuls run as out = lhsT.T @ rhs in bf16 with fp32
PSUM accumulation.

Schedule: every engine's issue order is pinned to emission order; emission is
software-pipelined so the PE never waits: attention runs S(tk) / exp(tk-1) /
O(tk-2) with the next head-pair's K-projection matmuls as PE filler under the
ACT exp stream; weights are host-repacked to bf16 in DMA-contiguous layouts
and streamed >=1MB per transfer well ahead of use.
"""

import numpy as np
import ml_dtypes

import concourse.bass as bass
from bass_rust import add_dep_helper
import concourse.mybir as mybir
import concourse.tile as tile
from concourse import bacc
from concourse.bass_utils import run_bass_kernel_spmd

B, T, C, H = 4, 1024, 1024, 16
HD = C // H            # 64
DFF = 4096
EPS = 1e-5
P = 128
R = 512                # query rows per core
FT = C // P            # 8 feature ptiles
RT = T // P            # 8 key-row tiles
NCORES = 8

F32 = mybir.dt.float32
BF16 = mybir.dt.bfloat16
AF = mybir.ActivationFunctionType
NPBF = ml_dtypes.bfloat16

_CACHE = {}


def _emit(nc, tc, d, flags):
    sync = nc.sync
    ve = nc.vector
    se = nc.scalar
    te = nc.tensor
    ge = nc.gpsimd

    # Pin every engine's issue order to emission order (the emitted program
    # is a feasible sequential order by construction; this stops the Tile
    # scheduler from hoisting an instruction onto a busy slot).
    _last = {}

    def _chain(key, inst):
        prev = _last.get(key)
        if prev is not None:
            add_dep_helper(inst.ins, prev.ins, sync=False,
                           reason=f"{key} emission-order chain")
        _last[key] = inst
        return inst

    def dma(out, in_):
        return _chain("sp", sync.dma_start(out=out, in_=in_))

    def gdma(out, in_):
        return _chain("pool", ge.dma_start(out=out, in_=in_))

    class _Chained:
        def __init__(self, eng, key):
            self._eng = eng
            self._key = key

        def __getattr__(self, name):
            fn = getattr(self._eng, name)

            def wrapped(*a, **k):
                return _chain(self._key, fn(*a, **k))

            return wrapped

    ve = _Chained(ve, "dve")
    se = _Chained(se, "act")
    te = _Chained(te, "pe")

    sb = tc.alloc_tile_pool(name="sb", bufs=1)
    ps = tc.alloc_tile_pool(name="ps", bufs=1, space="PSUM")

    # ---- constants ----
    ones_t = sb.tile([P, R], BF16, tag="ones", bufs=1, name="ones_t")
    dma(out=ones_t[:], in_=d["ones"][:, :])
    ones_f = sb.tile([1, P], mybir.dt.float32r, tag="onesf", bufs=1,
                     name="ones_f")
    dma(out=ones_f[:], in_=d["ones_f"][:, :])
    eps_t = sb.tile([1, 1], F32, tag="eps", bufs=1, name="eps_t")
    ve.memset(eps_t[:], EPS)
    invc_t = sb.tile([1, 1], F32, tag="eps2", bufs=1, name="invc_t")
    ve.memset(invc_t[:], 1.0 / C)

    # ---- tile makers -------------------------------------------------
    def act8():
        # activation sets [P, FT, R]; FIFO reuse order must respect liveness
        return sb.tile([P, FT, R], BF16, tag="act8", bufs=5, name="act8")

    def big_tile(shape, name):
        return sb.tile(shape, BF16, tag="big", bufs=2, name=name)

    def wfm_tile(shape, name):
        return sb.tile(shape, BF16, tag="wfm", bufs=3, name=name)

    def wrm_tile(shape, name):
        return sb.tile(shape, BF16, tag="wrm", bufs=2, name=name)

    def mm_ps():
        return ps.tile([P, R], F32, tag="mm", bufs=2, name="mm")

    def stat_s(dt=F32):
        # short-lived row stats (dead before the next three allocs of this tag)
        return sb.tile([1, R], dt, tag="statS", bufs=4, name="statS")

    def stat_l(dt=F32):
        # long-lived row stats (mu / rstd / mu*rstd within one layernorm)
        return sb.tile([1, R], dt, tag="statL", bufs=3, name="statL")

    # ---- optional-bias helpers (zero in the graded configuration) ----
    def bias_fm(psm, bias_ap, mt, n):
        """psm[P, n] += b[mt*P : mt*P+P] outer ones_n (feature-major)."""
        b_f = sb.tile([1, P], F32, tag="biaf", bufs=2, name="b_fmf")
        dma(out=b_f[:], in_=bias_ap[mt * P:(mt + 1) * P][None, :])
        b_t = sb.tile([1, P], BF16, tag="bia", bufs=2, name="b_fm")
        ve.tensor_copy(b_t[:], b_f[:])
        te.matmul(psm[:, 0:n], b_t[:], ones_t[0:1, 0:n], start=False, stop=True)

    def bias_rm(psm, bias_ap, cc):
        """psm[P, 512] += ones_col outer b[cc*512 : cc*512+512] (row-major)."""
        b_f = sb.tile([1, R], F32, tag="biaf", bufs=2, name="b_rmf")
        dma(out=b_f[:], in_=bias_ap[cc * R:(cc + 1) * R][None, :])
        b_t = sb.tile([1, R], BF16, tag="bia", bufs=2, name="b_rm")
        ve.tensor_copy(b_t[:], b_f[:])
        te.matmul(psm[:], ones_t[0:1, 0:P], b_t[:], start=False, stop=True)

    # ---- generic fm linear: one output ptile ------------------------
    def fm_group(w_tile, mt, rhs_slices, bias_ap=None, bias_mt=None):
        """psum[P, 512] = sum_kt w_tile[:, mt, kt, :].T @ rhs_slices[kt]."""
        psm = mm_ps()
        last = len(rhs_slices) - 1
        for kt, rhs in enumerate(rhs_slices):
            te.matmul(psm[:], w_tile[:, mt, kt, :], rhs,
                      start=(kt == 0), stop=(kt == last and bias_ap is None))
        if bias_ap is not None:
            bias_fm(psm, bias_ap, bias_mt if bias_mt is not None else mt, R)
        return psm

    # ================= layernorm (stats + normalize) =================
    # Stats are accumulated tile-by-tile as inputs appear (x2 mul + two
    # ones-matmuls); normalize uses PE broadcast of rstd / mu*rstd.
    class LNState:
        pass

    def ln_begin():
        st = LNState()
        st.sx = ps.tile([1, R], F32, tag="o", bufs=2, name="ln_sx")
        st.sx2 = ps.tile([1, R], F32, tag="o", bufs=2, name="ln_sx2")
        return st

    def ln_accum(st, x_sl, kt):
        x2 = sb.tile([P, R], BF16, tag="x2", bufs=2, name="x2")
        ve.tensor_mul(x2[:], x_sl, x_sl)
        te.matmul(st.sx[:], ones_t[:, 0:1], x_sl,
                  start=(kt == 0), stop=(kt == FT - 1))
        te.matmul(st.sx2[:], ones_t[:, 0:1], x2[:],
                  start=(kt == 0), stop=(kt == FT - 1))

    def ln_stats(st):
        # mu is NEGATED so the apply step is x*rstd_b + (-mu*rstd)_b — a
        # commutative add with the PSUM operand in the proven first slot
        mu = stat_l()
        ve.tensor_scalar_mul(mu[:], st.sx[:], -1.0 / C)
        ex2 = stat_s()
        ve.tensor_scalar_mul(ex2[:], st.sx2[:], 1.0 / C)
        mu2 = stat_s()
        ve.tensor_mul(mu2[:], mu[:], mu[:])
        var = stat_s()
        ve.tensor_sub(var[:], ex2[:], mu2[:])
        # rstd = exp(-0.5 * ln(var + eps)) — keeps ACT on the ln/exp table set
        lnv = stat_s()
        se.activation(out=lnv[:], in_=var[:], func=AF.Ln,
                      bias=eps_t[0:1, 0:1], scale=1.0)
        rstd = stat_l(BF16)
        se.activation(out=rstd[:], in_=lnv[:], func=AF.Exp, scale=-0.5)
        # broadcast via PE into [P, R] (uses the idle "st" PSUM space);
        # applies read these straight from PSUM (PSUM-first operand)
        rb = ps.tile([P, R], F32, tag="st", bufs=2, name="ln_rb")
        te.matmul(rb[:], ones_t[0:1, 0:P], rstd[:], start=True, stop=True)
        rbs = sb.tile([P, R], BF16, tag="lnb", bufs=2, name="ln_rbs")
        ve.tensor_copy(rbs[:], rb[:])
        musr = stat_l(BF16)
        ve.tensor_mul(musr[:], mu[:], rstd[:])
        mb = ps.tile([P, R], F32, tag="st", bufs=2, name="ln_mb")
        te.matmul(mb[:], ones_t[0:1, 0:P], musr[:], start=True, stop=True)
        mbs = sb.tile([P, R], BF16, tag="lnb", bufs=2, name="ln_mbs")
        ve.tensor_copy(mbs[:], mb[:])
        st.rbs, st.mbs = rbs, mbs

    def ln_apply(st, x_sl, out_sl, w_ap, b_ap, kt, out_dt=BF16):
        tmp = sb.tile([P, R], BF16, tag="lntmp", bufs=2, name="lntmp")
        ve.tensor_mul(tmp[:], x_sl, st.rbs[:])
        if w_ap is None and b_ap is None:
            ve.tensor_add(out_sl, tmp[:], st.mbs[:])
        else:
            nrm = sb.tile([P, R], BF16, tag="lntmp", bufs=2, name="lnnrm")
            ve.tensor_add(nrm[:], tmp[:], st.mbs[:])
            w_t = sb.tile([P, 1], F32, tag="lnw", bufs=4, name="lnw")
            if w_ap is not None:
                dma(out=w_t[:], in_=w_ap[kt * P:(kt + 1) * P][:, None])
            else:
                ve.memset(w_t[:], 1.0)
            b_t = sb.tile([P, 1], F32, tag="lnw", bufs=4, name="lnb")
            if b_ap is not None:
                dma(out=b_t[:], in_=b_ap[kt * P:(kt + 1) * P][:, None])
            else:
                ve.memset(b_t[:], 0.0)
            ve.scalar_tensor_tensor(
                out_sl, nrm[:], w_t[:], b_t[:].to_broadcast((P, R)),
                op0=mybir.AluOpType.mult, op1=mybir.AluOpType.add)

    # =================================================================
    # Prologue DMAs (sp ring, in order of first use)
    # =================================================================
    # q-gen inputs first so the PE can start ~6us in (Wq arrives per-mt,
    # group 0 only needs the first slice); the rest stream underneath the
    # q/v projection work
    yq_all = act8()                           # slot 0
    dma(out=yq_all[:], in_=d["yq"].rearrange("(kt p) r -> p kt r", p=P))
    wq_t = wfm_tile([P, FT, FT, P], "wq")     # wfm slot 0
    for mt in range(FT):
        dma(out=wq_t[:, mt, :, :], in_=d["Wq_attn"][:, mt, :, :])
    ykv_all = big_tile([P, FT, T], "ykv")     # big slot 0
    dma(out=ykv_all[:], in_=d["ykv"].rearrange("(kt p) r -> p kt r", p=P))
    wv_t = wrm_tile([P, FT, T], "wv")         # wrm slot 0
    dma(out=wv_t[:], in_=d["Wv_attn"][:])
    wk_t = wfm_tile([P, FT, FT, P], "wk")     # wfm slot 1
    dma(out=wk_t[:], in_=d["Wk_attn"][:])

    b_attn = d.get("b_attn")

    # ================= A: q generation (feature-major) ===============
    q_all = act8()                            # slot 1
    for mt in range(FT):
        psm = fm_group(wq_t, mt, [yq_all[:, kt, :] for kt in range(FT)],
                       bias_ap=b_attn, bias_mt=mt)
        ve.tensor_copy(q_all[:, mt, :], psm[:])

    # more prefetch: cross-attn inputs + proj weights
    xkv_all = big_tile([P, FT, T], "xkv")     # big slot 1
    dma(out=xkv_all[:], in_=d["xkv"].rearrange("(kt p) r -> p kt r", p=P))
    wven_t = wrm_tile([P, FT, T], "wven")     # wrm slot 1
    dma(out=wven_t[:], in_=d["Wv_en"][:])
    wproj_t = wfm_tile([P, FT, FT, P], "wproj")   # wfm slot 2
    dma(out=wproj_t[:], in_=d["W_proj_p"][:])

    # ================= B: v generation (row-major, + ones col) =======
    def gen_v(src_all, w_rm, v_tiles, bias_ap, pfx):
        for cc in range(2):
            for rt in range(RT):
                psm = mm_ps()
                for kt in range(FT):
                    te.matmul(psm[:],
                              src_all[:, kt, rt * P:(rt + 1) * P],
                              w_rm[:, kt, cc * R:(cc + 1) * R],
                              start=(kt == 0),
                              stop=(kt == FT - 1 and bias_ap is None))
                if bias_ap is not None:
                    bias_rm(psm, bias_ap, cc)
                if cc == 0:
                    v_tiles[rt] = sb.tile([P, H, HD + 1], BF16, tag="v65",
                                          bufs=RT, name=f"{pfx}{rt}")
                data = ve.tensor_copy(
                    v_tiles[rt][:, cc * 8:(cc + 1) * 8, 0:HD],
                    psm[:].rearrange("p (h d) -> p h d", d=HD))
                if cc == 0:
                    oc = ve.tensor_copy(v_tiles[rt][:, :, HD], ones_t[:, 0:H])
                    add_dep_helper(oc.ins, data.ins, sync=False,
                                   reason="ones col after v data (slot order)")

    v_sb = [None] * RT
    gen_v(ykv_all, wv_t,  v_sb,
          (d["b_attn"][2 * C:3 * C] if "b_attn" in d else None), "v")

    # ================= K-projection helper ===========================
    # k ptile hp covers heads (2hp, 2hp+1): features on partitions, T keys
    # free.  Emitted in 18 closures (16 matmuls + 2 evicts) so attention can
    # interleave them two-per-unit as PE filler under the exp stream.
    def k_steps(w_t, src_all, hp, bias_ap, box, pfx):
        steps = []
        psms = {}

        def mk_mm(cc, kt):
            def go():
                if kt == 0:
                    psms[cc] = mm_ps()
                te.matmul(psms[cc][:], w_t[:, hp, kt, :],
                          src_all[:, kt, cc * R:(cc + 1) * R],
                          start=(kt == 0),
                          stop=(kt == FT - 1 and bias_ap is None))
            return go

        def mk_ev(cc):
            def go():
                if bias_ap is not None:
                    bias_fm(psms[cc], bias_ap, hp, R)
                if cc == 0:
                    box[0] = sb.tile([P, T], BF16, tag="ksb", bufs=2,
                                     name=f"{pfx}{hp}")
                ve.tensor_copy(box[0][:, cc * R:(cc + 1) * R], psms[cc][:])
            return go

        for cc in range(2):
            for kt in range(FT):
                steps.append(mk_mm(cc, kt))
            steps.append(mk_ev(cc))
        return steps

    # ================= attention =====================================
    def attention(q_all_t, k_w, k_src, k_bias, v_tiles, o_all_t, pfx,
                  tail_filler):
        """tail_filler: list of closures run as PE filler during the last
        head-pair (which has no next-K to generate)."""

        def norm_head(o_ps):
            # DVE half of softmax normalization, emitted right after the
            # final O matmuls: reciprocal straight off the PSUM ones-row,
            # and the o halves staged into SBUF (so the later mul's PSUM
            # operand can be the PE-broadcast reciprocal instead)
            # ordering matched to each consumer's deadline: per-s den+oc
            # pairs free that o_ps slot for the next head-pair's O matmul
            # (unit 2); the reciprocal chains follow, landing just before
            # their rb matmuls (unit 3)
            rcp, ocp, dens = [], [], []
            for s in range(2):
                den = stat_s()
                ve.tensor_copy(den[:], o_ps[s][HD:HD + 1, :])
                dens.append(den)
                oc = sb.tile([HD, R], BF16, tag="rbs", bufs=2,
                             name=f"{pfx}ocp")
                ve.tensor_copy(oc[:], o_ps[s][0:HD, :])
                ocp.append(oc)
            for s in range(2):
                rc = stat_s()
                ve.reciprocal_approx_fast(rc[:], dens[s][:])
                rcb = stat_s(BF16)
                ve.tensor_copy(rcb[:], rc[:])
                rcp.append(rcb)
            return rcp, ocp

        def norm_tail(hp, rcp, ocp):
            # PE broadcast + final scale, hidden under the next head-pair's
            # S matmuls
            for s in range(2):
                rb = ps.tile([P, R], F32, tag="mm", bufs=2, name=f"{pfx}rb")
                te.matmul(rb[0:HD, :], ones_t[0:1, 0:HD], rcp[s][:],
                          start=True, stop=True)
                if s == 0:
                    ve.tensor_mul(o_all_t[0:HD, hp, :], rb[0:HD, :],
                                  ocp[s][:])
                else:
                    tmp = sb.tile([HD, R], BF16, tag="otmp", bufs=2,
                                  name=f"{pfx}otmp")
                    ve.tensor_mul(tmp[:], rb[0:HD, :], ocp[s][:])
                    gdma(out=o_all_t[HD:P, hp, :], in_=tmp[:])

        kbox = [None]
        for s0 in k_steps(k_w, k_src, 0, k_bias, kbox, pfx + "k"):
            s0()
        k_cur = kbox[0]
        pending = None          # (hp, o_ps) awaiting normalization
        for hp in range(FT):
            # next head-pair's K projection as filler
            if hp + 1 < FT:
                nbox = [None]
                filler = k_steps(k_w, k_src, hp + 1, k_bias, nbox, pfx + "k")
            else:
                nbox = None
                filler = list(tail_filler)
            o_ps = [None, None]
            es_q = [None] * RT
            fi = 0
            for u in range(RT + 3):
                if u < RT:
                    st_t = ps.tile([P, 2 * R], F32, tag="st", bufs=2,
                                   name=f"{pfx}st")
                    for s in range(2):
                        off = HD * s
                        te.matmul(st_t[:, s * R:(s + 1) * R],
                                  k_cur[off:off + HD, u * P:(u + 1) * P],
                                  q_all_t[off:off + HD, hp, :],
                                  start=True, stop=True)
                    es = sb.tile([P, 2 * R], BF16, tag="es", bufs=4,
                                 name=f"{pfx}es")
                    se.activation(out=es[:], in_=st_t[:], func=AF.Exp,
                                  scale=0.125)
                    es_q[u] = es
                if u >= 3:
                    tko = u - 3
                    if tko == 0:
                        o_ps[0] = ps.tile([HD + 1, R], F32, tag="o", bufs=2,
                                          name=f"{pfx}o0")
                        o_ps[1] = ps.tile([HD + 1, R], F32, tag="o", bufs=2,
                                          name=f"{pfx}o1")
                    for s in range(2):
                        te.matmul(o_ps[s][:],
                                  v_tiles[tko][:, 2 * hp + s, :],
                                  es_q[tko][:, s * R:(s + 1) * R],
                                  start=(tko == 0), stop=(tko == RT - 1))
                # filler pacing: one pop at u4/u9 so the K psum slot (tag
                # "mm") recycles cleanly around norm_tail's rb matmuls —
                # the cc0 evict lands at u4 just before rb, and the cc1
                # psum allocates at u5 just after the rb/mul pair
                for _ in range(1 if u in (4, RT + 1) else 2):
                    if fi < len(filler):
                        filler[fi]()
                        fi += 1
                if u == 4 and pending is not None:
                    # norm_tail here gets ~5 units of PE cover for the
                    # reciprocal chain emitted at the last head-pair's end
                    norm_tail(*pending)
                    pending = None
            while fi < len(filler):
                filler[fi]()
                fi += 1
            rcp, ocp = norm_head(o_ps)
            pending = (hp, rcp, ocp)
            if nbox is not None:
                k_cur = nbox[0]
        norm_tail(*pending)

    o_all = act8()                            # slot 2
    attention(q_all, wk_t, ykv_all,
              (d["b_attn"][C:2 * C] if "b_attn" in d else None),
              v_sb, o_all, "sa", [])

    # ================= proj + residual, LN1 stats interleaved ========
    # ln_accum lags its producer by one tile so the sums matmuls hide
    # under the next output tile's matmul group instead of stalling on
    # the 3-op DVE eviction chain
    y1_all = act8()                           # slot 3
    ln1 = ln_begin()
    for mt in range(FT):
        psm = fm_group(wproj_t, mt, [o_all[:, kt, :] for kt in range(FT)],
                       bias_ap=d.get("b_proj"), bias_mt=mt)
        ve.tensor_add(y1_all[:, mt, :], psm[:], yq_all[:, mt, :])
        if mt > 0:
            ln_accum(ln1, y1_all[:, mt - 1, :], mt - 1)
    ln_accum(ln1, y1_all[:, FT - 1, :], FT - 1)

    # prefetch cross-attn K weights (wfm slot 0: wq dead) and the q2/cproj
    # weights (slots 1/2: wk and wproj are dead once proj is emitted), so
    # the LN1 seam and cross-attn aren't gated on weight DMAs
    wken_t = wfm_tile([P, FT, FT, P], "wken")
    dma(out=wken_t[:], in_=d["Wk_en"][:])
    wq2_t = wfm_tile([P, FT, FT, P], "wq2")       # wfm slot 1 (wk dead)
    dma(out=wq2_t[:], in_=d["W_q_p"][:])
    wcproj_t = wfm_tile([P, FT, FT, P], "wcp")    # wfm slot 2 (wproj dead)
    dma(out=wcproj_t[:], in_=d["W_cproj_p"][:])

    # ================= v2 generation (PE) overlapping LN1 (DVE) ======
    v2_sb = [None] * RT
    gen_v(xkv_all, wven_t, v2_sb,
          (d["b_en"][C:2 * C] if "b_en" in d else None), "w")

    ln_stats(ln1)
    y1n_all = act8()                          # slot 4
    for mt in range(FT):
        ln_apply(ln1, y1_all[:, mt, :], y1n_all[:, mt, :],
                 d.get("ln_w"), d.get("ln_b"), mt)

    # ================= q2 generation =================================
    q2_all = act8()                           # slot 5
    for mt in range(FT):
        psm = fm_group(wq2_t, mt, [y1n_all[:, kt, :] for kt in range(FT)],
                       bias_ap=d.get("b_q"), bias_mt=mt)
        ve.tensor_copy(q2_all[:, mt, :], psm[:])

    # prefetch first two FFN d1 weight chunks under cross-attention
    wd1_t = [None] * 4
    for ch in range(2):
        wd1_t[ch] = wrm_tile([P, FT, FT, P], f"wd1_{ch}")
        dma(out=wd1_t[ch][:],
            in_=d["W_d1_p"][:, ch * FT:(ch + 1) * FT, :, :])

    # ================= cross attention ===============================
    o2_all = act8()                           # slot 0 (yq dead)
    attention(q2_all, wken_t, xkv_all,
              (d["b_en"][0:C] if "b_en" in d else None),
              v2_sb, o2_all, "ca", [])

    # ================= cproj + residual (into y1n), LN2 ==============
    ln2 = ln_begin()
    for mt in range(FT):
        psm = fm_group(wcproj_t, mt, [o2_all[:, kt, :] for kt in range(FT)],
                       bias_ap=d.get("b_cproj"), bias_mt=mt)
        ve.tensor_add(y1n_all[:, mt, :], psm[:], y1n_all[:, mt, :])
        if mt > 0:
            ln_accum(ln2, y1n_all[:, mt - 1, :], mt - 1)
    ln_accum(ln2, y1n_all[:, FT - 1, :], FT - 1)
    ln_stats(ln2)
    xin_all = act8()                          # slot 1 (q dead)
    for mt in range(FT):
        ln_apply(ln2, y1n_all[:, mt, :], xin_all[:, mt, :],
                 d.get("ln1_w"), d.get("ln1_b"), mt)

    # ================= FFN ===========================================
    # F1: h = xin @ W_d1 (feature-major, 32 output ptiles in 4 weight chunks)
    ht_a = big_tile([P, 16, R], "ht_a")       # big slot 0 (ykv dead)
    ht_b = big_tile([P, 16, R], "ht_b")       # big slot 1 (xkv dead)

    # first d2 weight tile prefetches under F1
    wd2_t = [None] * FT
    wd2_t[0] = wfm_tile([P, 32, P], "wd2_0")
    dma(out=wd2_t[0][:], in_=d["W_d2_p"][:, 0, :, :])

    def ht_sl(kt):
        return (ht_a if kt < 16 else ht_b)[:, kt % 16, :]

    for ch in range(4):
        if wd1_t[ch] is None:
            wd1_t[ch] = wrm_tile([P, FT, FT, P], f"wd1_{ch}")
            dma(out=wd1_t[ch][:],
                in_=d["W_d1_p"][:, ch * FT:(ch + 1) * FT, :, :])
        w1 = wd1_t[ch]
        # prefetch chunk ch+2 while computing on ch
        nxt = ch + 2
        if nxt < 4 and wd1_t[nxt] is None:
            wd1_t[nxt] = wrm_tile([P, FT, FT, P], f"wd1_{nxt}")
            dma(out=wd1_t[nxt][:],
                in_=d["W_d1_p"][:, nxt * FT:(nxt + 1) * FT, :, :])
        for kk in range(FT):
            kt = ch * FT + kk
            psm = mm_ps()
            for ck in range(FT):
                te.matmul(psm[:], w1[:, kk, ck, :], xin_all[:, ck, :],
                          start=(ck == 0),
                          stop=(ck == FT - 1 and "b_d1" not in d))
            if "b_d1" in d:
                bias_fm(psm, d["b_d1"], kt, R)
            ve.tensor_copy(ht_sl(kt), psm[:])

    # F2: z = h @ W_d2 + xin, LN3 stats interleaved
    ln3 = ln_begin()
    z_all = act8()                            # slot 2 (o dead)
    for mt in range(FT):
        if wd2_t[mt] is None:
            wd2_t[mt] = wfm_tile([P, 32, P], f"wd2_{mt}")
            dma(out=wd2_t[mt][:], in_=d["W_d2_p"][:, mt, :, :])
        w2 = wd2_t[mt]
        if mt + 1 < FT and wd2_t[mt + 1] is None:
            wd2_t[mt + 1] = wfm_tile([P, 32, P], f"wd2_{mt + 1}")
            dma(out=wd2_t[mt + 1][:], in_=d["W_d2_p"][:, mt + 1, :, :])
        psm = mm_ps()
        for kt in range(32):
            te.matmul(psm[:], w2[:, kt, :], ht_sl(kt),
                      start=(kt == 0), stop=(kt == 31 and "b_d2" not in d))
        if "b_d2" in d:
            bias_fm(psm, d["b_d2"], mt, R)
        ve.tensor_add(z_all[:, mt, :], psm[:], xin_all[:, mt, :])
        if mt > 0:
            ln_accum(ln3, z_all[:, mt - 1, :], mt - 1)
    ln_accum(ln3, z_all[:, FT - 1, :], FT - 1)
    ln_stats(ln3)

    for mt in range(FT):
        ot = sb.tile([P, R], BF16, tag="outt", bufs=2, name="out_t")
        ln_apply(ln3, z_all[:, mt, :], ot[:],
                 d.get("ln2_w"), d.get("ln2_b"), mt)
        dma(out=d["out"][mt * P:(mt + 1) * P, :], in_=ot[:])

    sb.release()
    ps.release()


def _fm_pack(W, n_out):
    """[C, n_out] -> [P, mt, kt, P] bf16 so lhsT tiles are DMA-contiguous."""
    W = np.asarray(W, np.float32)
    kt = W.shape[0] // P
    mt = n_out // P
    return np.ascontiguousarray(
        W.reshape(kt, P, mt, P).transpose(1, 2, 0, 3).astype(NPBF))


def _rm_pack(W):
    """[C, n_out] -> [P, kt, n_out] bf16 (rhs layout for row-major linears)."""
    W = np.asarray(W, np.float32)
    kt = W.shape[0] // P
    return np.ascontiguousarray(
        W.reshape(kt, P, W.shape[1]).transpose(1, 0, 2).astype(NPBF))


def _build(flags):
    nc = bacc.Bacc(trn_type="TRN2", target_bir_lowering=False, debug=False)
    d = {}

    def din(name, shape, dt=BF16):
        d[name] = nc.declare_dram_parameter(name, list(shape), dt,
                                            isOutput=False).ap()

    din("yq", (C, R))
    din("ykv", (C, T))
    din("xkv", (C, T))
    din("Wq_attn", (P, FT, FT, P))
    din("Wk_attn", (P, FT, FT, P))
    din("Wv_attn", (P, FT, T))
    din("W_proj_p", (P, FT, FT, P))
    din("Wk_en", (P, FT, FT, P))
    din("Wv_en", (P, FT, T))
    din("W_q_p", (P, FT, FT, P))
    din("W_cproj_p", (P, FT, FT, P))
    din("W_d1_p", (P, 32, FT, P))
    din("W_d2_p", (P, FT, 32, P))
    din("ones", (P, R))
    din("ones_f", (1, P), dt=mybir.dt.float32r)
    for nm, shape in (("b_attn", (3 * C,)), ("b_proj", (C,)), ("b_en", (2 * C,)),
                      ("b_q", (C,)), ("b_cproj", (C,)), ("b_d1", (DFF,)),
                      ("b_d2", (C,))):
        if nm in flags:
            din(nm, shape, dt=F32)
    for nm in ("ln_w", "ln_b", "ln1_w", "ln1_b", "ln2_w", "ln2_b"):
        if nm in flags:
            din(nm, (C,), dt=F32)
    d["out"] = nc.declare_dram_parameter("out", [C, R], BF16,
                                         isOutput=True).ap()

    with tile.TileContext(nc) as tc:
        _emit(nc, tc, d, flags)
    nc.compile()
    return nc


def _flags_of(inputs):
    flags = set()
    for nm in ("b_attn", "b_proj", "b_en", "b_q", "b_cproj", "b_d1", "b_d2"):
        if np.any(np.asarray(inputs[nm]) != 0):
            flags.add(nm)
    for nm, triv in (("ln_w", 1.0), ("ln_b", 0.0), ("ln1_w", 1.0),
                     ("ln1_b", 0.0), ("ln2_w", 1.0), ("ln2_b", 0.0)):
        if np.any(np.asarray(inputs[nm]) != triv):
            flags.add(nm)
    for a, b in (("ln_w", "ln_b"), ("ln1_w", "ln1_b"), ("ln2_w", "ln2_b")):
        if a in flags or b in flags:
            flags.add(a)
            flags.add(b)
    return flags


def _make_in_maps(inputs):
    flags = _flags_of(inputs)
    x = np.asarray(inputs["x"], np.float32)
    y = np.asarray(inputs["y"], np.float32)
    W_attn = np.asarray(inputs["W_attn"], np.float32)
    W_en = np.asarray(inputs["W_en"], np.float32)
    base = {
        "Wq_attn": _fm_pack(W_attn[:, 0:C], C),
        "Wk_attn": _fm_pack(W_attn[:, C:2 * C], C),
        "Wv_attn": _rm_pack(W_attn[:, 2 * C:3 * C]),
        "W_proj_p": _fm_pack(inputs["W_proj"], C),
        "Wk_en": _fm_pack(W_en[:, 0:C], C),
        "Wv_en": _rm_pack(W_en[:, C:2 * C]),
        "W_q_p": _fm_pack(inputs["W_q"], C),
        "W_cproj_p": _fm_pack(inputs["W_cproj"], C),
        "W_d1_p": np.ascontiguousarray(
            np.asarray(inputs["W_d1"], np.float32)
            .reshape(FT, P, 32, P).transpose(1, 2, 0, 3).astype(NPBF)),
        "W_d2_p": np.ascontiguousarray(
            np.asarray(inputs["W_d2"], np.float32)
            .reshape(32, P, FT, P).transpose(1, 2, 0, 3).astype(NPBF)),
        "ones": np.ones((P, R), NPBF),
        "ones_f": np.ones((1, P), np.float32),
    }
    for nm in flags:
        base[nm] = np.ascontiguousarray(np.asarray(inputs[nm], np.float32))
    yT = [np.ascontiguousarray(y[b].T.astype(NPBF)) for b in range(B)]
    xT = [np.ascontiguousarray(x[b].T.astype(NPBF)) for b in range(B)]
    in_maps = []
    for c in range(NCORES):
        b, h = divmod(c, 2)
        m = dict(base)
        m["ykv"] = yT[b]
        m["xkv"] = xT[b]
        m["yq"] = np.ascontiguousarray(yT[b][:, h * R:(h + 1) * R])
        in_maps.append(m)
    return in_maps


def kernel(x, y, W_attn, b_attn, W_proj, b_proj, ln_w, ln_b,
           W_en, b_en, W_q, b_q, W_cproj, b_cproj,
           ln1_w, ln1_b, ln2_w, ln2_b, W_d1, b_d1, W_d2, b_d2):
    inputs = dict(x=x, y=y, W_attn=W_attn, b_attn=b_attn, W_proj=W_proj,
                  b_proj=b_proj, ln_w=ln_w, ln_b=ln_b, W_en=W_en, b_en=b_en,
                  W_q=W_q, b_q=b_q, W_cproj=W_cproj, b_cproj=b_cproj,
                  ln1_w=ln1_w, ln1_b=ln1_b, ln2_w=ln2_w, ln2_b=ln2_b,
                  W_d1=W_d1, b_d1=b_d1, W_d2=W_d2, b_d2=b_d2)
    flags = _flags_of(inputs)
    key = tuple(sorted(flags))
    if key not in _CACHE:
        _CACHE[key] = _build(flags)
    nc = _CACHE[key]

    in_maps = _make_in_maps(inputs)
    res = run_bass_kernel_spmd(nc, in_maps, list(range(NCORES)))
    out = np.empty((B, T, C), np.float32)
    for c in range(NCORES):
        b, h = divmod(c, 2)
        out[b, h * R:(h + 1) * R, :] = \
            np.asarray(res.results[c]["out"], np.float32).T
    return out


# revision 70
# speedup vs baseline: 1.0607x; 1.0360x over previous
"""Decoder block on 8 TRN2 NeuronCores — pipelined bf16 version.

Sharding: core c -> (batch b=c//2, half h=c%2): 512 query rows through the
full decoder; K/V span the full T=1024 of that batch element. All activations
feature-major [C, rows]; matmuls run as out = lhsT.T @ rhs in bf16 with fp32
PSUM accumulation.

Schedule: every engine's issue order is pinned to emission order; emission is
software-pipelined so the PE never waits: attention runs S(tk) / exp(tk-1) /
O(tk-2) with the next head-pair's K-projection matmuls as PE filler under the
ACT exp stream; weights are host-repacked to bf16 in DMA-contiguous layouts
and streamed >=1MB per transfer well ahead of use.
"""

import numpy as np
import ml_dtypes

import concourse.bass as bass
from bass_rust import add_dep_helper
import concourse.mybir as mybir
import concourse.tile as tile
from concourse import bacc
from concourse.bass_utils import run_bass_kernel_spmd

B, T, C, H = 4, 1024, 1024, 16
HD = C // H            # 64
DFF = 4096
EPS = 1e-5
P = 128
R = 512                # query rows per core
FT = C // P            # 8 feature ptiles
RT = T // P            # 8 key-row tiles
NCORES = 8

F32 = mybir.dt.float32
BF16 = mybir.dt.bfloat16
AF = mybir.ActivationFunctionType
NPBF = ml_dtypes.bfloat16

_CACHE = {}


def _emit(nc, tc, d, flags):
    sync = nc.sync
    ve = nc.vector
    se = nc.scalar
    te = nc.tensor
    ge = nc.gpsimd

    # Pin every engine's issue order to emission order (the emitted program
    # is a feasible sequential order by construction; this stops the Tile
    # scheduler from hoisting an instruction onto a busy slot).
    _last = {}

    def _chain(key, inst):
        prev = _last.get(key)
        if prev is not None:
            add_dep_helper(inst.ins, prev.ins, sync=False,
                           reason=f"{key} emission-order chain")
        _last[key] = inst
        return inst

    def dma(out, in_):
        return _chain("sp", sync.dma_start(out=out, in_=in_))

    def gdma(out, in_):
        return _chain("pool", ge.dma_start(out=out, in_=in_))

    class _Chained:
        def __init__(self, eng, key):
            self._eng = eng
            self._key = key

        def __getattr__(self, name):
            fn = getattr(self._eng, name)

            def wrapped(*a, **k):
                return _chain(self._key, fn(*a, **k))

            return wrapped

    ve = _Chained(ve, "dve")
    se = _Chained(se, "act")
    te = _Chained(te, "pe")

    sb = tc.alloc_tile_pool(name="sb", bufs=1)
    ps = tc.alloc_tile_pool(name="ps", bufs=1, space="PSUM")

    # ---- first input DMA ahead of the tiny constants: the q-projection
    # inputs gate the PE start
    yq_first = sb.tile([P, FT, R], BF16, tag="act8", bufs=5, name="act8")
    dma(out=yq_first[:], in_=d["yq"].rearrange("(kt p) r -> p kt r", p=P))

    # ---- constants ----
    ones_t = sb.tile([P, R], BF16, tag="ones", bufs=1, name="ones_t")
    dma(out=ones_t[:], in_=d["ones"][:, :])
    ones_f = sb.tile([1, P], mybir.dt.float32r, tag="onesf", bufs=1,
                     name="ones_f")
    dma(out=ones_f[:], in_=d["ones_f"][:, :])
    eps_t = sb.tile([1, 1], F32, tag="eps", bufs=1, name="eps_t")
    ve.memset(eps_t[:], EPS)
    invc_t = sb.tile([1, 1], F32, tag="eps2", bufs=1, name="invc_t")
    ve.memset(invc_t[:], 1.0 / C)

    # ---- tile makers -------------------------------------------------
    def act8():
        # activation sets [P, FT, R]; FIFO reuse order must respect liveness
        return sb.tile([P, FT, R], BF16, tag="act8", bufs=5, name="act8")

    def big_tile(shape, name):
        return sb.tile(shape, BF16, tag="big", bufs=2, name=name)

    def wfm_tile(shape, name):
        return sb.tile(shape, BF16, tag="wfm", bufs=3, name=name)

    def wrm_tile(shape, name):
        return sb.tile(shape, BF16, tag="wrm", bufs=2, name=name)

    def mm_ps():
        return ps.tile([P, R], F32, tag="mm", bufs=2, name="mm")

    def stat_s(dt=F32):
        # short-lived row stats (dead before the next three allocs of this tag)
        return sb.tile([1, R], dt, tag="statS", bufs=4, name="statS")

    def stat_l(dt=F32):
        # long-lived row stats (mu / rstd / mu*rstd within one layernorm)
        return sb.tile([1, R], dt, tag="statL", bufs=3, name="statL")

    # ---- optional-bias helpers (zero in the graded configuration) ----
    def bias_fm(psm, bias_ap, mt, n):
        """psm[P, n] += b[mt*P : mt*P+P] outer ones_n (feature-major)."""
        b_f = sb.tile([1, P], F32, tag="biaf", bufs=2, name="b_fmf")
        dma(out=b_f[:], in_=bias_ap[mt * P:(mt + 1) * P][None, :])
        b_t = sb.tile([1, P], BF16, tag="bia", bufs=2, name="b_fm")
        ve.tensor_copy(b_t[:], b_f[:])
        te.matmul(psm[:, 0:n], b_t[:], ones_t[0:1, 0:n], start=False, stop=True)

    def bias_rm(psm, bias_ap, cc):
        """psm[P, 512] += ones_col outer b[cc*512 : cc*512+512] (row-major)."""
        b_f = sb.tile([1, R], F32, tag="biaf", bufs=2, name="b_rmf")
        dma(out=b_f[:], in_=bias_ap[cc * R:(cc + 1) * R][None, :])
        b_t = sb.tile([1, R], BF16, tag="bia", bufs=2, name="b_rm")
        ve.tensor_copy(b_t[:], b_f[:])
        te.matmul(psm[:], ones_t[0:1, 0:P], b_t[:], start=False, stop=True)

    # ---- generic fm linear: one output ptile ------------------------
    def fm_group(w_tile, mt, rhs_slices, bias_ap=None, bias_mt=None):
        """psum[P, 512] = sum_kt w_tile[:, mt, kt, :].T @ rhs_slices[kt]."""
        psm = mm_ps()
        last = len(rhs_slices) - 1
        for kt, rhs in enumerate(rhs_slices):
            te.matmul(psm[:], w_tile[:, mt, kt, :], rhs,
                      start=(kt == 0), stop=(kt == last and bias_ap is None))
        if bias_ap is not None:
            bias_fm(psm, bias_ap, bias_mt if bias_mt is not None else mt, R)
        return psm

    # ================= layernorm (stats + normalize) =================
    # Stats are accumulated tile-by-tile as inputs appear (x2 mul + two
    # ones-matmuls); normalize uses PE broadcast of rstd / mu*rstd.
    class LNState:
        pass

    def ln_begin():
        st = LNState()
        st.sx = ps.tile([1, R], F32, tag="o", bufs=2, name="ln_sx")
        st.sx2 = ps.tile([1, R], F32, tag="o", bufs=2, name="ln_sx2")
        return st

    def ln_accum(st, x_sl, kt):
        x2 = sb.tile([P, R], BF16, tag="x2", bufs=2, name="x2")
        ve.tensor_mul(x2[:], x_sl, x_sl)
        te.matmul(st.sx[:], ones_t[:, 0:1], x_sl,
                  start=(kt == 0), stop=(kt == FT - 1))
        te.matmul(st.sx2[:], ones_t[:, 0:1], x2[:],
                  start=(kt == 0), stop=(kt == FT - 1))

    def ln_stats(st):
        # mu is NEGATED so the apply step is x*rstd_b + (-mu*rstd)_b — a
        # commutative add with the PSUM operand in the proven first slot
        mu = stat_l()
        ve.tensor_scalar_mul(mu[:], st.sx[:], -1.0 / C)
        ex2 = stat_s()
        ve.tensor_scalar_mul(ex2[:], st.sx2[:], 1.0 / C)
        mu2 = stat_s()
        ve.tensor_mul(mu2[:], mu[:], mu[:])
        var = stat_s()
        ve.tensor_sub(var[:], ex2[:], mu2[:])
        # rstd = exp(-0.5 * ln(var + eps)) — keeps ACT on the ln/exp table set
        lnv = stat_s()
        se.activation(out=lnv[:], in_=var[:], func=AF.Ln,
                      bias=eps_t[0:1, 0:1], scale=1.0)
        rstd = stat_l(BF16)
        se.activation(out=rstd[:], in_=lnv[:], func=AF.Exp, scale=-0.5)
        # broadcast via PE into [P, R] (uses the idle "st" PSUM space);
        # applies read these straight from PSUM (PSUM-first operand)
        rb = ps.tile([P, R], F32, tag="st", bufs=2, name="ln_rb")
        te.matmul(rb[:], ones_t[0:1, 0:P], rstd[:], start=True, stop=True)
        rbs = sb.tile([P, R], BF16, tag="lnb", bufs=2, name="ln_rbs")
        ve.tensor_copy(rbs[:], rb[:])
        musr = stat_l(BF16)
        ve.tensor_mul(musr[:], mu[:], rstd[:])
        mb = ps.tile([P, R], F32, tag="st", bufs=2, name="ln_mb")
        te.matmul(mb[:], ones_t[0:1, 0:P], musr[:], start=True, stop=True)
        mbs = sb.tile([P, R], BF16, tag="lnb", bufs=2, name="ln_mbs")
        ve.tensor_copy(mbs[:], mb[:])
        st.rbs, st.mbs = rbs, mbs

    def ln_apply(st, x_sl, out_sl, w_ap, b_ap, kt, out_dt=BF16):
        tmp = sb.tile([P, R], BF16, tag="lntmp", bufs=2, name="lntmp")
        ve.tensor_mul(tmp[:], x_sl, st.rbs[:])
        if w_ap is None and b_ap is None:
            ve.tensor_add(out_sl, tmp[:], st.mbs[:])
        else:
            nrm = sb.tile([P, R], BF16, tag="lntmp", bufs=2, name="lnnrm")
            ve.tensor_add(nrm[:], tmp[:], st.mbs[:])
            w_t = sb.tile([P, 1], F32, tag="lnw", bufs=4, name="lnw")
            if w_ap is not None:
                dma(out=w_t[:], in_=w_ap[kt * P:(kt + 1) * P][:, None])
            else:
                ve.memset(w_t[:], 1.0)
            b_t = sb.tile([P, 1], F32, tag="lnw", bufs=4, name="lnb")
            if b_ap is not None:
                dma(out=b_t[:], in_=b_ap[kt * P:(kt + 1) * P][:, None])
            else:
                ve.memset(b_t[:], 0.0)
            ve.scalar_tensor_tensor(
                out_sl, nrm[:], w_t[:], b_t[:].to_broadcast((P, R)),
                op0=mybir.AluOpType.mult, op1=mybir.AluOpType.add)

    # =================================================================
    # Prologue DMAs (sp ring, in order of first use)
    # =================================================================
    # q-gen inputs first so the PE can start ~6us in (Wq arrives per-mt,
    # group 0 only needs the first slice); the rest stream underneath the
    # q/v projection work
    yq_all = yq_first                         # slot 0 (DMA'd above)
    wq_t = wfm_tile([P, FT, FT, P], "wq")     # wfm slot 0
    for mt in range(FT):
        dma(out=wq_t[:, mt, :, :], in_=d["Wq_attn"][:, mt, :, :])
    ykv_all = big_tile([P, FT, T], "ykv")     # big slot 0
    dma(out=ykv_all[:], in_=d["ykv"].rearrange("(kt p) r -> p kt r", p=P))
    wv_t = wrm_tile([P, FT, T], "wv")         # wrm slot 0
    dma(out=wv_t[:], in_=d["Wv_attn"][:])
    wk_t = wfm_tile([P, FT, FT, P], "wk")     # wfm slot 1
    dma(out=wk_t[:], in_=d["Wk_attn"][:])

    b_attn = d.get("b_attn")

    # ================= A: q generation (feature-major) ===============
    q_all = act8()                            # slot 1
    for mt in range(FT):
        psm = fm_group(wq_t, mt, [yq_all[:, kt, :] for kt in range(FT)],
                       bias_ap=b_attn, bias_mt=mt)
        ve.tensor_copy(q_all[:, mt, :], psm[:])

    # more prefetch: cross-attn inputs + proj weights
    xkv_all = big_tile([P, FT, T], "xkv")     # big slot 1
    dma(out=xkv_all[:], in_=d["xkv"].rearrange("(kt p) r -> p kt r", p=P))
    wven_t = wrm_tile([P, FT, T], "wven")     # wrm slot 1
    dma(out=wven_t[:], in_=d["Wv_en"][:])
    wproj_t = wfm_tile([P, FT, FT, P], "wproj")   # wfm slot 2
    dma(out=wproj_t[:], in_=d["W_proj_p"][:])

    # ================= B: v generation (row-major, + ones col) =======
    def gen_v(src_all, w_rm, v_tiles, bias_ap, pfx):
        for cc in range(2):
            for rt in range(RT):
                psm = mm_ps()
                for kt in range(FT):
                    te.matmul(psm[:],
                              src_all[:, kt, rt * P:(rt + 1) * P],
                              w_rm[:, kt, cc * R:(cc + 1) * R],
                              start=(kt == 0),
                              stop=(kt == FT - 1 and bias_ap is None))
                if bias_ap is not None:
                    bias_rm(psm, bias_ap, cc)
                if cc == 0:
                    v_tiles[rt] = sb.tile([P, H, HD + 1], BF16, tag="v65",
                                          bufs=RT, name=f"{pfx}{rt}")
                data = ve.tensor_copy(
                    v_tiles[rt][:, cc * 8:(cc + 1) * 8, 0:HD],
                    psm[:].rearrange("p (h d) -> p h d", d=HD))
                if cc == 0:
                    oc = ve.tensor_copy(v_tiles[rt][:, :, HD], ones_t[:, 0:H])
                    add_dep_helper(oc.ins, data.ins, sync=False,
                                   reason="ones col after v data (slot order)")

    v_sb = [None] * RT
    gen_v(ykv_all, wv_t,  v_sb,
          (d["b_attn"][2 * C:3 * C] if "b_attn" in d else None), "v")

    # ================= K-projection helper ===========================
    # k ptile hp covers heads (2hp, 2hp+1): features on partitions, T keys
    # free.  Emitted in 18 closures (16 matmuls + 2 evicts) so attention can
    # interleave them two-per-unit as PE filler under the exp stream.
    def k_steps(w_t, src_all, hp, bias_ap, box, pfx):
        steps = []
        psms = {}

        def mk_mm(cc, kt):
            def go():
                if kt == 0:
                    psms[cc] = mm_ps()
                te.matmul(psms[cc][:], w_t[:, hp, kt, :],
                          src_all[:, kt, cc * R:(cc + 1) * R],
                          start=(kt == 0),
                          stop=(kt == FT - 1 and bias_ap is None))
            return go

        def mk_ev(cc):
            def go():
                if bias_ap is not None:
                    bias_fm(psms[cc], bias_ap, hp, R)
                if cc == 0:
                    box[0] = sb.tile([P, T], BF16, tag="ksb", bufs=2,
                                     name=f"{pfx}{hp}")
                ve.tensor_copy(box[0][:, cc * R:(cc + 1) * R], psms[cc][:])
            return go

        for cc in range(2):
            for kt in range(FT):
                steps.append(mk_mm(cc, kt))
            steps.append(mk_ev(cc))
        return steps

    # ================= attention =====================================
    def attention(q_all_t, k_w, k_src, k_bias, v_tiles, o_all_t, pfx,
                  tail_filler):
        """tail_filler: list of closures run as PE filler during the last
        head-pair (which has no next-K to generate)."""

        def norm_head(o_ps):
            # DVE half of softmax normalization, emitted right after the
            # final O matmuls: reciprocal straight off the PSUM ones-row,
            # and the o halves staged into SBUF (so the later mul's PSUM
            # operand can be the PE-broadcast reciprocal instead)
            # ordering matched to each consumer's deadline: per-s den+oc
            # pairs free that o_ps slot for the next head-pair's O matmul
            # (unit 2); the reciprocal chains follow, landing just before
            # their rb matmuls (unit 3)
            rcp, ocp, dens = [], [], []
            for s in range(2):
                den = stat_s()
                ve.tensor_copy(den[:], o_ps[s][HD:HD + 1, :])
                dens.append(den)
                oc = sb.tile([HD, R], BF16, tag="rbs", bufs=2,
                             name=f"{pfx}ocp")
                ve.tensor_copy(oc[:], o_ps[s][0:HD, :])
                ocp.append(oc)
            for s in range(2):
                rc = stat_s()
                ve.reciprocal_approx_fast(rc[:], dens[s][:])
                rcb = stat_s(BF16)
                ve.tensor_copy(rcb[:], rc[:])
                rcp.append(rcb)
            return rcp, ocp

        def norm_tail(hp, rcp, ocp):
            # PE broadcast + final scale, hidden under the next head-pair's
            # S matmuls
            for s in range(2):
                rb = ps.tile([P, R], F32, tag="mm", bufs=2, name=f"{pfx}rb")
                te.matmul(rb[0:HD, :], ones_t[0:1, 0:HD], rcp[s][:],
                          start=True, stop=True)
                if s == 0:
                    ve.tensor_mul(o_all_t[0:HD, hp, :], rb[0:HD, :],
                                  ocp[s][:])
                else:
                    tmp = sb.tile([HD, R], BF16, tag="otmp", bufs=2,
                                  name=f"{pfx}otmp")
                    ve.tensor_mul(tmp[:], rb[0:HD, :], ocp[s][:])
                    gdma(out=o_all_t[HD:P, hp, :], in_=tmp[:])

        kbox = [None]
        for s0 in k_steps(k_w, k_src, 0, k_bias, kbox, pfx + "k"):
            s0()
        k_cur = kbox[0]
        pending = None          # (hp, o_ps) awaiting normalization
        for hp in range(FT):
            # next head-pair's K projection as filler
            if hp + 1 < FT:
                nbox = [None]
                filler = k_steps(k_w, k_src, hp + 1, k_bias, nbox, pfx + "k")
            else:
                nbox = None
                filler = list(tail_filler)
            o_ps = [None, None]
            es_q = [None] * RT
            fi = 0
            for u in range(RT + 3):
                if u < RT:
                    st_t = ps.tile([P, 2 * R], F32, tag="st", bufs=2,
                                   name=f"{pfx}st")
                    for s in range(2):
                        off = HD * s
                        te.matmul(st_t[:, s * R:(s + 1) * R],
                                  k_cur[off:off + HD, u * P:(u + 1) * P],
                                  q_all_t[off:off + HD, hp, :],
                                  start=True, stop=True)
                    es = sb.tile([P, 2 * R], BF16, tag="es", bufs=4,
                                 name=f"{pfx}es")
                    se.activation(out=es[:], in_=st_t[:], func=AF.Exp,
                                  scale=0.125)
                    es_q[u] = es
                if u >= 3:
                    tko = u - 3
                    if tko == 0:
                        o_ps[0] = ps.tile([HD + 1, R], F32, tag="o", bufs=2,
                                          name=f"{pfx}o0")
                        o_ps[1] = ps.tile([HD + 1, R], F32, tag="o", bufs=2,
                                          name=f"{pfx}o1")
                    for s in range(2):
                        te.matmul(o_ps[s][:],
                                  v_tiles[tko][:, 2 * hp + s, :],
                                  es_q[tko][:, s * R:(s + 1) * R],
                                  start=(tko == 0), stop=(tko == RT - 1))
                # filler pacing: one pop at u4/u9 so the K psum slot (tag
                # "mm") recycles cleanly around norm_tail's rb matmuls —
                # the cc0 evict lands at u4 just before rb, and the cc1
                # psum allocates at u5 just after the rb/mul pair
                for _ in range(1 if u in (4, RT + 1) else 2):
                    if fi < len(filler):
                        filler[fi]()
                        fi += 1
                if u == 4 and pending is not None:
                    # norm_tail here gets ~5 units of PE cover for the
                    # reciprocal chain emitted at the last head-pair's end
                    norm_tail(*pending)
                    pending = None
            while fi < len(filler):
                filler[fi]()
                fi += 1
            rcp, ocp = norm_head(o_ps)
            pending = (hp, rcp, ocp)
            if nbox is not None:
                k_cur = nbox[0]
        norm_tail(*pending)

    o_all = act8()                            # slot 2
    attention(q_all, wk_t, ykv_all,
              (d["b_attn"][C:2 * C] if "b_attn" in d else None),
              v_sb, o_all, "sa", [])

    # ================= proj + residual, LN1 stats interleaved ========
    # ln_accum lags its producer by one tile so the sums matmuls hide
    # under the next output tile's matmul group instead of stalling on
    # the 3-op DVE eviction chain
    y1_all = act8()                           # slot 3
    ln1 = ln_begin()
    for mt in range(FT):
        psm = fm_group(wproj_t, mt, [o_all[:, kt, :] for kt in range(FT)],
                       bias_ap=d.get("b_proj"), bias_mt=mt)
        # lagged ln_accum BEFORE this tile's eviction: its x2 isn't queued
        # behind the new residual add on the DVE
        if mt > 0:
            ln_accum(ln1, y1_all[:, mt - 1, :], mt - 1)
        ve.tensor_add(y1_all[:, mt, :], psm[:], yq_all[:, mt, :])
    ln_accum(ln1, y1_all[:, FT - 1, :], FT - 1)

    # prefetch cross-attn K weights (wfm slot 0: wq dead) and the q2/cproj
    # weights (slots 1/2: wk and wproj are dead once proj is emitted), so
    # the LN1 seam and cross-attn aren't gated on weight DMAs
    wken_t = wfm_tile([P, FT, FT, P], "wken")
    dma(out=wken_t[:], in_=d["Wk_en"][:])
    wq2_t = wfm_tile([P, FT, FT, P], "wq2")       # wfm slot 1 (wk dead)
    dma(out=wq2_t[:], in_=d["W_q_p"][:])
    wcproj_t = wfm_tile([P, FT, FT, P], "wcp")    # wfm slot 2 (wproj dead)
    dma(out=wcproj_t[:], in_=d["W_cproj_p"][:])

    # ================= v2 generation (PE) overlapping LN1 (DVE) ======
    v2_sb = [None] * RT
    gen_v(xkv_all, wven_t, v2_sb,
          (d["b_en"][C:2 * C] if "b_en" in d else None), "w")

    ln_stats(ln1)
    y1n_all = act8()                          # slot 4
    for mt in range(FT):
        ln_apply(ln1, y1_all[:, mt, :], y1n_all[:, mt, :],
                 d.get("ln_w"), d.get("ln_b"), mt)

    # ================= q2 generation =================================
    q2_all = act8()                           # slot 5
    for mt in range(FT):
        psm = fm_group(wq2_t, mt, [y1n_all[:, kt, :] for kt in range(FT)],
                       bias_ap=d.get("b_q"), bias_mt=mt)
        ve.tensor_copy(q2_all[:, mt, :], psm[:])

    # prefetch first two FFN d1 weight chunks under cross-attention
    wd1_t = [None] * 4
    for ch in range(2):
        wd1_t[ch] = wrm_tile([P, FT, FT, P], f"wd1_{ch}")
        dma(out=wd1_t[ch][:],
            in_=d["W_d1_p"][:, ch * FT:(ch + 1) * FT, :, :])

    # ================= cross attention ===============================
    o2_all = act8()                           # slot 0 (yq dead)
    attention(q2_all, wken_t, xkv_all,
              (d["b_en"][0:C] if "b_en" in d else None),
              v2_sb, o2_all, "ca", [])

    # ================= cproj + residual (into y1n), LN2 ==============
    ln2 = ln_begin()
    for mt in range(FT):
        psm = fm_group(wcproj_t, mt, [o2_all[:, kt, :] for kt in range(FT)],
                       bias_ap=d.get("b_cproj"), bias_mt=mt)
        if mt > 0:
            ln_accum(ln2, y1n_all[:, mt - 1, :], mt - 1)
        ve.tensor_add(y1n_all[:, mt, :], psm[:], y1n_all[:, mt, :])
    ln_accum(ln2, y1n_all[:, FT - 1, :], FT - 1)
    ln_stats(ln2)
    xin_all = act8()                          # slot 1 (q dead)
    for mt in range(FT):
        ln_apply(ln2, y1n_all[:, mt, :], xin_all[:, mt, :],
                 d.get("ln1_w"), d.get("ln1_b"), mt)

    # ================= FFN ===========================================
    # F1: h = xin @ W_d1 (feature-major, 32 output ptiles in 4 weight chunks)
    ht_a = big_tile([P, 16, R], "ht_a")       # big slot 0 (ykv dead)
    ht_b = big_tile([P, 16, R], "ht_b")       # big slot 1 (xkv dead)

    # first d2 weight tile prefetches under F1
    wd2_t = [None] * FT
    wd2_t[0] = wfm_tile([P, 32, P], "wd2_0")
    dma(out=wd2_t[0][:], in_=d["W_d2_p"][:, 0, :, :])

    def ht_sl(kt):
        return (ht_a if kt < 16 else ht_b)[:, kt % 16, :]

    for ch in range(4):
        if wd1_t[ch] is None:
            wd1_t[ch] = wrm_tile([P, FT, FT, P], f"wd1_{ch}")
            dma(out=wd1_t[ch][:],
                in_=d["W_d1_p"][:, ch * FT:(ch + 1) * FT, :, :])
        w1 = wd1_t[ch]
        # prefetch chunk ch+2 while computing on ch
        nxt = ch + 2
        if nxt < 4 and wd1_t[nxt] is None:
            wd1_t[nxt] = wrm_tile([P, FT, FT, P], f"wd1_{nxt}")
            dma(out=wd1_t[nxt][:],
                in_=d["W_d1_p"][:, nxt * FT:(nxt + 1) * FT, :, :])
        for kk in range(FT):
            kt = ch * FT + kk
            psm = mm_ps()
            for ck in range(FT):
                te.matmul(psm[:], w1[:, kk, ck, :], xin_all[:, ck, :],
                          start=(ck == 0),
                          stop=(ck == FT - 1 and "b_d1" not in d))
            if "b_d1" in d:
                bias_fm(psm, d["b_d1"], kt, R)
            ve.tensor_copy(ht_sl(kt), psm[:])

    # F2: z = h @ W_d2 + xin, LN3 stats interleaved
    ln3 = ln_begin()
    z_all = act8()                            # slot 2 (o dead)
    for mt in range(FT):
        if wd2_t[mt] is None:
            wd2_t[mt] = wfm_tile([P, 32, P], f"wd2_{mt}")
            dma(out=wd2_t[mt][:], in_=d["W_d2_p"][:, mt, :, :])
        w2 = wd2_t[mt]
        if mt + 1 < FT and wd2_t[mt + 1] is None:
            wd2_t[mt + 1] = wfm_tile([P, 32, P], f"wd2_{mt + 1}")
            dma(out=wd2_t[mt + 1][:], in_=d["W_d2_p"][:, mt + 1, :, :])
        psm = mm_ps()
        for kt in range(32):
            te.matmul(psm[:], w2[:, kt, :], ht_sl(kt),
                      start=(kt == 0), stop=(kt == 31 and "b_d2" not in d))
        if "b_d2" in d:
            bias_fm(psm, d["b_d2"], mt, R)
        if mt > 0:
            ln_accum(ln3, z_all[:, mt - 1, :], mt - 1)
        ve.tensor_add(z_all[:, mt, :], psm[:], xin_all[:, mt, :])
    ln_accum(ln3, z_all[:, FT - 1, :], FT - 1)
    ln_stats(ln3)

    for mt in range(FT):
        ot = sb.tile([P, R], BF16, tag="outt", bufs=2, name="out_t")
        ln_apply(ln3, z_all[:, mt, :], ot[:],
                 d.get("ln2_w"), d.get("ln2_b"), mt)
        dma(out=d["out"][mt * P:(mt + 1) * P, :], in_=ot[:])

    sb.release()
    ps.release()


def _fm_pack(W, n_out):
    """[C, n_out] -> [P, mt, kt, P] bf16 so lhsT tiles are DMA-contiguous."""
    W = np.asarray(W, np.float32)
    kt = W.shape[0] // P
    mt = n_out // P
    return np.ascontiguousarray(
        W.reshape(kt, P, mt, P).transpose(1, 2, 0, 3).astype(NPBF))


def _rm_pack(W):
    """[C, n_out] -> [P, kt, n_out] bf16 (rhs layout for row-major linears)."""
    W = np.asarray(W, np.float32)
    kt = W.shape[0] // P
    return np.ascontiguousarray(
        W.reshape(kt, P, W.shape[1]).transpose(1, 0, 2).astype(NPBF))


def _build(flags):
    nc = bacc.Bacc(trn_type="TRN2", target_bir_lowering=False, debug=False)
    d = {}

    def din(name, shape, dt=BF16):
        d[name] = nc.declare_dram_parameter(name, list(shape), dt,
                                            isOutput=False).ap()

    din("yq", (C, R))
    din("ykv", (C, T))
    din("xkv", (C, T))
    din("Wq_attn", (P, FT, FT, P))
    din("Wk_attn", (P, FT, FT, P))
    din("Wv_attn", (P, FT, T))
    din("W_proj_p", (P, FT, FT, P))
    din("Wk_en", (P, FT, FT, P))
    din("Wv_en", (P, FT, T))
    din("W_q_p", (P, FT, FT, P))
    din("W_cproj_p", (P, FT, FT, P))
    din("W_d1_p", (P, 32, FT, P))
    din("W_d2_p", (P, FT, 32, P))
    din("ones", (P, R))
    din("ones_f", (1, P), dt=mybir.dt.float32r)
    for nm, shape in (("b_attn", (3 * C,)), ("b_proj", (C,)), ("b_en", (2 * C,)),
                      ("b_q", (C,)), ("b_cproj", (C,)), ("b_d1", (DFF,)),
                      ("b_d2", (C,))):
        if nm in flags:
            din(nm, shape, dt=F32)
    for nm in ("ln_w", "ln_b", "ln1_w", "ln1_b", "ln2_w", "ln2_b"):
        if nm in flags:
            din(nm, (C,), dt=F32)
    d["out"] = nc.declare_dram_parameter("out", [C, R], BF16,
                                         isOutput=True).ap()

    with tile.TileContext(nc) as tc:
        _emit(nc, tc, d, flags)
    nc.compile()
    return nc


def _flags_of(inputs):
    flags = set()
    for nm in ("b_attn", "b_proj", "b_en", "b_q", "b_cproj", "b_d1", "b_d2"):
        if np.any(np.asarray(inputs[nm]) != 0):
            flags.add(nm)
    for nm, triv in (("ln_w", 1.0), ("ln_b", 0.0), ("ln1_w", 1.0),
                     ("ln1_b", 0.0), ("ln2_w", 1.0), ("ln2_b", 0.0)):
        if np.any(np.asarray(inputs[nm]) != triv):
            flags.add(nm)
    for a, b in (("ln_w", "ln_b"), ("ln1_w", "ln1_b"), ("ln2_w", "ln2_b")):
        if a in flags or b in flags:
            flags.add(a)
            flags.add(b)
    return flags


def _make_in_maps(inputs):
    flags = _flags_of(inputs)
    x = np.asarray(inputs["x"], np.float32)
    y = np.asarray(inputs["y"], np.float32)
    W_attn = np.asarray(inputs["W_attn"], np.float32)
    W_en = np.asarray(inputs["W_en"], np.float32)
    base = {
        "Wq_attn": _fm_pack(W_attn[:, 0:C], C),
        "Wk_attn": _fm_pack(W_attn[:, C:2 * C], C),
        "Wv_attn": _rm_pack(W_attn[:, 2 * C:3 * C]),
        "W_proj_p": _fm_pack(inputs["W_proj"], C),
        "Wk_en": _fm_pack(W_en[:, 0:C], C),
        "Wv_en": _rm_pack(W_en[:, C:2 * C]),
        "W_q_p": _fm_pack(inputs["W_q"], C),
        "W_cproj_p": _fm_pack(inputs["W_cproj"], C),
        "W_d1_p": np.ascontiguousarray(
            np.asarray(inputs["W_d1"], np.float32)
            .reshape(FT, P, 32, P).transpose(1, 2, 0, 3).astype(NPBF)),
        "W_d2_p": np.ascontiguousarray(
            np.asarray(inputs["W_d2"], np.float32)
            .reshape(32, P, FT, P).transpose(1, 2, 0, 3).astype(NPBF)),
        "ones": np.ones((P, R), NPBF),
        "ones_f": np.ones((1, P), np.float32),
    }
    for nm in flags:
        base[nm] = np.ascontiguousarray(np.asarray(inputs[nm], np.float32))
    yT = [np.ascontiguousarray(y[b].T.astype(NPBF)) for b in range(B)]
    xT = [np.ascontiguousarray(x[b].T.astype(NPBF)) for b in range(B)]
    in_maps = []
    for c in range(NCORES):
        b, h = divmod(c, 2)
        m = dict(base)
        m["ykv"] = yT[b]
        m["xkv"] = xT[b]
        m["yq"] = np.ascontiguousarray(yT[b][:, h * R:(h + 1) * R])
        in_maps.append(m)
    return in_maps


def kernel(x, y, W_attn, b_attn, W_proj, b_proj, ln_w, ln_b,
           W_en, b_en, W_q, b_q, W_cproj, b_cproj,
           ln1_w, ln1_b, ln2_w, ln2_b, W_d1, b_d1, W_d2, b_d2):
    inputs = dict(x=x, y=y, W_attn=W_attn, b_attn=b_attn, W_proj=W_proj,
                  b_proj=b_proj, ln_w=ln_w, ln_b=ln_b, W_en=W_en, b_en=b_en,
                  W_q=W_q, b_q=b_q, W_cproj=W_cproj, b_cproj=b_cproj,
                  ln1_w=ln1_w, ln1_b=ln1_b, ln2_w=ln2_w, ln2_b=ln2_b,
                  W_d1=W_d1, b_d1=b_d1, W_d2=W_d2, b_d2=b_d2)
    flags = _flags_of(inputs)
    key = tuple(sorted(flags))
    if key not in _CACHE:
        _CACHE[key] = _build(flags)
    nc = _CACHE[key]

    in_maps = _make_in_maps(inputs)
    res = run_bass_kernel_spmd(nc, in_maps, list(range(NCORES)))
    out = np.empty((B, T, C), np.float32)
    for c in range(NCORES):
        b, h = divmod(c, 2)
        out[b, h * R:(h + 1) * R, :] = \
            np.asarray(res.results[c]["out"], np.float32).T
    return out


# revision 71
# speedup vs baseline: 1.0661x; 1.0051x over previous
"""Decoder block on 8 TRN2 NeuronCores — pipelined bf16 version.

Sharding: core c -> (batch b=c//2, half h=c%2): 512 query rows through the
full decoder; K/V span the full T=1024 of that batch element. All activations
feature-major [C, rows]; matmuls run as out = lhsT.T @ rhs in bf16 with fp32
PSUM accumulation.

Schedule: every engine's issue order is pinned to emission order; emission is
software-pipelined so the PE never waits: attention runs S(tk) / exp(tk-1) /
O(tk-2) with the next head-pair's K-projection matmuls as PE filler under the
ACT exp stream; weights are host-repacked to bf16 in DMA-contiguous layouts
and streamed >=1MB per transfer well ahead of use.
"""

import numpy as np
import ml_dtypes

import concourse.bass as bass
from bass_rust import add_dep_helper
import concourse.mybir as mybir
import concourse.tile as tile
from concourse import bacc
from concourse.bass_utils import run_bass_kernel_spmd

B, T, C, H = 4, 1024, 1024, 16
HD = C // H            # 64
DFF = 4096
EPS = 1e-5
P = 128
R = 512                # query rows per core
FT = C // P            # 8 feature ptiles
RT = T // P            # 8 key-row tiles
NCORES = 8

F32 = mybir.dt.float32
BF16 = mybir.dt.bfloat16
AF = mybir.ActivationFunctionType
NPBF = ml_dtypes.bfloat16

_CACHE = {}


def _emit(nc, tc, d, flags):
    sync = nc.sync
    ve = nc.vector
    se = nc.scalar
    te = nc.tensor
    ge = nc.gpsimd

    # Pin every engine's issue order to emission order (the emitted program
    # is a feasible sequential order by construction; this stops the Tile
    # scheduler from hoisting an instruction onto a busy slot).
    _last = {}

    def _chain(key, inst):
        prev = _last.get(key)
        if prev is not None:
            add_dep_helper(inst.ins, prev.ins, sync=False,
                           reason=f"{key} emission-order chain")
        _last[key] = inst
        return inst

    def dma(out, in_):
        return _chain("sp", sync.dma_start(out=out, in_=in_))

    def gdma(out, in_):
        return _chain("pool", ge.dma_start(out=out, in_=in_))

    class _Chained:
        def __init__(self, eng, key):
            self._eng = eng
            self._key = key

        def __getattr__(self, name):
            fn = getattr(self._eng, name)

            def wrapped(*a, **k):
                return _chain(self._key, fn(*a, **k))

            return wrapped

    ve = _Chained(ve, "dve")
    se = _Chained(se, "act")
    te = _Chained(te, "pe")

    sb = tc.alloc_tile_pool(name="sb", bufs=1)
    ps = tc.alloc_tile_pool(name="ps", bufs=1, space="PSUM")

    # ---- first input DMA ahead of the tiny constants: the q-projection
    # inputs gate the PE start
    yq_first = sb.tile([P, FT, R], BF16, tag="act8", bufs=5, name="act8")
    dma(out=yq_first[:], in_=d["yq"].rearrange("(kt p) r -> p kt r", p=P))

    # ---- constants ----
    ones_t = sb.tile([P, R], BF16, tag="ones", bufs=1, name="ones_t")
    dma(out=ones_t[:], in_=d["ones"][:, :])
    ones_f = sb.tile([1, P], mybir.dt.float32r, tag="onesf", bufs=1,
                     name="ones_f")
    dma(out=ones_f[:], in_=d["ones_f"][:, :])
    eps_t = sb.tile([1, 1], F32, tag="eps", bufs=1, name="eps_t")
    ve.memset(eps_t[:], EPS)
    invc_t = sb.tile([1, 1], F32, tag="eps2", bufs=1, name="invc_t")
    ve.memset(invc_t[:], 1.0 / C)

    # ---- tile makers -------------------------------------------------
    def act8():
        # activation sets [P, FT, R]; FIFO reuse order must respect liveness
        return sb.tile([P, FT, R], BF16, tag="act8", bufs=5, name="act8")

    def big_tile(shape, name):
        return sb.tile(shape, BF16, tag="big", bufs=2, name=name)

    def wfm_tile(shape, name):
        return sb.tile(shape, BF16, tag="wfm", bufs=3, name=name)

    def wrm_tile(shape, name):
        return sb.tile(shape, BF16, tag="wrm", bufs=2, name=name)

    def mm_ps():
        return ps.tile([P, R], F32, tag="mm", bufs=2, name="mm")

    def stat_s(dt=F32):
        # short-lived row stats (dead before the next three allocs of this tag)
        return sb.tile([1, R], dt, tag="statS", bufs=4, name="statS")

    def stat_l(dt=F32):
        # long-lived row stats (mu / rstd / mu*rstd within one layernorm)
        return sb.tile([1, R], dt, tag="statL", bufs=3, name="statL")

    # ---- optional-bias helpers (zero in the graded configuration) ----
    def bias_fm(psm, bias_ap, mt, n):
        """psm[P, n] += b[mt*P : mt*P+P] outer ones_n (feature-major)."""
        b_f = sb.tile([1, P], F32, tag="biaf", bufs=2, name="b_fmf")
        dma(out=b_f[:], in_=bias_ap[mt * P:(mt + 1) * P][None, :])
        b_t = sb.tile([1, P], BF16, tag="bia", bufs=2, name="b_fm")
        ve.tensor_copy(b_t[:], b_f[:])
        te.matmul(psm[:, 0:n], b_t[:], ones_t[0:1, 0:n], start=False, stop=True)

    def bias_rm(psm, bias_ap, cc):
        """psm[P, 512] += ones_col outer b[cc*512 : cc*512+512] (row-major)."""
        b_f = sb.tile([1, R], F32, tag="biaf", bufs=2, name="b_rmf")
        dma(out=b_f[:], in_=bias_ap[cc * R:(cc + 1) * R][None, :])
        b_t = sb.tile([1, R], BF16, tag="bia", bufs=2, name="b_rm")
        ve.tensor_copy(b_t[:], b_f[:])
        te.matmul(psm[:], ones_t[0:1, 0:P], b_t[:], start=False, stop=True)

    # ---- generic fm linear: one output ptile ------------------------
    def fm_group(w_tile, mt, rhs_slices, bias_ap=None, bias_mt=None):
        """psum[P, 512] = sum_kt w_tile[:, mt, kt, :].T @ rhs_slices[kt]."""
        psm = mm_ps()
        last = len(rhs_slices) - 1
        for kt, rhs in enumerate(rhs_slices):
            te.matmul(psm[:], w_tile[:, mt, kt, :], rhs,
                      start=(kt == 0), stop=(kt == last and bias_ap is None))
        if bias_ap is not None:
            bias_fm(psm, bias_ap, bias_mt if bias_mt is not None else mt, R)
        return psm

    # ================= layernorm (stats + normalize) =================
    # Stats are accumulated tile-by-tile as inputs appear (x2 mul + two
    # ones-matmuls); normalize uses PE broadcast of rstd / mu*rstd.
    class LNState:
        pass

    def ln_begin():
        st = LNState()
        st.sx = ps.tile([1, R], F32, tag="o", bufs=2, name="ln_sx")
        st.sx2 = ps.tile([1, R], F32, tag="o", bufs=2, name="ln_sx2")
        return st

    def ln_accum(st, x_sl, kt):
        # x^2 on the scalar engine — idle outside attention — so the sums
        # matmuls don't queue behind the DVE eviction chain
        x2 = sb.tile([P, R], BF16, tag="x2", bufs=2, name="x2")
        se.activation(out=x2[:], in_=x_sl, func=AF.Square)
        te.matmul(st.sx[:], ones_t[:, 0:1], x_sl,
                  start=(kt == 0), stop=(kt == FT - 1))
        te.matmul(st.sx2[:], ones_t[:, 0:1], x2[:],
                  start=(kt == 0), stop=(kt == FT - 1))

    def ln_stats(st):
        # mu is NEGATED so the apply step is x*rstd_b + (-mu*rstd)_b — a
        # commutative add with the PSUM operand in the proven first slot
        mu = stat_l()
        ve.tensor_scalar_mul(mu[:], st.sx[:], -1.0 / C)
        ex2 = stat_s()
        ve.tensor_scalar_mul(ex2[:], st.sx2[:], 1.0 / C)
        mu2 = stat_s()
        ve.tensor_mul(mu2[:], mu[:], mu[:])
        var = stat_s()
        ve.tensor_sub(var[:], ex2[:], mu2[:])
        # rstd = exp(-0.5 * ln(var + eps)) — keeps ACT on the ln/exp table set
        lnv = stat_s()
        se.activation(out=lnv[:], in_=var[:], func=AF.Ln,
                      bias=eps_t[0:1, 0:1], scale=1.0)
        rstd = stat_l(BF16)
        se.activation(out=rstd[:], in_=lnv[:], func=AF.Exp, scale=-0.5)
        # broadcast via PE into [P, R] (uses the idle "st" PSUM space);
        # applies read these straight from PSUM (PSUM-first operand)
        rb = ps.tile([P, R], F32, tag="st", bufs=2, name="ln_rb")
        te.matmul(rb[:], ones_t[0:1, 0:P], rstd[:], start=True, stop=True)
        rbs = sb.tile([P, R], BF16, tag="lnb", bufs=2, name="ln_rbs")
        ve.tensor_copy(rbs[:], rb[:])
        musr = stat_l(BF16)
        ve.tensor_mul(musr[:], mu[:], rstd[:])
        mb = ps.tile([P, R], F32, tag="st", bufs=2, name="ln_mb")
        te.matmul(mb[:], ones_t[0:1, 0:P], musr[:], start=True, stop=True)
        mbs = sb.tile([P, R], BF16, tag="lnb", bufs=2, name="ln_mbs")
        ve.tensor_copy(mbs[:], mb[:])
        st.rbs, st.mbs = rbs, mbs

    def ln_apply(st, x_sl, out_sl, w_ap, b_ap, kt, out_dt=BF16):
        tmp = sb.tile([P, R], BF16, tag="lntmp", bufs=2, name="lntmp")
        ve.tensor_mul(tmp[:], x_sl, st.rbs[:])
        if w_ap is None and b_ap is None:
            ve.tensor_add(out_sl, tmp[:], st.mbs[:])
        else:
            nrm = sb.tile([P, R], BF16, tag="lntmp", bufs=2, name="lnnrm")
            ve.tensor_add(nrm[:], tmp[:], st.mbs[:])
            w_t = sb.tile([P, 1], F32, tag="lnw", bufs=4, name="lnw")
            if w_ap is not None:
                dma(out=w_t[:], in_=w_ap[kt * P:(kt + 1) * P][:, None])
            else:
                ve.memset(w_t[:], 1.0)
            b_t = sb.tile([P, 1], F32, tag="lnw", bufs=4, name="lnb")
            if b_ap is not None:
                dma(out=b_t[:], in_=b_ap[kt * P:(kt + 1) * P][:, None])
            else:
                ve.memset(b_t[:], 0.0)
            ve.scalar_tensor_tensor(
                out_sl, nrm[:], w_t[:], b_t[:].to_broadcast((P, R)),
                op0=mybir.AluOpType.mult, op1=mybir.AluOpType.add)

    # =================================================================
    # Prologue DMAs (sp ring, in order of first use)
    # =================================================================
    # q-gen inputs first so the PE can start ~6us in (Wq arrives per-mt,
    # group 0 only needs the first slice); the rest stream underneath the
    # q/v projection work
    yq_all = yq_first                         # slot 0 (DMA'd above)
    wq_t = wfm_tile([P, FT, FT, P], "wq")     # wfm slot 0
    for mt in range(FT):
        dma(out=wq_t[:, mt, :, :], in_=d["Wq_attn"][:, mt, :, :])
    ykv_all = big_tile([P, FT, T], "ykv")     # big slot 0
    dma(out=ykv_all[:], in_=d["ykv"].rearrange("(kt p) r -> p kt r", p=P))
    wv_t = wrm_tile([P, FT, T], "wv")         # wrm slot 0
    dma(out=wv_t[:], in_=d["Wv_attn"][:])
    wk_t = wfm_tile([P, FT, FT, P], "wk")     # wfm slot 1
    dma(out=wk_t[:], in_=d["Wk_attn"][:])

    b_attn = d.get("b_attn")

    # ================= A: q generation (feature-major) ===============
    q_all = act8()                            # slot 1
    for mt in range(FT):
        psm = fm_group(wq_t, mt, [yq_all[:, kt, :] for kt in range(FT)],
                       bias_ap=b_attn, bias_mt=mt)
        ve.tensor_copy(q_all[:, mt, :], psm[:])

    # more prefetch: cross-attn inputs + proj weights
    xkv_all = big_tile([P, FT, T], "xkv")     # big slot 1
    dma(out=xkv_all[:], in_=d["xkv"].rearrange("(kt p) r -> p kt r", p=P))
    wven_t = wrm_tile([P, FT, T], "wven")     # wrm slot 1
    dma(out=wven_t[:], in_=d["Wv_en"][:])
    wproj_t = wfm_tile([P, FT, FT, P], "wproj")   # wfm slot 2
    dma(out=wproj_t[:], in_=d["W_proj_p"][:])

    # ================= B: v generation (row-major, + ones col) =======
    def gen_v(src_all, w_rm, v_tiles, bias_ap, pfx):
        for cc in range(2):
            for rt in range(RT):
                psm = mm_ps()
                for kt in range(FT):
                    te.matmul(psm[:],
                              src_all[:, kt, rt * P:(rt + 1) * P],
                              w_rm[:, kt, cc * R:(cc + 1) * R],
                              start=(kt == 0),
                              stop=(kt == FT - 1 and bias_ap is None))
                if bias_ap is not None:
                    bias_rm(psm, bias_ap, cc)
                if cc == 0:
                    v_tiles[rt] = sb.tile([P, H, HD + 1], BF16, tag="v65",
                                          bufs=RT, name=f"{pfx}{rt}")
                data = ve.tensor_copy(
                    v_tiles[rt][:, cc * 8:(cc + 1) * 8, 0:HD],
                    psm[:].rearrange("p (h d) -> p h d", d=HD))
                if cc == 0:
                    oc = ve.tensor_copy(v_tiles[rt][:, :, HD], ones_t[:, 0:H])
                    add_dep_helper(oc.ins, data.ins, sync=False,
                                   reason="ones col after v data (slot order)")

    v_sb = [None] * RT
    gen_v(ykv_all, wv_t,  v_sb,
          (d["b_attn"][2 * C:3 * C] if "b_attn" in d else None), "v")

    # ================= K-projection helper ===========================
    # k ptile hp covers heads (2hp, 2hp+1): features on partitions, T keys
    # free.  Emitted in 18 closures (16 matmuls + 2 evicts) so attention can
    # interleave them two-per-unit as PE filler under the exp stream.
    def k_steps(w_t, src_all, hp, bias_ap, box, pfx):
        steps = []
        psms = {}

        def mk_mm(cc, kt):
            def go():
                if kt == 0:
                    psms[cc] = mm_ps()
                te.matmul(psms[cc][:], w_t[:, hp, kt, :],
                          src_all[:, kt, cc * R:(cc + 1) * R],
                          start=(kt == 0),
                          stop=(kt == FT - 1 and bias_ap is None))
            return go

        def mk_ev(cc):
            def go():
                if bias_ap is not None:
                    bias_fm(psms[cc], bias_ap, hp, R)
                if cc == 0:
                    box[0] = sb.tile([P, T], BF16, tag="ksb", bufs=2,
                                     name=f"{pfx}{hp}")
                ve.tensor_copy(box[0][:, cc * R:(cc + 1) * R], psms[cc][:])
            return go

        for cc in range(2):
            for kt in range(FT):
                steps.append(mk_mm(cc, kt))
            steps.append(mk_ev(cc))
        return steps

    # ================= attention =====================================
    def attention(q_all_t, k_w, k_src, k_bias, v_tiles, o_all_t, pfx,
                  tail_filler):
        """tail_filler: list of closures run as PE filler during the last
        head-pair (which has no next-K to generate)."""

        def norm_head(o_ps):
            # DVE half of softmax normalization, emitted right after the
            # final O matmuls: reciprocal straight off the PSUM ones-row,
            # and the o halves staged into SBUF (so the later mul's PSUM
            # operand can be the PE-broadcast reciprocal instead)
            # ordering matched to each consumer's deadline: per-s den+oc
            # pairs free that o_ps slot for the next head-pair's O matmul
            # (unit 2); the reciprocal chains follow, landing just before
            # their rb matmuls (unit 3)
            rcp, ocp, dens = [], [], []
            for s in range(2):
                den = stat_s()
                ve.tensor_copy(den[:], o_ps[s][HD:HD + 1, :])
                dens.append(den)
                oc = sb.tile([HD, R], BF16, tag="rbs", bufs=2,
                             name=f"{pfx}ocp")
                ve.tensor_copy(oc[:], o_ps[s][0:HD, :])
                ocp.append(oc)
            for s in range(2):
                rc = stat_s()
                ve.reciprocal_approx_fast(rc[:], dens[s][:])
                rcb = stat_s(BF16)
                ve.tensor_copy(rcb[:], rc[:])
                rcp.append(rcb)
            return rcp, ocp

        def norm_tail(hp, rcp, ocp):
            # PE broadcast + final scale, hidden under the next head-pair's
            # S matmuls
            for s in range(2):
                rb = ps.tile([P, R], F32, tag="mm", bufs=2, name=f"{pfx}rb")
                te.matmul(rb[0:HD, :], ones_t[0:1, 0:HD], rcp[s][:],
                          start=True, stop=True)
                if s == 0:
                    ve.tensor_mul(o_all_t[0:HD, hp, :], rb[0:HD, :],
                                  ocp[s][:])
                else:
                    tmp = sb.tile([HD, R], BF16, tag="otmp", bufs=2,
                                  name=f"{pfx}otmp")
                    ve.tensor_mul(tmp[:], rb[0:HD, :], ocp[s][:])
                    gdma(out=o_all_t[HD:P, hp, :], in_=tmp[:])

        kbox = [None]
        for s0 in k_steps(k_w, k_src, 0, k_bias, kbox, pfx + "k"):
            s0()
        k_cur = kbox[0]
        pending = None          # (hp, o_ps) awaiting normalization
        for hp in range(FT):
            # next head-pair's K projection as filler
            if hp + 1 < FT:
                nbox = [None]
                filler = k_steps(k_w, k_src, hp + 1, k_bias, nbox, pfx + "k")
            else:
                nbox = None
                filler = list(tail_filler)
            o_ps = [None, None]
            es_q = [None] * RT
            fi = 0
            for u in range(RT + 3):
                if u < RT:
                    st_t = ps.tile([P, 2 * R], F32, tag="st", bufs=2,
                                   name=f"{pfx}st")
                    for s in range(2):
                        off = HD * s
                        te.matmul(st_t[:, s * R:(s + 1) * R],
                                  k_cur[off:off + HD, u * P:(u + 1) * P],
                                  q_all_t[off:off + HD, hp, :],
                                  start=True, stop=True)
                    es = sb.tile([P, 2 * R], BF16, tag="es", bufs=4,
                                 name=f"{pfx}es")
                    se.activation(out=es[:], in_=st_t[:], func=AF.Exp,
                                  scale=0.125)
                    es_q[u] = es
                if u >= 3:
                    tko = u - 3
                    if tko == 0:
                        o_ps[0] = ps.tile([HD + 1, R], F32, tag="o", bufs=2,
                                          name=f"{pfx}o0")
                        o_ps[1] = ps.tile([HD + 1, R], F32, tag="o", bufs=2,
                                          name=f"{pfx}o1")
                    for s in range(2):
                        te.matmul(o_ps[s][:],
                                  v_tiles[tko][:, 2 * hp + s, :],
                                  es_q[tko][:, s * R:(s + 1) * R],
                                  start=(tko == 0), stop=(tko == RT - 1))
                # filler pacing: one pop at u4/u9 so the K psum slot (tag
                # "mm") recycles cleanly around norm_tail's rb matmuls —
                # the cc0 evict lands at u4 just before rb, and the cc1
                # psum allocates at u5 just after the rb/mul pair
                for _ in range(1 if u in (4, RT + 1) else 2):
                    if fi < len(filler):
                        filler[fi]()
                        fi += 1
                if u == 4 and pending is not None:
                    # norm_tail here gets ~5 units of PE cover for the
                    # reciprocal chain emitted at the last head-pair's end
                    norm_tail(*pending)
                    pending = None
            while fi < len(filler):
                filler[fi]()
                fi += 1
            rcp, ocp = norm_head(o_ps)
            pending = (hp, rcp, ocp)
            if nbox is not None:
                k_cur = nbox[0]
        norm_tail(*pending)

    o_all = act8()                            # slot 2
    attention(q_all, wk_t, ykv_all,
              (d["b_attn"][C:2 * C] if "b_attn" in d else None),
              v_sb, o_all, "sa", [])

    # ================= proj + residual, LN1 stats interleaved ========
    # ln_accum lags its producer by one tile so the sums matmuls hide
    # under the next output tile's matmul group instead of stalling on
    # the 3-op DVE eviction chain
    y1_all = act8()                           # slot 3
    ln1 = ln_begin()
    for mt in range(FT):
        psm = fm_group(wproj_t, mt, [o_all[:, kt, :] for kt in range(FT)],
                       bias_ap=d.get("b_proj"), bias_mt=mt)
        # lagged ln_accum BEFORE this tile's eviction: its x2 isn't queued
        # behind the new residual add on the DVE
        if mt > 0:
            ln_accum(ln1, y1_all[:, mt - 1, :], mt - 1)
        ve.tensor_add(y1_all[:, mt, :], psm[:], yq_all[:, mt, :])
    ln_accum(ln1, y1_all[:, FT - 1, :], FT - 1)

    # prefetch cross-attn K weights (wfm slot 0: wq dead) and the q2/cproj
    # weights (slots 1/2: wk and wproj are dead once proj is emitted), so
    # the LN1 seam and cross-attn aren't gated on weight DMAs
    wken_t = wfm_tile([P, FT, FT, P], "wken")
    dma(out=wken_t[:], in_=d["Wk_en"][:])
    wq2_t = wfm_tile([P, FT, FT, P], "wq2")       # wfm slot 1 (wk dead)
    dma(out=wq2_t[:], in_=d["W_q_p"][:])
    wcproj_t = wfm_tile([P, FT, FT, P], "wcp")    # wfm slot 2 (wproj dead)
    dma(out=wcproj_t[:], in_=d["W_cproj_p"][:])

    # ================= v2 generation (PE) overlapping LN1 (DVE) ======
    v2_sb = [None] * RT
    gen_v(xkv_all, wven_t, v2_sb,
          (d["b_en"][C:2 * C] if "b_en" in d else None), "w")

    ln_stats(ln1)
    y1n_all = act8()                          # slot 4
    for mt in range(FT):
        ln_apply(ln1, y1_all[:, mt, :], y1n_all[:, mt, :],
                 d.get("ln_w"), d.get("ln_b"), mt)

    # ================= q2 generation =================================
    q2_all = act8()                           # slot 5
    for mt in range(FT):
        psm = fm_group(wq2_t, mt, [y1n_all[:, kt, :] for kt in range(FT)],
                       bias_ap=d.get("b_q"), bias_mt=mt)
        ve.tensor_copy(q2_all[:, mt, :], psm[:])

    # prefetch first two FFN d1 weight chunks under cross-attention
    wd1_t = [None] * 4
    for ch in range(2):
        wd1_t[ch] = wrm_tile([P, FT, FT, P], f"wd1_{ch}")
        dma(out=wd1_t[ch][:],
            in_=d["W_d1_p"][:, ch * FT:(ch + 1) * FT, :, :])

    # ================= cross attention ===============================
    o2_all = act8()                           # slot 0 (yq dead)
    attention(q2_all, wken_t, xkv_all,
              (d["b_en"][0:C] if "b_en" in d else None),
              v2_sb, o2_all, "ca", [])

    # ================= cproj + residual (into y1n), LN2 ==============
    ln2 = ln_begin()
    for mt in range(FT):
        psm = fm_group(wcproj_t, mt, [o2_all[:, kt, :] for kt in range(FT)],
                       bias_ap=d.get("b_cproj"), bias_mt=mt)
        if mt > 0:
            ln_accum(ln2, y1n_all[:, mt - 1, :], mt - 1)
        ve.tensor_add(y1n_all[:, mt, :], psm[:], y1n_all[:, mt, :])
    ln_accum(ln2, y1n_all[:, FT - 1, :], FT - 1)
    ln_stats(ln2)
    xin_all = act8()                          # slot 1 (q dead)
    for mt in range(FT):
        ln_apply(ln2, y1n_all[:, mt, :], xin_all[:, mt, :],
                 d.get("ln1_w"), d.get("ln1_b"), mt)

    # ================= FFN ===========================================
    # F1: h = xin @ W_d1 (feature-major, 32 output ptiles in 4 weight chunks)
    ht_a = big_tile([P, 16, R], "ht_a")       # big slot 0 (ykv dead)
    ht_b = big_tile([P, 16, R], "ht_b")       # big slot 1 (xkv dead)

    # first d2 weight tile prefetches under F1
    wd2_t = [None] * FT
    wd2_t[0] = wfm_tile([P, 32, P], "wd2_0")
    dma(out=wd2_t[0][:], in_=d["W_d2_p"][:, 0, :, :])

    def ht_sl(kt):
        return (ht_a if kt < 16 else ht_b)[:, kt % 16, :]

    for ch in range(4):
        if wd1_t[ch] is None:
            wd1_t[ch] = wrm_tile([P, FT, FT, P], f"wd1_{ch}")
            dma(out=wd1_t[ch][:],
                in_=d["W_d1_p"][:, ch * FT:(ch + 1) * FT, :, :])
        w1 = wd1_t[ch]
        # prefetch chunk ch+2 while computing on ch
        nxt = ch + 2
        if nxt < 4 and wd1_t[nxt] is None:
            wd1_t[nxt] = wrm_tile([P, FT, FT, P], f"wd1_{nxt}")
            dma(out=wd1_t[nxt][:],
                in_=d["W_d1_p"][:, nxt * FT:(nxt + 1) * FT, :, :])
        for kk in range(FT):
            kt = ch * FT + kk
            psm = mm_ps()
            for ck in range(FT):
                te.matmul(psm[:], w1[:, kk, ck, :], xin_all[:, ck, :],
                          start=(ck == 0),
                          stop=(ck == FT - 1 and "b_d1" not in d))
            if "b_d1" in d:
                bias_fm(psm, d["b_d1"], kt, R)
            ve.tensor_copy(ht_sl(kt), psm[:])

    # F2: z = h @ W_d2 + xin, LN3 stats interleaved
    ln3 = ln_begin()
    z_all = act8()                            # slot 2 (o dead)
    for mt in range(FT):
        if wd2_t[mt] is None:
            wd2_t[mt] = wfm_tile([P, 32, P], f"wd2_{mt}")
            dma(out=wd2_t[mt][:], in_=d["W_d2_p"][:, mt, :, :])
        w2 = wd2_t[mt]
        if mt + 1 < FT and wd2_t[mt + 1] is None:
            wd2_t[mt + 1] = wfm_tile([P, 32, P], f"wd2_{mt + 1}")
            dma(out=wd2_t[mt + 1][:], in_=d["W_d2_p"][:, mt + 1, :, :])
        psm = mm_ps()
        for kt in range(32):
            te.matmul(psm[:], w2[:, kt, :], ht_sl(kt),
                      start=(kt == 0), stop=(kt == 31 and "b_d2" not in d))
        if "b_d2" in d:
            bias_fm(psm, d["b_d2"], mt, R)
        if mt > 0:
            ln_accum(ln3, z_all[:, mt - 1, :], mt - 1)
        ve.tensor_add(z_all[:, mt, :], psm[:], xin_all[:, mt, :])
    ln_accum(ln3, z_all[:, FT - 1, :], FT - 1)
    ln_stats(ln3)

    for mt in range(FT):
        ot = sb.tile([P, R], BF16, tag="outt", bufs=2, name="out_t")
        ln_apply(ln3, z_all[:, mt, :], ot[:],
                 d.get("ln2_w"), d.get("ln2_b"), mt)
        dma(out=d["out"][mt * P:(mt + 1) * P, :], in_=ot[:])

    sb.release()
    ps.release()


def _fm_pack(W, n_out):
    """[C, n_out] -> [P, mt, kt, P] bf16 so lhsT tiles are DMA-contiguous."""
    W = np.asarray(W, np.float32)
    kt = W.shape[0] // P
    mt = n_out // P
    return np.ascontiguousarray(
        W.reshape(kt, P, mt, P).transpose(1, 2, 0, 3).astype(NPBF))


def _rm_pack(W):
    """[C, n_out] -> [P, kt, n_out] bf16 (rhs layout for row-major linears)."""
    W = np.asarray(W, np.float32)
    kt = W.shape[0] // P
    return np.ascontiguousarray(
        W.reshape(kt, P, W.shape[1]).transpose(1, 0, 2).astype(NPBF))


def _build(flags):
    nc = bacc.Bacc(trn_type="TRN2", target_bir_lowering=False, debug=False)
    d = {}

    def din(name, shape, dt=BF16):
        d[name] = nc.declare_dram_parameter(name, list(shape), dt,
                                            isOutput=False).ap()

    din("yq", (C, R))
    din("ykv", (C, T))
    din("xkv", (C, T))
    din("Wq_attn", (P, FT, FT, P))
    din("Wk_attn", (P, FT, FT, P))
    din("Wv_attn", (P, FT, T))
    din("W_proj_p", (P, FT, FT, P))
    din("Wk_en", (P, FT, FT, P))
    din("Wv_en", (P, FT, T))
    din("W_q_p", (P, FT, FT, P))
    din("W_cproj_p", (P, FT, FT, P))
    din("W_d1_p", (P, 32, FT, P))
    din("W_d2_p", (P, FT, 32, P))
    din("ones", (P, R))
    din("ones_f", (1, P), dt=mybir.dt.float32r)
    for nm, shape in (("b_attn", (3 * C,)), ("b_proj", (C,)), ("b_en", (2 * C,)),
                      ("b_q", (C,)), ("b_cproj", (C,)), ("b_d1", (DFF,)),
                      ("b_d2", (C,))):
        if nm in flags:
            din(nm, shape, dt=F32)
    for nm in ("ln_w", "ln_b", "ln1_w", "ln1_b", "ln2_w", "ln2_b"):
        if nm in flags:
            din(nm, (C,), dt=F32)
    d["out"] = nc.declare_dram_parameter("out", [C, R], BF16,
                                         isOutput=True).ap()

    with tile.TileContext(nc) as tc:
        _emit(nc, tc, d, flags)
    nc.compile()
    return nc


def _flags_of(inputs):
    flags = set()
    for nm in ("b_attn", "b_proj", "b_en", "b_q", "b_cproj", "b_d1", "b_d2"):
        if np.any(np.asarray(inputs[nm]) != 0):
            flags.add(nm)
    for nm, triv in (("ln_w", 1.0), ("ln_b", 0.0), ("ln1_w", 1.0),
                     ("ln1_b", 0.0), ("ln2_w", 1.0), ("ln2_b", 0.0)):
        if np.any(np.asarray(inputs[nm]) != triv):
            flags.add(nm)
    for a, b in (("ln_w", "ln_b"), ("ln1_w", "ln1_b"), ("ln2_w", "ln2_b")):
        if a in flags or b in flags:
            flags.add(a)
            flags.add(b)
    return flags


def _make_in_maps(inputs):
    flags = _flags_of(inputs)
    x = np.asarray(inputs["x"], np.float32)
    y = np.asarray(inputs["y"], np.float32)
    W_attn = np.asarray(inputs["W_attn"], np.float32)
    W_en = np.asarray(inputs["W_en"], np.float32)
    base = {
        "Wq_attn": _fm_pack(W_attn[:, 0:C], C),
        "Wk_attn": _fm_pack(W_attn[:, C:2 * C], C),
        "Wv_attn": _rm_pack(W_attn[:, 2 * C:3 * C]),
        "W_proj_p": _fm_pack(inputs["W_proj"], C),
        "Wk_en": _fm_pack(W_en[:, 0:C], C),
        "Wv_en": _rm_pack(W_en[:, C:2 * C]),
        "W_q_p": _fm_pack(inputs["W_q"], C),
        "W_cproj_p": _fm_pack(inputs["W_cproj"], C),
        "W_d1_p": np.ascontiguousarray(
            np.asarray(inputs["W_d1"], np.float32)
            .reshape(FT, P, 32, P).transpose(1, 2, 0, 3).astype(NPBF)),
        "W_d2_p": np.ascontiguousarray(
            np.asarray(inputs["W_d2"], np.float32)
            .reshape(32, P, FT, P).transpose(1, 2, 0, 3).astype(NPBF)),
        "ones": np.ones((P, R), NPBF),
        "ones_f": np.ones((1, P), np.float32),
    }
    for nm in flags:
        base[nm] = np.ascontiguousarray(np.asarray(inputs[nm], np.float32))
    yT = [np.ascontiguousarray(y[b].T.astype(NPBF)) for b in range(B)]
    xT = [np.ascontiguousarray(x[b].T.astype(NPBF)) for b in range(B)]
    in_maps = []
    for c in range(NCORES):
        b, h = divmod(c, 2)
        m = dict(base)
        m["ykv"] = yT[b]
        m["xkv"] = xT[b]
        m["yq"] = np.ascontiguousarray(yT[b][:, h * R:(h + 1) * R])
        in_maps.append(m)
    return in_maps


def kernel(x, y, W_attn, b_attn, W_proj, b_proj, ln_w, ln_b,
           W_en, b_en, W_q, b_q, W_cproj, b_cproj,
           ln1_w, ln1_b, ln2_w, ln2_b, W_d1, b_d1, W_d2, b_d2):
    inputs = dict(x=x, y=y, W_attn=W_attn, b_attn=b_attn, W_proj=W_proj,
                  b_proj=b_proj, ln_w=ln_w, ln_b=ln_b, W_en=W_en, b_en=b_en,
                  W_q=W_q, b_q=b_q, W_cproj=W_cproj, b_cproj=b_cproj,
                  ln1_w=ln1_w, ln1_b=ln1_b, ln2_w=ln2_w, ln2_b=ln2_b,
                  W_d1=W_d1, b_d1=b_d1, W_d2=W_d2, b_d2=b_d2)
    flags = _flags_of(inputs)
    key = tuple(sorted(flags))
    if key not in _CACHE:
        _CACHE[key] = _build(flags)
    nc = _CACHE[key]

    in_maps = _make_in_maps(inputs)
    res = run_bass_kernel_spmd(nc, in_maps, list(range(NCORES)))
    out = np.empty((B, T, C), np.float32)
    for c in range(NCORES):
        b, h = divmod(c, 2)
        out[b, h * R:(h + 1) * R, :] = \
            np.asarray(res.results[c]["out"], np.float32).T
    return out


# revision 72
# speedup vs baseline: 1.0751x; 1.0084x over previous
"""Decoder block on 8 TRN2 NeuronCores — pipelined bf16 version.

Sharding: core c -> (batch b=c//2, half h=c%2): 512 query rows through the
full decoder; K/V span the full T=1024 of that batch element. All activations
feature-major [C, rows]; matmuls run as out = lhsT.T @ rhs in bf16 with fp32
PSUM accumulation.

Schedule: every engine's issue order is pinned to emission order; emission is
software-pipelined so the PE never waits: attention runs S(tk) / exp(tk-1) /
O(tk-2) with the next head-pair's K-projection matmuls as PE filler under the
ACT exp stream; weights are host-repacked to bf16 in DMA-contiguous layouts
and streamed >=1MB per transfer well ahead of use.
"""

import numpy as np
import ml_dtypes

import concourse.bass as bass
from bass_rust import add_dep_helper
import concourse.mybir as mybir
import concourse.tile as tile
from concourse import bacc
from concourse.bass_utils import run_bass_kernel_spmd

B, T, C, H = 4, 1024, 1024, 16
HD = C // H            # 64
DFF = 4096
EPS = 1e-5
P = 128
R = 512                # query rows per core
FT = C // P            # 8 feature ptiles
RT = T // P            # 8 key-row tiles
NCORES = 8

F32 = mybir.dt.float32
BF16 = mybir.dt.bfloat16
AF = mybir.ActivationFunctionType
NPBF = ml_dtypes.bfloat16

_CACHE = {}


def _emit(nc, tc, d, flags):
    sync = nc.sync
    ve = nc.vector
    se = nc.scalar
    te = nc.tensor
    ge = nc.gpsimd

    # Pin every engine's issue order to emission order (the emitted program
    # is a feasible sequential order by construction; this stops the Tile
    # scheduler from hoisting an instruction onto a busy slot).
    _last = {}

    def _chain(key, inst):
        prev = _last.get(key)
        if prev is not None:
            add_dep_helper(inst.ins, prev.ins, sync=False,
                           reason=f"{key} emission-order chain")
        _last[key] = inst
        return inst

    def dma(out, in_):
        return _chain("sp", sync.dma_start(out=out, in_=in_))

    def gdma(out, in_):
        return _chain("pool", ge.dma_start(out=out, in_=in_))

    class _Chained:
        def __init__(self, eng, key):
            self._eng = eng
            self._key = key

        def __getattr__(self, name):
            fn = getattr(self._eng, name)

            def wrapped(*a, **k):
                return _chain(self._key, fn(*a, **k))

            return wrapped

    ve = _Chained(ve, "dve")
    se = _Chained(se, "act")
    te = _Chained(te, "pe")

    sb = tc.alloc_tile_pool(name="sb", bufs=1)
    ps = tc.alloc_tile_pool(name="ps", bufs=1, space="PSUM")

    # ---- first input DMA ahead of the tiny constants: the q-projection
    # inputs gate the PE start
    yq_first = sb.tile([P, FT, R], BF16, tag="act8", bufs=5, name="act8")
    dma(out=yq_first[:], in_=d["yq"].rearrange("(kt p) r -> p kt r", p=P))

    # ---- constants ----
    ones_t = sb.tile([P, R], BF16, tag="ones", bufs=1, name="ones_t")
    dma(out=ones_t[:], in_=d["ones"][:, :])
    ones_f = sb.tile([1, P], mybir.dt.float32r, tag="onesf", bufs=1,
                     name="ones_f")
    dma(out=ones_f[:], in_=d["ones_f"][:, :])
    eps_t = sb.tile([1, 1], F32, tag="eps", bufs=1, name="eps_t")
    ve.memset(eps_t[:], EPS)
    invc_t = sb.tile([1, 1], F32, tag="eps2", bufs=1, name="invc_t")
    ve.memset(invc_t[:], 1.0 / C)

    # ---- tile makers -------------------------------------------------
    def act8():
        # activation sets [P, FT, R]; FIFO reuse order must respect liveness
        return sb.tile([P, FT, R], BF16, tag="act8", bufs=5, name="act8")

    def big_tile(shape, name):
        return sb.tile(shape, BF16, tag="big", bufs=2, name=name)

    def wfm_tile(shape, name):
        return sb.tile(shape, BF16, tag="wfm", bufs=3, name=name)

    def wrm_tile(shape, name):
        return sb.tile(shape, BF16, tag="wrm", bufs=2, name=name)

    def mm_ps():
        return ps.tile([P, R], F32, tag="mm", bufs=2, name="mm")

    def stat_s(dt=F32):
        # short-lived row stats (dead before the next three allocs of this tag)
        return sb.tile([1, R], dt, tag="statS", bufs=4, name="statS")

    def stat_l(dt=F32):
        # long-lived row stats (mu / rstd / mu*rstd within one layernorm)
        return sb.tile([1, R], dt, tag="statL", bufs=3, name="statL")

    # ---- optional-bias helpers (zero in the graded configuration) ----
    def bias_fm(psm, bias_ap, mt, n):
        """psm[P, n] += b[mt*P : mt*P+P] outer ones_n (feature-major)."""
        b_f = sb.tile([1, P], F32, tag="biaf", bufs=2, name="b_fmf")
        dma(out=b_f[:], in_=bias_ap[mt * P:(mt + 1) * P][None, :])
        b_t = sb.tile([1, P], BF16, tag="bia", bufs=2, name="b_fm")
        ve.tensor_copy(b_t[:], b_f[:])
        te.matmul(psm[:, 0:n], b_t[:], ones_t[0:1, 0:n], start=False, stop=True)

    def bias_rm(psm, bias_ap, cc):
        """psm[P, 512] += ones_col outer b[cc*512 : cc*512+512] (row-major)."""
        b_f = sb.tile([1, R], F32, tag="biaf", bufs=2, name="b_rmf")
        dma(out=b_f[:], in_=bias_ap[cc * R:(cc + 1) * R][None, :])
        b_t = sb.tile([1, R], BF16, tag="bia", bufs=2, name="b_rm")
        ve.tensor_copy(b_t[:], b_f[:])
        te.matmul(psm[:], ones_t[0:1, 0:P], b_t[:], start=False, stop=True)

    # ---- generic fm linear: one output ptile ------------------------
    def fm_group(w_tile, mt, rhs_slices, bias_ap=None, bias_mt=None):
        """psum[P, 512] = sum_kt w_tile[:, mt, kt, :].T @ rhs_slices[kt]."""
        psm = mm_ps()
        last = len(rhs_slices) - 1
        for kt, rhs in enumerate(rhs_slices):
            te.matmul(psm[:], w_tile[:, mt, kt, :], rhs,
                      start=(kt == 0), stop=(kt == last and bias_ap is None))
        if bias_ap is not None:
            bias_fm(psm, bias_ap, bias_mt if bias_mt is not None else mt, R)
        return psm

    # ================= layernorm (stats + normalize) =================
    # Stats are accumulated tile-by-tile as inputs appear (x2 mul + two
    # ones-matmuls); normalize uses PE broadcast of rstd / mu*rstd.
    class LNState:
        pass

    def ln_begin():
        st = LNState()
        st.sx = ps.tile([1, R], F32, tag="o", bufs=2, name="ln_sx")
        st.sx2 = ps.tile([1, R], F32, tag="o", bufs=2, name="ln_sx2")
        return st

    def ln_accum(st, x_sl, kt):
        # x^2 on the scalar engine — idle outside attention — so the sums
        # matmuls don't queue behind the DVE eviction chain
        x2 = sb.tile([P, R], BF16, tag="x2", bufs=2, name="x2")
        se.activation(out=x2[:], in_=x_sl, func=AF.Square)
        te.matmul(st.sx[:], ones_t[:, 0:1], x_sl,
                  start=(kt == 0), stop=(kt == FT - 1))
        te.matmul(st.sx2[:], ones_t[:, 0:1], x2[:],
                  start=(kt == 0), stop=(kt == FT - 1))

    def ln_stats(st):
        # mu is NEGATED so the apply step is x*rstd_b + (-mu*rstd)_b — a
        # commutative add with the PSUM operand in the proven first slot
        # mu/ex2/mu2 on the scalar engine (empty at the seams, and closer
        # to PSUM) so the stats chain isn't queued behind DVE evictions
        mu = stat_l()
        se.activation(out=mu[:], in_=st.sx[:], func=AF.Copy, scale=-1.0 / C)
        ex2 = stat_s()
        se.activation(out=ex2[:], in_=st.sx2[:], func=AF.Copy, scale=1.0 / C)
        mu2 = stat_s()
        se.activation(out=mu2[:], in_=mu[:], func=AF.Square)
        var = stat_s()
        ve.tensor_sub(var[:], ex2[:], mu2[:])
        # rstd = exp(-0.5 * ln(var + eps)) — keeps ACT on the ln/exp table set
        lnv = stat_s()
        se.activation(out=lnv[:], in_=var[:], func=AF.Ln,
                      bias=eps_t[0:1, 0:1], scale=1.0)
        rstd = stat_l(BF16)
        se.activation(out=rstd[:], in_=lnv[:], func=AF.Exp, scale=-0.5)
        # broadcast via PE into [P, R] (uses the idle "st" PSUM space);
        # applies read these straight from PSUM (PSUM-first operand)
        rb = ps.tile([P, R], F32, tag="st", bufs=2, name="ln_rb")
        te.matmul(rb[:], ones_t[0:1, 0:P], rstd[:], start=True, stop=True)
        rbs = sb.tile([P, R], BF16, tag="lnb", bufs=2, name="ln_rbs")
        ve.tensor_copy(rbs[:], rb[:])
        musr = stat_l(BF16)
        ve.tensor_mul(musr[:], mu[:], rstd[:])
        mb = ps.tile([P, R], F32, tag="st", bufs=2, name="ln_mb")
        te.matmul(mb[:], ones_t[0:1, 0:P], musr[:], start=True, stop=True)
        mbs = sb.tile([P, R], BF16, tag="lnb", bufs=2, name="ln_mbs")
        ve.tensor_copy(mbs[:], mb[:])
        st.rbs, st.mbs = rbs, mbs

    def ln_apply(st, x_sl, out_sl, w_ap, b_ap, kt, out_dt=BF16):
        tmp = sb.tile([P, R], BF16, tag="lntmp", bufs=2, name="lntmp")
        ve.tensor_mul(tmp[:], x_sl, st.rbs[:])
        if w_ap is None and b_ap is None:
            ve.tensor_add(out_sl, tmp[:], st.mbs[:])
        else:
            nrm = sb.tile([P, R], BF16, tag="lntmp", bufs=2, name="lnnrm")
            ve.tensor_add(nrm[:], tmp[:], st.mbs[:])
            w_t = sb.tile([P, 1], F32, tag="lnw", bufs=4, name="lnw")
            if w_ap is not None:
                dma(out=w_t[:], in_=w_ap[kt * P:(kt + 1) * P][:, None])
            else:
                ve.memset(w_t[:], 1.0)
            b_t = sb.tile([P, 1], F32, tag="lnw", bufs=4, name="lnb")
            if b_ap is not None:
                dma(out=b_t[:], in_=b_ap[kt * P:(kt + 1) * P][:, None])
            else:
                ve.memset(b_t[:], 0.0)
            ve.scalar_tensor_tensor(
                out_sl, nrm[:], w_t[:], b_t[:].to_broadcast((P, R)),
                op0=mybir.AluOpType.mult, op1=mybir.AluOpType.add)

    # =================================================================
    # Prologue DMAs (sp ring, in order of first use)
    # =================================================================
    # q-gen inputs first so the PE can start ~6us in (Wq arrives per-mt,
    # group 0 only needs the first slice); the rest stream underneath the
    # q/v projection work
    yq_all = yq_first                         # slot 0 (DMA'd above)
    wq_t = wfm_tile([P, FT, FT, P], "wq")     # wfm slot 0
    for mt in range(FT):
        dma(out=wq_t[:, mt, :, :], in_=d["Wq_attn"][:, mt, :, :])
    ykv_all = big_tile([P, FT, T], "ykv")     # big slot 0
    dma(out=ykv_all[:], in_=d["ykv"].rearrange("(kt p) r -> p kt r", p=P))
    wv_t = wrm_tile([P, FT, T], "wv")         # wrm slot 0
    dma(out=wv_t[:], in_=d["Wv_attn"][:])
    wk_t = wfm_tile([P, FT, FT, P], "wk")     # wfm slot 1
    dma(out=wk_t[:], in_=d["Wk_attn"][:])

    b_attn = d.get("b_attn")

    # ================= A: q generation (feature-major) ===============
    q_all = act8()                            # slot 1
    for mt in range(FT):
        psm = fm_group(wq_t, mt, [yq_all[:, kt, :] for kt in range(FT)],
                       bias_ap=b_attn, bias_mt=mt)
        ve.tensor_copy(q_all[:, mt, :], psm[:])

    # more prefetch: cross-attn inputs + proj weights
    xkv_all = big_tile([P, FT, T], "xkv")     # big slot 1
    dma(out=xkv_all[:], in_=d["xkv"].rearrange("(kt p) r -> p kt r", p=P))
    wven_t = wrm_tile([P, FT, T], "wven")     # wrm slot 1
    dma(out=wven_t[:], in_=d["Wv_en"][:])
    wproj_t = wfm_tile([P, FT, FT, P], "wproj")   # wfm slot 2
    dma(out=wproj_t[:], in_=d["W_proj_p"][:])

    # ================= B: v generation (row-major, + ones col) =======
    def gen_v(src_all, w_rm, v_tiles, bias_ap, pfx):
        for cc in range(2):
            for rt in range(RT):
                psm = mm_ps()
                for kt in range(FT):
                    te.matmul(psm[:],
                              src_all[:, kt, rt * P:(rt + 1) * P],
                              w_rm[:, kt, cc * R:(cc + 1) * R],
                              start=(kt == 0),
                              stop=(kt == FT - 1 and bias_ap is None))
                if bias_ap is not None:
                    bias_rm(psm, bias_ap, cc)
                if cc == 0:
                    v_tiles[rt] = sb.tile([P, H, HD + 1], BF16, tag="v65",
                                          bufs=RT, name=f"{pfx}{rt}")
                data = ve.tensor_copy(
                    v_tiles[rt][:, cc * 8:(cc + 1) * 8, 0:HD],
                    psm[:].rearrange("p (h d) -> p h d", d=HD))
                if cc == 0:
                    oc = ve.tensor_copy(v_tiles[rt][:, :, HD], ones_t[:, 0:H])
                    add_dep_helper(oc.ins, data.ins, sync=False,
                                   reason="ones col after v data (slot order)")

    v_sb = [None] * RT
    gen_v(ykv_all, wv_t,  v_sb,
          (d["b_attn"][2 * C:3 * C] if "b_attn" in d else None), "v")

    # ================= K-projection helper ===========================
    # k ptile hp covers heads (2hp, 2hp+1): features on partitions, T keys
    # free.  Emitted in 18 closures (16 matmuls + 2 evicts) so attention can
    # interleave them two-per-unit as PE filler under the exp stream.
    def k_steps(w_t, src_all, hp, bias_ap, box, pfx):
        steps = []
        psms = {}

        def mk_mm(cc, kt):
            def go():
                if kt == 0:
                    psms[cc] = mm_ps()
                te.matmul(psms[cc][:], w_t[:, hp, kt, :],
                          src_all[:, kt, cc * R:(cc + 1) * R],
                          start=(kt == 0),
                          stop=(kt == FT - 1 and bias_ap is None))
            return go

        def mk_ev(cc):
            def go():
                if bias_ap is not None:
                    bias_fm(psms[cc], bias_ap, hp, R)
                if cc == 0:
                    box[0] = sb.tile([P, T], BF16, tag="ksb", bufs=2,
                                     name=f"{pfx}{hp}")
                ve.tensor_copy(box[0][:, cc * R:(cc + 1) * R], psms[cc][:])
            return go

        for cc in range(2):
            for kt in range(FT):
                steps.append(mk_mm(cc, kt))
            steps.append(mk_ev(cc))
        return steps

    # ================= attention =====================================
    def attention(q_all_t, k_w, k_src, k_bias, v_tiles, o_all_t, pfx,
                  tail_filler):
        """tail_filler: list of closures run as PE filler during the last
        head-pair (which has no next-K to generate)."""

        def norm_head(o_ps):
            # DVE half of softmax normalization, emitted right after the
            # final O matmuls: reciprocal straight off the PSUM ones-row,
            # and the o halves staged into SBUF (so the later mul's PSUM
            # operand can be the PE-broadcast reciprocal instead)
            # ordering matched to each consumer's deadline: per-s den+oc
            # pairs free that o_ps slot for the next head-pair's O matmul
            # (unit 2); the reciprocal chains follow, landing just before
            # their rb matmuls (unit 3)
            rcp, ocp, dens = [], [], []
            for s in range(2):
                den = stat_s()
                ve.tensor_copy(den[:], o_ps[s][HD:HD + 1, :])
                dens.append(den)
                oc = sb.tile([HD, R], BF16, tag="rbs", bufs=2,
                             name=f"{pfx}ocp")
                ve.tensor_copy(oc[:], o_ps[s][0:HD, :])
                ocp.append(oc)
            for s in range(2):
                rc = stat_s()
                ve.reciprocal_approx_fast(rc[:], dens[s][:])
                rcb = stat_s(BF16)
                ve.tensor_copy(rcb[:], rc[:])
                rcp.append(rcb)
            return rcp, ocp

        def norm_tail(hp, rcp, ocp):
            # PE broadcast + final scale, hidden under the next head-pair's
            # S matmuls
            for s in range(2):
                rb = ps.tile([P, R], F32, tag="mm", bufs=2, name=f"{pfx}rb")
                te.matmul(rb[0:HD, :], ones_t[0:1, 0:HD], rcp[s][:],
                          start=True, stop=True)
                if s == 0:
                    ve.tensor_mul(o_all_t[0:HD, hp, :], rb[0:HD, :],
                                  ocp[s][:])
                else:
                    tmp = sb.tile([HD, R], BF16, tag="otmp", bufs=2,
                                  name=f"{pfx}otmp")
                    ve.tensor_mul(tmp[:], rb[0:HD, :], ocp[s][:])
                    gdma(out=o_all_t[HD:P, hp, :], in_=tmp[:])

        kbox = [None]
        for s0 in k_steps(k_w, k_src, 0, k_bias, kbox, pfx + "k"):
            s0()
        k_cur = kbox[0]
        pending = None          # (hp, o_ps) awaiting normalization
        for hp in range(FT):
            # next head-pair's K projection as filler
            if hp + 1 < FT:
                nbox = [None]
                filler = k_steps(k_w, k_src, hp + 1, k_bias, nbox, pfx + "k")
            else:
                nbox = None
                filler = list(tail_filler)
            o_ps = [None, None]
            es_q = [None] * RT
            fi = 0
            for u in range(RT + 3):
                if u < RT:
                    st_t = ps.tile([P, 2 * R], F32, tag="st", bufs=2,
                                   name=f"{pfx}st")
                    for s in range(2):
                        off = HD * s
                        te.matmul(st_t[:, s * R:(s + 1) * R],
                                  k_cur[off:off + HD, u * P:(u + 1) * P],
                                  q_all_t[off:off + HD, hp, :],
                                  start=True, stop=True)
                    es = sb.tile([P, 2 * R], BF16, tag="es", bufs=4,
                                 name=f"{pfx}es")
                    se.activation(out=es[:], in_=st_t[:], func=AF.Exp,
                                  scale=0.125)
                    es_q[u] = es
                if u >= 3:
                    tko = u - 3
                    if tko == 0:
                        o_ps[0] = ps.tile([HD + 1, R], F32, tag="o", bufs=2,
                                          name=f"{pfx}o0")
                        o_ps[1] = ps.tile([HD + 1, R], F32, tag="o", bufs=2,
                                          name=f"{pfx}o1")
                    for s in range(2):
                        te.matmul(o_ps[s][:],
                                  v_tiles[tko][:, 2 * hp + s, :],
                                  es_q[tko][:, s * R:(s + 1) * R],
                                  start=(tko == 0), stop=(tko == RT - 1))
                # filler pacing: one pop at u4/u9 so the K psum slot (tag
                # "mm") recycles cleanly around norm_tail's rb matmuls —
                # the cc0 evict lands at u4 just before rb, and the cc1
                # psum allocates at u5 just after the rb/mul pair
                for _ in range(1 if u in (4, RT + 1) else 2):
                    if fi < len(filler):
                        filler[fi]()
                        fi += 1
                if u == 4 and pending is not None:
                    # norm_tail here gets ~5 units of PE cover for the
                    # reciprocal chain emitted at the last head-pair's end
                    norm_tail(*pending)
                    pending = None
            while fi < len(filler):
                filler[fi]()
                fi += 1
            rcp, ocp = norm_head(o_ps)
            pending = (hp, rcp, ocp)
            if nbox is not None:
                k_cur = nbox[0]
        norm_tail(*pending)

    o_all = act8()                            # slot 2
    attention(q_all, wk_t, ykv_all,
              (d["b_attn"][C:2 * C] if "b_attn" in d else None),
              v_sb, o_all, "sa", [])

    # ================= proj + residual, LN1 stats interleaved ========
    # ln_accum lags its producer by one tile so the sums matmuls hide
    # under the next output tile's matmul group instead of stalling on
    # the 3-op DVE eviction chain
    y1_all = act8()                           # slot 3
    ln1 = ln_begin()
    for mt in range(FT):
        psm = fm_group(wproj_t, mt, [o_all[:, kt, :] for kt in range(FT)],
                       bias_ap=d.get("b_proj"), bias_mt=mt)
        # lagged ln_accum BEFORE this tile's eviction: its x2 isn't queued
        # behind the new residual add on the DVE
        if mt > 0:
            ln_accum(ln1, y1_all[:, mt - 1, :], mt - 1)
        ve.tensor_add(y1_all[:, mt, :], psm[:], yq_all[:, mt, :])
    ln_accum(ln1, y1_all[:, FT - 1, :], FT - 1)

    # prefetch cross-attn K weights (wfm slot 0: wq dead) and the q2/cproj
    # weights (slots 1/2: wk and wproj are dead once proj is emitted), so
    # the LN1 seam and cross-attn aren't gated on weight DMAs
    wken_t = wfm_tile([P, FT, FT, P], "wken")
    dma(out=wken_t[:], in_=d["Wk_en"][:])
    wq2_t = wfm_tile([P, FT, FT, P], "wq2")       # wfm slot 1 (wk dead)
    dma(out=wq2_t[:], in_=d["W_q_p"][:])
    wcproj_t = wfm_tile([P, FT, FT, P], "wcp")    # wfm slot 2 (wproj dead)
    dma(out=wcproj_t[:], in_=d["W_cproj_p"][:])

    # ================= v2 generation (PE) overlapping LN1 (DVE) ======
    v2_sb = [None] * RT
    gen_v(xkv_all, wven_t, v2_sb,
          (d["b_en"][C:2 * C] if "b_en" in d else None), "w")

    ln_stats(ln1)
    y1n_all = act8()                          # slot 4
    for mt in range(FT):
        ln_apply(ln1, y1_all[:, mt, :], y1n_all[:, mt, :],
                 d.get("ln_w"), d.get("ln_b"), mt)

    # ================= q2 generation =================================
    q2_all = act8()                           # slot 5
    for mt in range(FT):
        psm = fm_group(wq2_t, mt, [y1n_all[:, kt, :] for kt in range(FT)],
                       bias_ap=d.get("b_q"), bias_mt=mt)
        ve.tensor_copy(q2_all[:, mt, :], psm[:])

    # prefetch first two FFN d1 weight chunks under cross-attention
    wd1_t = [None] * 4
    for ch in range(2):
        wd1_t[ch] = wrm_tile([P, FT, FT, P], f"wd1_{ch}")
        dma(out=wd1_t[ch][:],
            in_=d["W_d1_p"][:, ch * FT:(ch + 1) * FT, :, :])

    # ================= cross attention ===============================
    o2_all = act8()                           # slot 0 (yq dead)
    attention(q2_all, wken_t, xkv_all,
              (d["b_en"][0:C] if "b_en" in d else None),
              v2_sb, o2_all, "ca", [])

    # ================= cproj + residual (into y1n), LN2 ==============
    ln2 = ln_begin()
    for mt in range(FT):
        psm = fm_group(wcproj_t, mt, [o2_all[:, kt, :] for kt in range(FT)],
                       bias_ap=d.get("b_cproj"), bias_mt=mt)
        if mt > 0:
            ln_accum(ln2, y1n_all[:, mt - 1, :], mt - 1)
        ve.tensor_add(y1n_all[:, mt, :], psm[:], y1n_all[:, mt, :])
    ln_accum(ln2, y1n_all[:, FT - 1, :], FT - 1)
    ln_stats(ln2)
    xin_all = act8()                          # slot 1 (q dead)
    for mt in range(FT):
        ln_apply(ln2, y1n_all[:, mt, :], xin_all[:, mt, :],
                 d.get("ln1_w"), d.get("ln1_b"), mt)

    # ================= FFN ===========================================
    # F1: h = xin @ W_d1 (feature-major, 32 output ptiles in 4 weight chunks)
    ht_a = big_tile([P, 16, R], "ht_a")       # big slot 0 (ykv dead)
    ht_b = big_tile([P, 16, R], "ht_b")       # big slot 1 (xkv dead)

    # first d2 weight tile prefetches under F1
    wd2_t = [None] * FT
    wd2_t[0] = wfm_tile([P, 32, P], "wd2_0")
    dma(out=wd2_t[0][:], in_=d["W_d2_p"][:, 0, :, :])

    def ht_sl(kt):
        return (ht_a if kt < 16 else ht_b)[:, kt % 16, :]

    for ch in range(4):
        if wd1_t[ch] is None:
            wd1_t[ch] = wrm_tile([P, FT, FT, P], f"wd1_{ch}")
            dma(out=wd1_t[ch][:],
                in_=d["W_d1_p"][:, ch * FT:(ch + 1) * FT, :, :])
        w1 = wd1_t[ch]
        # prefetch chunk ch+2 while computing on ch
        nxt = ch + 2
        if nxt < 4 and wd1_t[nxt] is None:
            wd1_t[nxt] = wrm_tile([P, FT, FT, P], f"wd1_{nxt}")
            dma(out=wd1_t[nxt][:],
                in_=d["W_d1_p"][:, nxt * FT:(nxt + 1) * FT, :, :])
        for kk in range(FT):
            kt = ch * FT + kk
            psm = mm_ps()
            for ck in range(FT):
                te.matmul(psm[:], w1[:, kk, ck, :], xin_all[:, ck, :],
                          start=(ck == 0),
                          stop=(ck == FT - 1 and "b_d1" not in d))
            if "b_d1" in d:
                bias_fm(psm, d["b_d1"], kt, R)
            ve.tensor_copy(ht_sl(kt), psm[:])

    # F2: z = h @ W_d2 + xin, LN3 stats interleaved
    ln3 = ln_begin()
    z_all = act8()                            # slot 2 (o dead)
    for mt in range(FT):
        if wd2_t[mt] is None:
            wd2_t[mt] = wfm_tile([P, 32, P], f"wd2_{mt}")
            dma(out=wd2_t[mt][:], in_=d["W_d2_p"][:, mt, :, :])
        w2 = wd2_t[mt]
        if mt + 1 < FT and wd2_t[mt + 1] is None:
            wd2_t[mt + 1] = wfm_tile([P, 32, P], f"wd2_{mt + 1}")
            dma(out=wd2_t[mt + 1][:], in_=d["W_d2_p"][:, mt + 1, :, :])
        psm = mm_ps()
        for kt in range(32):
            te.matmul(psm[:], w2[:, kt, :], ht_sl(kt),
                      start=(kt == 0), stop=(kt == 31 and "b_d2" not in d))
        if "b_d2" in d:
            bias_fm(psm, d["b_d2"], mt, R)
        if mt > 0:
            ln_accum(ln3, z_all[:, mt - 1, :], mt - 1)
        ve.tensor_add(z_all[:, mt, :], psm[:], xin_all[:, mt, :])
    ln_accum(ln3, z_all[:, FT - 1, :], FT - 1)
    ln_stats(ln3)

    for mt in range(FT):
        ot = sb.tile([P, R], BF16, tag="outt", bufs=2, name="out_t")
        ln_apply(ln3, z_all[:, mt, :], ot[:],
                 d.get("ln2_w"), d.get("ln2_b"), mt)
        dma(out=d["out"][mt * P:(mt + 1) * P, :], in_=ot[:])

    sb.release()
    ps.release()


def _fm_pack(W, n_out):
    """[C, n_out] -> [P, mt, kt, P] bf16 so lhsT tiles are DMA-contiguous."""
    W = np.asarray(W, np.float32)
    kt = W.shape[0] // P
    mt = n_out // P
    return np.ascontiguousarray(
        W.reshape(kt, P, mt, P).transpose(1, 2, 0, 3).astype(NPBF))


def _rm_pack(W):
    """[C, n_out] -> [P, kt, n_out] bf16 (rhs layout for row-major linears)."""
    W = np.asarray(W, np.float32)
    kt = W.shape[0] // P
    return np.ascontiguousarray(
        W.reshape(kt, P, W.shape[1]).transpose(1, 0, 2).astype(NPBF))


def _build(flags):
    nc = bacc.Bacc(trn_type="TRN2", target_bir_lowering=False, debug=False)
    d = {}

    def din(name, shape, dt=BF16):
        d[name] = nc.declare_dram_parameter(name, list(shape), dt,
                                            isOutput=False).ap()

    din("yq", (C, R))
    din("ykv", (C, T))
    din("xkv", (C, T))
    din("Wq_attn", (P, FT, FT, P))
    din("Wk_attn", (P, FT, FT, P))
    din("Wv_attn", (P, FT, T))
    din("W_proj_p", (P, FT, FT, P))
    din("Wk_en", (P, FT, FT, P))
    din("Wv_en", (P, FT, T))
    din("W_q_p", (P, FT, FT, P))
    din("W_cproj_p", (P, FT, FT, P))
    din("W_d1_p", (P, 32, FT, P))
    din("W_d2_p", (P, FT, 32, P))
    din("ones", (P, R))
    din("ones_f", (1, P), dt=mybir.dt.float32r)
    for nm, shape in (("b_attn", (3 * C,)), ("b_proj", (C,)), ("b_en", (2 * C,)),
                      ("b_q", (C,)), ("b_cproj", (C,)), ("b_d1", (DFF,)),
                      ("b_d2", (C,))):
        if nm in flags:
            din(nm, shape, dt=F32)
    for nm in ("ln_w", "ln_b", "ln1_w", "ln1_b", "ln2_w", "ln2_b"):
        if nm in flags:
            din(nm, (C,), dt=F32)
    d["out"] = nc.declare_dram_parameter("out", [C, R], BF16,
                                         isOutput=True).ap()

    with tile.TileContext(nc) as tc:
        _emit(nc, tc, d, flags)
    nc.compile()
    return nc


def _flags_of(inputs):
    flags = set()
    for nm in ("b_attn", "b_proj", "b_en", "b_q", "b_cproj", "b_d1", "b_d2"):
        if np.any(np.asarray(inputs[nm]) != 0):
            flags.add(nm)
    for nm, triv in (("ln_w", 1.0), ("ln_b", 0.0), ("ln1_w", 1.0),
                     ("ln1_b", 0.0), ("ln2_w", 1.0), ("ln2_b", 0.0)):
        if np.any(np.asarray(inputs[nm]) != triv):
            flags.add(nm)
    for a, b in (("ln_w", "ln_b"), ("ln1_w", "ln1_b"), ("ln2_w", "ln2_b")):
        if a in flags or b in flags:
            flags.add(a)
            flags.add(b)
    return flags


def _make_in_maps(inputs):
    flags = _flags_of(inputs)
    x = np.asarray(inputs["x"], np.float32)
    y = np.asarray(inputs["y"], np.float32)
    W_attn = np.asarray(inputs["W_attn"], np.float32)
    W_en = np.asarray(inputs["W_en"], np.float32)
    base = {
        "Wq_attn": _fm_pack(W_attn[:, 0:C], C),
        "Wk_attn": _fm_pack(W_attn[:, C:2 * C], C),
        "Wv_attn": _rm_pack(W_attn[:, 2 * C:3 * C]),
        "W_proj_p": _fm_pack(inputs["W_proj"], C),
        "Wk_en": _fm_pack(W_en[:, 0:C], C),
        "Wv_en": _rm_pack(W_en[:, C:2 * C]),
        "W_q_p": _fm_pack(inputs["W_q"], C),
        "W_cproj_p": _fm_pack(inputs["W_cproj"], C),
        "W_d1_p": np.ascontiguousarray(
            np.asarray(inputs["W_d1"], np.float32)
            .reshape(FT, P, 32, P).transpose(1, 2, 0, 3).astype(NPBF)),
        "W_d2_p": np.ascontiguousarray(
            np.asarray(inputs["W_d2"], np.float32)
            .reshape(32, P, FT, P).transpose(1, 2, 0, 3).astype(NPBF)),
        "ones": np.ones((P, R), NPBF),
        "ones_f": np.ones((1, P), np.float32),
    }
    for nm in flags:
        base[nm] = np.ascontiguousarray(np.asarray(inputs[nm], np.float32))
    yT = [np.ascontiguousarray(y[b].T.astype(NPBF)) for b in range(B)]
    xT = [np.ascontiguousarray(x[b].T.astype(NPBF)) for b in range(B)]
    in_maps = []
    for c in range(NCORES):
        b, h = divmod(c, 2)
        m = dict(base)
        m["ykv"] = yT[b]
        m["xkv"] = xT[b]
        m["yq"] = np.ascontiguousarray(yT[b][:, h * R:(h + 1) * R])
        in_maps.append(m)
    return in_maps


def kernel(x, y, W_attn, b_attn, W_proj, b_proj, ln_w, ln_b,
           W_en, b_en, W_q, b_q, W_cproj, b_cproj,
           ln1_w, ln1_b, ln2_w, ln2_b, W_d1, b_d1, W_d2, b_d2):
    inputs = dict(x=x, y=y, W_attn=W_attn, b_attn=b_attn, W_proj=W_proj,
                  b_proj=b_proj, ln_w=ln_w, ln_b=ln_b, W_en=W_en, b_en=b_en,
                  W_q=W_q, b_q=b_q, W_cproj=W_cproj, b_cproj=b_cproj,
                  ln1_w=ln1_w, ln1_b=ln1_b, ln2_w=ln2_w, ln2_b=ln2_b,
                  W_d1=W_d1, b_d1=b_d1, W_d2=W_d2, b_d2=b_d2)
    flags = _flags_of(inputs)
    key = tuple(sorted(flags))
    if key not in _CACHE:
        _CACHE[key] = _build(flags)
    nc = _CACHE[key]

    in_maps = _make_in_maps(inputs)
    res = run_bass_kernel_spmd(nc, in_maps, list(range(NCORES)))
    out = np.empty((B, T, C), np.float32)
    for c in range(NCORES):
        b, h = divmod(c, 2)
        out[b, h * R:(h + 1) * R, :] = \
            np.asarray(res.results[c]["out"], np.float32).T
    return out


# revision 73
# speedup vs baseline: 1.0870x; 1.0110x over previous
"""Decoder block on 8 TRN2 NeuronCores — pipelined bf16 version.

Sharding: core c -> (batch b=c//2, half h=c%2): 512 query rows through the
full decoder; K/V span the full T=1024 of that batch element. All activations
feature-major [C, rows]; matmuls run as out = lhsT.T @ rhs in bf16 with fp32
PSUM accumulation.

Schedule: every engine's issue order is pinned to emission order; emission is
software-pipelined so the PE never waits: attention runs S(tk) / exp(tk-1) /
O(tk-2) with the next head-pair's K-projection matmuls as PE filler under the
ACT exp stream; weights are host-repacked to bf16 in DMA-contiguous layouts
and streamed >=1MB per transfer well ahead of use.
"""

import numpy as np
import ml_dtypes

import concourse.bass as bass
from bass_rust import add_dep_helper
import concourse.mybir as mybir
import concourse.tile as tile
from concourse import bacc
from concourse.bass_utils import run_bass_kernel_spmd

B, T, C, H = 4, 1024, 1024, 16
HD = C // H            # 64
DFF = 4096
EPS = 1e-5
P = 128
R = 512                # query rows per core
FT = C // P            # 8 feature ptiles
RT = T // P            # 8 key-row tiles
NCORES = 8

F32 = mybir.dt.float32
BF16 = mybir.dt.bfloat16
AF = mybir.ActivationFunctionType
NPBF = ml_dtypes.bfloat16

_CACHE = {}


def _emit(nc, tc, d, flags):
    sync = nc.sync
    ve = nc.vector
    se = nc.scalar
    te = nc.tensor
    ge = nc.gpsimd

    # Pin every engine's issue order to emission order (the emitted program
    # is a feasible sequential order by construction; this stops the Tile
    # scheduler from hoisting an instruction onto a busy slot).
    _last = {}

    def _chain(key, inst):
        prev = _last.get(key)
        if prev is not None:
            add_dep_helper(inst.ins, prev.ins, sync=False,
                           reason=f"{key} emission-order chain")
        _last[key] = inst
        return inst

    def dma(out, in_):
        return _chain("sp", sync.dma_start(out=out, in_=in_))

    def gdma(out, in_):
        return _chain("pool", ge.dma_start(out=out, in_=in_))

    class _Chained:
        def __init__(self, eng, key):
            self._eng = eng
            self._key = key

        def __getattr__(self, name):
            fn = getattr(self._eng, name)

            def wrapped(*a, **k):
                return _chain(self._key, fn(*a, **k))

            return wrapped

    ve = _Chained(ve, "dve")
    se = _Chained(se, "act")
    te = _Chained(te, "pe")

    sb = tc.alloc_tile_pool(name="sb", bufs=1)
    ps = tc.alloc_tile_pool(name="ps", bufs=1, space="PSUM")

    # ---- first input DMA ahead of the tiny constants: the q-projection
    # inputs gate the PE start
    yq_first = sb.tile([P, FT, R], BF16, tag="act8", bufs=5, name="act8")
    dma(out=yq_first[:], in_=d["yq"].rearrange("(kt p) r -> p kt r", p=P))

    # ---- constants ----
    ones_t = sb.tile([P, R], BF16, tag="ones", bufs=1, name="ones_t")
    dma(out=ones_t[:], in_=d["ones"][:, :])
    ones_f = sb.tile([1, P], mybir.dt.float32r, tag="onesf", bufs=1,
                     name="ones_f")
    dma(out=ones_f[:], in_=d["ones_f"][:, :])
    eps_t = sb.tile([1, 1], F32, tag="eps", bufs=1, name="eps_t")
    ve.memset(eps_t[:], EPS)
    invc_t = sb.tile([1, 1], F32, tag="eps2", bufs=1, name="invc_t")
    ve.memset(invc_t[:], 1.0 / C)

    # ---- tile makers -------------------------------------------------
    def act8():
        # activation sets [P, FT, R]; FIFO reuse order must respect liveness
        return sb.tile([P, FT, R], BF16, tag="act8", bufs=5, name="act8")

    def big_tile(shape, name):
        return sb.tile(shape, BF16, tag="big", bufs=2, name=name)

    def wfm_tile(shape, name):
        return sb.tile(shape, BF16, tag="wfm", bufs=3, name=name)

    def wrm_tile(shape, name):
        return sb.tile(shape, BF16, tag="wrm", bufs=2, name=name)

    def mm_ps():
        return ps.tile([P, R], F32, tag="mm", bufs=2, name="mm")

    def stat_s(dt=F32):
        # short-lived row stats (dead before the next three allocs of this tag)
        return sb.tile([1, R], dt, tag="statS", bufs=4, name="statS")

    def stat_l(dt=F32):
        # long-lived row stats (mu / rstd / mu*rstd within one layernorm)
        return sb.tile([1, R], dt, tag="statL", bufs=3, name="statL")

    # ---- optional-bias helpers (zero in the graded configuration) ----
    def bias_fm(psm, bias_ap, mt, n):
        """psm[P, n] += b[mt*P : mt*P+P] outer ones_n (feature-major)."""
        b_f = sb.tile([1, P], F32, tag="biaf", bufs=2, name="b_fmf")
        dma(out=b_f[:], in_=bias_ap[mt * P:(mt + 1) * P][None, :])
        b_t = sb.tile([1, P], BF16, tag="bia", bufs=2, name="b_fm")
        ve.tensor_copy(b_t[:], b_f[:])
        te.matmul(psm[:, 0:n], b_t[:], ones_t[0:1, 0:n], start=False, stop=True)

    def bias_rm(psm, bias_ap, cc):
        """psm[P, 512] += ones_col outer b[cc*512 : cc*512+512] (row-major)."""
        b_f = sb.tile([1, R], F32, tag="biaf", bufs=2, name="b_rmf")
        dma(out=b_f[:], in_=bias_ap[cc * R:(cc + 1) * R][None, :])
        b_t = sb.tile([1, R], BF16, tag="bia", bufs=2, name="b_rm")
        ve.tensor_copy(b_t[:], b_f[:])
        te.matmul(psm[:], ones_t[0:1, 0:P], b_t[:], start=False, stop=True)

    # ---- generic fm linear: one output ptile ------------------------
    def fm_group(w_tile, mt, rhs_slices, bias_ap=None, bias_mt=None):
        """psum[P, 512] = sum_kt w_tile[:, mt, kt, :].T @ rhs_slices[kt]."""
        psm = mm_ps()
        last = len(rhs_slices) - 1
        for kt, rhs in enumerate(rhs_slices):
            te.matmul(psm[:], w_tile[:, mt, kt, :], rhs,
                      start=(kt == 0), stop=(kt == last and bias_ap is None))
        if bias_ap is not None:
            bias_fm(psm, bias_ap, bias_mt if bias_mt is not None else mt, R)
        return psm

    # ================= layernorm (stats + normalize) =================
    # Stats are accumulated tile-by-tile as inputs appear (x2 mul + two
    # ones-matmuls); normalize uses PE broadcast of rstd / mu*rstd.
    class LNState:
        pass

    def ln_begin():
        st = LNState()
        st.sx = ps.tile([1, R], F32, tag="o", bufs=2, name="ln_sx")
        st.sx2 = ps.tile([1, R], F32, tag="o", bufs=2, name="ln_sx2")
        return st

    def ln_accum(st, x_sl, kt):
        # x^2 on the scalar engine — idle outside attention — so the sums
        # matmuls don't queue behind the DVE eviction chain
        x2 = sb.tile([P, R], BF16, tag="x2", bufs=2, name="x2")
        se.activation(out=x2[:], in_=x_sl, func=AF.Square)
        te.matmul(st.sx[:], ones_t[:, 0:1], x_sl,
                  start=(kt == 0), stop=(kt == FT - 1))
        te.matmul(st.sx2[:], ones_t[:, 0:1], x2[:],
                  start=(kt == 0), stop=(kt == FT - 1))

    def ln_stats(st):
        # mu is NEGATED so the apply step is x*rstd_b + (-mu*rstd)_b — a
        # commutative add with the PSUM operand in the proven first slot
        # mu/ex2/mu2 on the scalar engine (empty at the seams, and closer
        # to PSUM) so the stats chain isn't queued behind DVE evictions
        mu = stat_l()
        se.activation(out=mu[:], in_=st.sx[:], func=AF.Copy, scale=-1.0 / C)
        ex2 = stat_s()
        se.activation(out=ex2[:], in_=st.sx2[:], func=AF.Copy, scale=1.0 / C)
        mu2 = stat_s()
        se.activation(out=mu2[:], in_=mu[:], func=AF.Square)
        var = stat_s()
        ve.tensor_sub(var[:], ex2[:], mu2[:])
        # rstd = exp(-0.5 * ln(var + eps)) — keeps ACT on the ln/exp table set
        lnv = stat_s()
        se.activation(out=lnv[:], in_=var[:], func=AF.Ln,
                      bias=eps_t[0:1, 0:1], scale=1.0)
        rstd = stat_l(BF16)
        se.activation(out=rstd[:], in_=lnv[:], func=AF.Exp, scale=-0.5)
        # broadcast via PE into [P, R] (uses the idle "st" PSUM space);
        # applies read these straight from PSUM (PSUM-first operand)
        rb = ps.tile([P, R], F32, tag="st", bufs=2, name="ln_rb")
        te.matmul(rb[:], ones_t[0:1, 0:P], rstd[:], start=True, stop=True)
        # broadcast staging on ACT (idle here, PSUM-proximate) so the DVE
        # queue only carries musr + the applies at the seam
        rbs = sb.tile([P, R], BF16, tag="lnb", bufs=2, name="ln_rbs")
        se.activation(out=rbs[:], in_=rb[:], func=AF.Copy)
        musr = stat_l(BF16)
        ve.tensor_mul(musr[:], mu[:], rstd[:])
        mb = ps.tile([P, R], F32, tag="st", bufs=2, name="ln_mb")
        te.matmul(mb[:], ones_t[0:1, 0:P], musr[:], start=True, stop=True)
        mbs = sb.tile([P, R], BF16, tag="lnb", bufs=2, name="ln_mbs")
        se.activation(out=mbs[:], in_=mb[:], func=AF.Copy)
        st.rbs, st.mbs = rbs, mbs

    def ln_apply(st, x_sl, out_sl, w_ap, b_ap, kt, out_dt=BF16):
        tmp = sb.tile([P, R], BF16, tag="lntmp", bufs=2, name="lntmp")
        ve.tensor_mul(tmp[:], x_sl, st.rbs[:])
        if w_ap is None and b_ap is None:
            ve.tensor_add(out_sl, tmp[:], st.mbs[:])
        else:
            nrm = sb.tile([P, R], BF16, tag="lntmp", bufs=2, name="lnnrm")
            ve.tensor_add(nrm[:], tmp[:], st.mbs[:])
            w_t = sb.tile([P, 1], F32, tag="lnw", bufs=4, name="lnw")
            if w_ap is not None:
                dma(out=w_t[:], in_=w_ap[kt * P:(kt + 1) * P][:, None])
            else:
                ve.memset(w_t[:], 1.0)
            b_t = sb.tile([P, 1], F32, tag="lnw", bufs=4, name="lnb")
            if b_ap is not None:
                dma(out=b_t[:], in_=b_ap[kt * P:(kt + 1) * P][:, None])
            else:
                ve.memset(b_t[:], 0.0)
            ve.scalar_tensor_tensor(
                out_sl, nrm[:], w_t[:], b_t[:].to_broadcast((P, R)),
                op0=mybir.AluOpType.mult, op1=mybir.AluOpType.add)

    # =================================================================
    # Prologue DMAs (sp ring, in order of first use)
    # =================================================================
    # q-gen inputs first so the PE can start ~6us in (Wq arrives per-mt,
    # group 0 only needs the first slice); the rest stream underneath the
    # q/v projection work
    yq_all = yq_first                         # slot 0 (DMA'd above)
    wq_t = wfm_tile([P, FT, FT, P], "wq")     # wfm slot 0
    for mt in range(FT):
        dma(out=wq_t[:, mt, :, :], in_=d["Wq_attn"][:, mt, :, :])
    ykv_all = big_tile([P, FT, T], "ykv")     # big slot 0
    dma(out=ykv_all[:], in_=d["ykv"].rearrange("(kt p) r -> p kt r", p=P))
    wv_t = wrm_tile([P, FT, T], "wv")         # wrm slot 0
    dma(out=wv_t[:], in_=d["Wv_attn"][:])
    wk_t = wfm_tile([P, FT, FT, P], "wk")     # wfm slot 1
    dma(out=wk_t[:], in_=d["Wk_attn"][:])

    b_attn = d.get("b_attn")

    # ================= A: q generation (feature-major) ===============
    q_all = act8()                            # slot 1
    for mt in range(FT):
        psm = fm_group(wq_t, mt, [yq_all[:, kt, :] for kt in range(FT)],
                       bias_ap=b_attn, bias_mt=mt)
        ve.tensor_copy(q_all[:, mt, :], psm[:])

    # more prefetch: cross-attn inputs + proj weights
    xkv_all = big_tile([P, FT, T], "xkv")     # big slot 1
    dma(out=xkv_all[:], in_=d["xkv"].rearrange("(kt p) r -> p kt r", p=P))
    wven_t = wrm_tile([P, FT, T], "wven")     # wrm slot 1
    dma(out=wven_t[:], in_=d["Wv_en"][:])
    wproj_t = wfm_tile([P, FT, FT, P], "wproj")   # wfm slot 2
    dma(out=wproj_t[:], in_=d["W_proj_p"][:])

    # ================= B: v generation (row-major, + ones col) =======
    def gen_v(src_all, w_rm, v_tiles, bias_ap, pfx):
        for cc in range(2):
            for rt in range(RT):
                psm = mm_ps()
                for kt in range(FT):
                    te.matmul(psm[:],
                              src_all[:, kt, rt * P:(rt + 1) * P],
                              w_rm[:, kt, cc * R:(cc + 1) * R],
                              start=(kt == 0),
                              stop=(kt == FT - 1 and bias_ap is None))
                if bias_ap is not None:
                    bias_rm(psm, bias_ap, cc)
                if cc == 0:
                    v_tiles[rt] = sb.tile([P, H, HD + 1], BF16, tag="v65",
                                          bufs=RT, name=f"{pfx}{rt}")
                data = ve.tensor_copy(
                    v_tiles[rt][:, cc * 8:(cc + 1) * 8, 0:HD],
                    psm[:].rearrange("p (h d) -> p h d", d=HD))
                if cc == 0:
                    oc = ve.tensor_copy(v_tiles[rt][:, :, HD], ones_t[:, 0:H])
                    add_dep_helper(oc.ins, data.ins, sync=False,
                                   reason="ones col after v data (slot order)")

    v_sb = [None] * RT
    gen_v(ykv_all, wv_t,  v_sb,
          (d["b_attn"][2 * C:3 * C] if "b_attn" in d else None), "v")

    # ================= K-projection helper ===========================
    # k ptile hp covers heads (2hp, 2hp+1): features on partitions, T keys
    # free.  Emitted in 18 closures (16 matmuls + 2 evicts) so attention can
    # interleave them two-per-unit as PE filler under the exp stream.
    def k_steps(w_t, src_all, hp, bias_ap, box, pfx):
        steps = []
        psms = {}

        def mk_mm(cc, kt):
            def go():
                if kt == 0:
                    psms[cc] = mm_ps()
                te.matmul(psms[cc][:], w_t[:, hp, kt, :],
                          src_all[:, kt, cc * R:(cc + 1) * R],
                          start=(kt == 0),
                          stop=(kt == FT - 1 and bias_ap is None))
            return go

        def mk_ev(cc):
            def go():
                if bias_ap is not None:
                    bias_fm(psms[cc], bias_ap, hp, R)
                if cc == 0:
                    box[0] = sb.tile([P, T], BF16, tag="ksb", bufs=2,
                                     name=f"{pfx}{hp}")
                ve.tensor_copy(box[0][:, cc * R:(cc + 1) * R], psms[cc][:])
            return go

        for cc in range(2):
            for kt in range(FT):
                steps.append(mk_mm(cc, kt))
            steps.append(mk_ev(cc))
        return steps

    # ================= attention =====================================
    def attention(q_all_t, k_w, k_src, k_bias, v_tiles, o_all_t, pfx,
                  tail_filler):
        """tail_filler: list of closures run as PE filler during the last
        head-pair (which has no next-K to generate)."""

        def norm_head(o_ps):
            # DVE half of softmax normalization, emitted right after the
            # final O matmuls: reciprocal straight off the PSUM ones-row,
            # and the o halves staged into SBUF (so the later mul's PSUM
            # operand can be the PE-broadcast reciprocal instead)
            # ordering matched to each consumer's deadline: per-s den+oc
            # pairs free that o_ps slot for the next head-pair's O matmul
            # (unit 2); the reciprocal chains follow, landing just before
            # their rb matmuls (unit 3)
            rcp, ocp, dens = [], [], []
            for s in range(2):
                den = stat_s()
                ve.tensor_copy(den[:], o_ps[s][HD:HD + 1, :])
                dens.append(den)
                oc = sb.tile([HD, R], BF16, tag="rbs", bufs=2,
                             name=f"{pfx}ocp")
                ve.tensor_copy(oc[:], o_ps[s][0:HD, :])
                ocp.append(oc)
            for s in range(2):
                rc = stat_s()
                ve.reciprocal_approx_fast(rc[:], dens[s][:])
                rcb = stat_s(BF16)
                ve.tensor_copy(rcb[:], rc[:])
                rcp.append(rcb)
            return rcp, ocp

        def norm_tail(hp, rcp, ocp):
            # PE broadcast + final scale, hidden under the next head-pair's
            # S matmuls
            for s in range(2):
                rb = ps.tile([P, R], F32, tag="mm", bufs=2, name=f"{pfx}rb")
                te.matmul(rb[0:HD, :], ones_t[0:1, 0:HD], rcp[s][:],
                          start=True, stop=True)
                if s == 0:
                    ve.tensor_mul(o_all_t[0:HD, hp, :], rb[0:HD, :],
                                  ocp[s][:])
                else:
                    tmp = sb.tile([HD, R], BF16, tag="otmp", bufs=2,
                                  name=f"{pfx}otmp")
                    ve.tensor_mul(tmp[:], rb[0:HD, :], ocp[s][:])
                    gdma(out=o_all_t[HD:P, hp, :], in_=tmp[:])

        kbox = [None]
        for s0 in k_steps(k_w, k_src, 0, k_bias, kbox, pfx + "k"):
            s0()
        k_cur = kbox[0]
        pending = None          # (hp, o_ps) awaiting normalization
        for hp in range(FT):
            # next head-pair's K projection as filler
            if hp + 1 < FT:
                nbox = [None]
                filler = k_steps(k_w, k_src, hp + 1, k_bias, nbox, pfx + "k")
            else:
                nbox = None
                filler = list(tail_filler)
            o_ps = [None, None]
            es_q = [None] * RT
            fi = 0
            for u in range(RT + 3):
                if u < RT:
                    st_t = ps.tile([P, 2 * R], F32, tag="st", bufs=2,
                                   name=f"{pfx}st")
                    for s in range(2):
                        off = HD * s
                        te.matmul(st_t[:, s * R:(s + 1) * R],
                                  k_cur[off:off + HD, u * P:(u + 1) * P],
                                  q_all_t[off:off + HD, hp, :],
                                  start=True, stop=True)
                    es = sb.tile([P, 2 * R], BF16, tag="es", bufs=4,
                                 name=f"{pfx}es")
                    se.activation(out=es[:], in_=st_t[:], func=AF.Exp,
                                  scale=0.125)
                    es_q[u] = es
                if u >= 3:
                    tko = u - 3
                    if tko == 0:
                        o_ps[0] = ps.tile([HD + 1, R], F32, tag="o", bufs=2,
                                          name=f"{pfx}o0")
                        o_ps[1] = ps.tile([HD + 1, R], F32, tag="o", bufs=2,
                                          name=f"{pfx}o1")
                    for s in range(2):
                        te.matmul(o_ps[s][:],
                                  v_tiles[tko][:, 2 * hp + s, :],
                                  es_q[tko][:, s * R:(s + 1) * R],
                                  start=(tko == 0), stop=(tko == RT - 1))
                # filler pacing: one pop at u4/u9 so the K psum slot (tag
                # "mm") recycles cleanly around norm_tail's rb matmuls —
                # the cc0 evict lands at u4 just before rb, and the cc1
                # psum allocates at u5 just after the rb/mul pair
                for _ in range(1 if u in (4, RT + 1) else 2):
                    if fi < len(filler):
                        filler[fi]()
                        fi += 1
                if u == 4 and pending is not None:
                    # norm_tail here gets ~5 units of PE cover for the
                    # reciprocal chain emitted at the last head-pair's end
                    norm_tail(*pending)
                    pending = None
            while fi < len(filler):
                filler[fi]()
                fi += 1
            rcp, ocp = norm_head(o_ps)
            pending = (hp, rcp, ocp)
            if nbox is not None:
                k_cur = nbox[0]
        norm_tail(*pending)

    o_all = act8()                            # slot 2
    attention(q_all, wk_t, ykv_all,
              (d["b_attn"][C:2 * C] if "b_attn" in d else None),
              v_sb, o_all, "sa", [])

    # ================= proj + residual, LN1 stats interleaved ========
    # ln_accum lags its producer by one tile so the sums matmuls hide
    # under the next output tile's matmul group instead of stalling on
    # the 3-op DVE eviction chain
    y1_all = act8()                           # slot 3
    ln1 = ln_begin()
    for mt in range(FT):
        psm = fm_group(wproj_t, mt, [o_all[:, kt, :] for kt in range(FT)],
                       bias_ap=d.get("b_proj"), bias_mt=mt)
        # lagged ln_accum BEFORE this tile's eviction: its x2 isn't queued
        # behind the new residual add on the DVE
        if mt > 0:
            ln_accum(ln1, y1_all[:, mt - 1, :], mt - 1)
        ve.tensor_add(y1_all[:, mt, :], psm[:], yq_all[:, mt, :])
    ln_accum(ln1, y1_all[:, FT - 1, :], FT - 1)

    # prefetch cross-attn K weights (wfm slot 0: wq dead) and the q2/cproj
    # weights (slots 1/2: wk and wproj are dead once proj is emitted), so
    # the LN1 seam and cross-attn aren't gated on weight DMAs
    wken_t = wfm_tile([P, FT, FT, P], "wken")
    dma(out=wken_t[:], in_=d["Wk_en"][:])
    wq2_t = wfm_tile([P, FT, FT, P], "wq2")       # wfm slot 1 (wk dead)
    dma(out=wq2_t[:], in_=d["W_q_p"][:])
    wcproj_t = wfm_tile([P, FT, FT, P], "wcp")    # wfm slot 2 (wproj dead)
    dma(out=wcproj_t[:], in_=d["W_cproj_p"][:])

    # ================= v2 generation (PE) overlapping LN1 (DVE) ======
    v2_sb = [None] * RT
    gen_v(xkv_all, wven_t, v2_sb,
          (d["b_en"][C:2 * C] if "b_en" in d else None), "w")

    ln_stats(ln1)
    y1n_all = act8()                          # slot 4
    for mt in range(FT):
        ln_apply(ln1, y1_all[:, mt, :], y1n_all[:, mt, :],
                 d.get("ln_w"), d.get("ln_b"), mt)

    # ================= q2 generation =================================
    q2_all = act8()                           # slot 5
    for mt in range(FT):
        psm = fm_group(wq2_t, mt, [y1n_all[:, kt, :] for kt in range(FT)],
                       bias_ap=d.get("b_q"), bias_mt=mt)
        ve.tensor_copy(q2_all[:, mt, :], psm[:])

    # prefetch first two FFN d1 weight chunks under cross-attention
    wd1_t = [None] * 4
    for ch in range(2):
        wd1_t[ch] = wrm_tile([P, FT, FT, P], f"wd1_{ch}")
        dma(out=wd1_t[ch][:],
            in_=d["W_d1_p"][:, ch * FT:(ch + 1) * FT, :, :])

    # ================= cross attention ===============================
    o2_all = act8()                           # slot 0 (yq dead)
    attention(q2_all, wken_t, xkv_all,
              (d["b_en"][0:C] if "b_en" in d else None),
              v2_sb, o2_all, "ca", [])

    # ================= cproj + residual (into y1n), LN2 ==============
    ln2 = ln_begin()
    for mt in range(FT):
        psm = fm_group(wcproj_t, mt, [o2_all[:, kt, :] for kt in range(FT)],
                       bias_ap=d.get("b_cproj"), bias_mt=mt)
        if mt > 0:
            ln_accum(ln2, y1n_all[:, mt - 1, :], mt - 1)
        ve.tensor_add(y1n_all[:, mt, :], psm[:], y1n_all[:, mt, :])
    ln_accum(ln2, y1n_all[:, FT - 1, :], FT - 1)
    ln_stats(ln2)
    xin_all = act8()                          # slot 1 (q dead)
    for mt in range(FT):
        ln_apply(ln2, y1n_all[:, mt, :], xin_all[:, mt, :],
                 d.get("ln1_w"), d.get("ln1_b"), mt)

    # ================= FFN ===========================================
    # F1: h = xin @ W_d1 (feature-major, 32 output ptiles in 4 weight chunks)
    ht_a = big_tile([P, 16, R], "ht_a")       # big slot 0 (ykv dead)
    ht_b = big_tile([P, 16, R], "ht_b")       # big slot 1 (xkv dead)

    # first d2 weight tile prefetches under F1
    wd2_t = [None] * FT
    wd2_t[0] = wfm_tile([P, 32, P], "wd2_0")
    dma(out=wd2_t[0][:], in_=d["W_d2_p"][:, 0, :, :])

    def ht_sl(kt):
        return (ht_a if kt < 16 else ht_b)[:, kt % 16, :]

    for ch in range(4):
        if wd1_t[ch] is None:
            wd1_t[ch] = wrm_tile([P, FT, FT, P], f"wd1_{ch}")
            dma(out=wd1_t[ch][:],
                in_=d["W_d1_p"][:, ch * FT:(ch + 1) * FT, :, :])
        w1 = wd1_t[ch]
        # prefetch chunk ch+2 while computing on ch
        nxt = ch + 2
        if nxt < 4 and wd1_t[nxt] is None:
            wd1_t[nxt] = wrm_tile([P, FT, FT, P], f"wd1_{nxt}")
            dma(out=wd1_t[nxt][:],
                in_=d["W_d1_p"][:, nxt * FT:(nxt + 1) * FT, :, :])
        for kk in range(FT):
            kt = ch * FT + kk
            psm = mm_ps()
            for ck in range(FT):
                te.matmul(psm[:], w1[:, kk, ck, :], xin_all[:, ck, :],
                          start=(ck == 0),
                          stop=(ck == FT - 1 and "b_d1" not in d))
            if "b_d1" in d:
                bias_fm(psm, d["b_d1"], kt, R)
            ve.tensor_copy(ht_sl(kt), psm[:])

    # F2: z = h @ W_d2 + xin, LN3 stats interleaved
    ln3 = ln_begin()
    z_all = act8()                            # slot 2 (o dead)
    for mt in range(FT):
        if wd2_t[mt] is None:
            wd2_t[mt] = wfm_tile([P, 32, P], f"wd2_{mt}")
            dma(out=wd2_t[mt][:], in_=d["W_d2_p"][:, mt, :, :])
        w2 = wd2_t[mt]
        if mt + 1 < FT and wd2_t[mt + 1] is None:
            wd2_t[mt + 1] = wfm_tile([P, 32, P], f"wd2_{mt + 1}")
            dma(out=wd2_t[mt + 1][:], in_=d["W_d2_p"][:, mt + 1, :, :])
        psm = mm_ps()
        for kt in range(32):
            te.matmul(psm[:], w2[:, kt, :], ht_sl(kt),
                      start=(kt == 0), stop=(kt == 31 and "b_d2" not in d))
        if "b_d2" in d:
            bias_fm(psm, d["b_d2"], mt, R)
        if mt > 0:
            ln_accum(ln3, z_all[:, mt - 1, :], mt - 1)
        ve.tensor_add(z_all[:, mt, :], psm[:], xin_all[:, mt, :])
    ln_accum(ln3, z_all[:, FT - 1, :], FT - 1)
    ln_stats(ln3)

    for mt in range(FT):
        ot = sb.tile([P, R], BF16, tag="outt", bufs=2, name="out_t")
        ln_apply(ln3, z_all[:, mt, :], ot[:],
                 d.get("ln2_w"), d.get("ln2_b"), mt)
        dma(out=d["out"][mt * P:(mt + 1) * P, :], in_=ot[:])

    sb.release()
    ps.release()


def _fm_pack(W, n_out):
    """[C, n_out] -> [P, mt, kt, P] bf16 so lhsT tiles are DMA-contiguous."""
    W = np.asarray(W, np.float32)
    kt = W.shape[0] // P
    mt = n_out // P
    return np.ascontiguousarray(
        W.reshape(kt, P, mt, P).transpose(1, 2, 0, 3).astype(NPBF))


def _rm_pack(W):
    """[C, n_out] -> [P, kt, n_out] bf16 (rhs layout for row-major linears)."""
    W = np.asarray(W, np.float32)
    kt = W.shape[0] // P
    return np.ascontiguousarray(
        W.reshape(kt, P, W.shape[1]).transpose(1, 0, 2).astype(NPBF))


def _build(flags):
    nc = bacc.Bacc(trn_type="TRN2", target_bir_lowering=False, debug=False)
    d = {}

    def din(name, shape, dt=BF16):
        d[name] = nc.declare_dram_parameter(name, list(shape), dt,
                                            isOutput=False).ap()

    din("yq", (C, R))
    din("ykv", (C, T))
    din("xkv", (C, T))
    din("Wq_attn", (P, FT, FT, P))
    din("Wk_attn", (P, FT, FT, P))
    din("Wv_attn", (P, FT, T))
    din("W_proj_p", (P, FT, FT, P))
    din("Wk_en", (P, FT, FT, P))
    din("Wv_en", (P, FT, T))
    din("W_q_p", (P, FT, FT, P))
    din("W_cproj_p", (P, FT, FT, P))
    din("W_d1_p", (P, 32, FT, P))
    din("W_d2_p", (P, FT, 32, P))
    din("ones", (P, R))
    din("ones_f", (1, P), dt=mybir.dt.float32r)
    for nm, shape in (("b_attn", (3 * C,)), ("b_proj", (C,)), ("b_en", (2 * C,)),
                      ("b_q", (C,)), ("b_cproj", (C,)), ("b_d1", (DFF,)),
                      ("b_d2", (C,))):
        if nm in flags:
            din(nm, shape, dt=F32)
    for nm in ("ln_w", "ln_b", "ln1_w", "ln1_b", "ln2_w", "ln2_b"):
        if nm in flags:
            din(nm, (C,), dt=F32)
    d["out"] = nc.declare_dram_parameter("out", [C, R], BF16,
                                         isOutput=True).ap()

    with tile.TileContext(nc) as tc:
        _emit(nc, tc, d, flags)
    nc.compile()
    return nc


def _flags_of(inputs):
    flags = set()
    for nm in ("b_attn", "b_proj", "b_en", "b_q", "b_cproj", "b_d1", "b_d2"):
        if np.any(np.asarray(inputs[nm]) != 0):
            flags.add(nm)
    for nm, triv in (("ln_w", 1.0), ("ln_b", 0.0), ("ln1_w", 1.0),
                     ("ln1_b", 0.0), ("ln2_w", 1.0), ("ln2_b", 0.0)):
        if np.any(np.asarray(inputs[nm]) != triv):
            flags.add(nm)
    for a, b in (("ln_w", "ln_b"), ("ln1_w", "ln1_b"), ("ln2_w", "ln2_b")):
        if a in flags or b in flags:
            flags.add(a)
            flags.add(b)
    return flags


def _make_in_maps(inputs):
    flags = _flags_of(inputs)
    x = np.asarray(inputs["x"], np.float32)
    y = np.asarray(inputs["y"], np.float32)
    W_attn = np.asarray(inputs["W_attn"], np.float32)
    W_en = np.asarray(inputs["W_en"], np.float32)
    base = {
        "Wq_attn": _fm_pack(W_attn[:, 0:C], C),
        "Wk_attn": _fm_pack(W_attn[:, C:2 * C], C),
        "Wv_attn": _rm_pack(W_attn[:, 2 * C:3 * C]),
        "W_proj_p": _fm_pack(inputs["W_proj"], C),
        "Wk_en": _fm_pack(W_en[:, 0:C], C),
        "Wv_en": _rm_pack(W_en[:, C:2 * C]),
        "W_q_p": _fm_pack(inputs["W_q"], C),
        "W_cproj_p": _fm_pack(inputs["W_cproj"], C),
        "W_d1_p": np.ascontiguousarray(
            np.asarray(inputs["W_d1"], np.float32)
            .reshape(FT, P, 32, P).transpose(1, 2, 0, 3).astype(NPBF)),
        "W_d2_p": np.ascontiguousarray(
            np.asarray(inputs["W_d2"], np.float32)
            .reshape(32, P, FT, P).transpose(1, 2, 0, 3).astype(NPBF)),
        "ones": np.ones((P, R), NPBF),
        "ones_f": np.ones((1, P), np.float32),
    }
    for nm in flags:
        base[nm] = np.ascontiguousarray(np.asarray(inputs[nm], np.float32))
    yT = [np.ascontiguousarray(y[b].T.astype(NPBF)) for b in range(B)]
    xT = [np.ascontiguousarray(x[b].T.astype(NPBF)) for b in range(B)]
    in_maps = []
    for c in range(NCORES):
        b, h = divmod(c, 2)
        m = dict(base)
        m["ykv"] = yT[b]
        m["xkv"] = xT[b]
        m["yq"] = np.ascontiguousarray(yT[b][:, h * R:(h + 1) * R])
        in_maps.append(m)
    return in_maps


def kernel(x, y, W_attn, b_attn, W_proj, b_proj, ln_w, ln_b,
           W_en, b_en, W_q, b_q, W_cproj, b_cproj,
           ln1_w, ln1_b, ln2_w, ln2_b, W_d1, b_d1, W_d2, b_d2):
    inputs = dict(x=x, y=y, W_attn=W_attn, b_attn=b_attn, W_proj=W_proj,
                  b_proj=b_proj, ln_w=ln_w, ln_b=ln_b, W_en=W_en, b_en=b_en,
                  W_q=W_q, b_q=b_q, W_cproj=W_cproj, b_cproj=b_cproj,
                  ln1_w=ln1_w, ln1_b=ln1_b, ln2_w=ln2_w, ln2_b=ln2_b,
                  W_d1=W_d1, b_d1=b_d1, W_d2=W_d2, b_d2=b_d2)
    flags = _flags_of(inputs)
    key = tuple(sorted(flags))
    if key not in _CACHE:
        _CACHE[key] = _build(flags)
    nc = _CACHE[key]

    in_maps = _make_in_maps(inputs)
    res = run_bass_kernel_spmd(nc, in_maps, list(range(NCORES)))
    out = np.empty((B, T, C), np.float32)
    for c in range(NCORES):
        b, h = divmod(c, 2)
        out[b, h * R:(h + 1) * R, :] = \
            np.asarray(res.results[c]["out"], np.float32).T
    return out
